# revision 1
# baseline (speedup 1.0000x reference)
"""Trainium2 Bass kernel for BinderEnergyGuidance (retrieval_knn).

Math (per batch b of 16):
  d[b,n,m]   = ||binder[b,n] - target[m]||           (N=1024, M=8192)
  attract[b] = mean of the k=204 smallest per-row min-distances
  repel[b]   = sum relu(3 - d)^2
  out[b]     = 10*attract[b] + 5*repel[b]

Strategy: data-parallel over the batch axis, 2 batches per NeuronCore.
Per core:
  - TensorE: d2 = |x|^2 + |y|^2 - 2 x.y as ONE matmul per tile.  The
    K axis uses partition groups at 0/32/64 (compute engines can only
    start at partition multiples of 32; the gap rows are zeroed):
      k 0-2 :  lhsT x_k     . rhs -2*y_k
      k 32-34: lhsT x_k^2   . rhs 1
      k 64-66: lhsT 1       . rhs y_k^2
  - VectorE: tc = clamp(d2, 0, 9) (one tensor_scalar, PSUM->SBUF) and
    per-row min via tensor_reduce(min).  Clamping at 9 is exact:
    clash^2 = (3 - min(d,3))^2 needs no mask, and the 204th-smallest
    min-dist per batch is ~0.7 << 3, so clamped rows never enter top-k.
  - ScalarE: dc = sqrt(tc); Square(3 - dc) with fused per-row
    accumulation -> repel partial sums.
  - Top-k via rank selection: rank_i = #{j : v_j < v_i} with
    tensor_scalar(is_lt)+accumulate against a broadcast row of
    min-dists (PE transpose + DMA flatten + DMA partition-broadcast);
    select rank < 204, dot with v, partition-sum by matmul.

All DMA producers are funneled through V ops where a matmul consumes
them: PE matmul (S3_LW) carries at most 3 semaphore waits and every
DMA completion lands on a different rotating queue semaphore.

Self-contained: hardcodes shapes for binder[16,1024,3], target[8192,3].
"""

import numpy as np
from contextlib import ExitStack

import concourse.bass as bass
import concourse.bacc as bacc
import concourse.tile as tile
from concourse import mybir
from concourse.bass_utils import run_bass_kernel_spmd
from concourse.masks import make_identity

F32 = mybir.dt.float32
F32R = mybir.dt.float32r
AF = mybir.ActivationFunctionType
OP = mybir.AluOpType
AX = mybir.AxisListType

B, N, MT = 16, 1024, 8192
NCORES = 8
BC = B // NCORES          # batches per core
TOPK = 204                # int(0.2 * N)
CLASH = 3.0
CLASH2 = CLASH * CLASH
ATTRACT_SCALE, REPEL_SCALE = 10.0, 5.0

P = 128                   # SBUF partitions
NCHUNK = N // P           # 8 row-chunks per batch
MTILE = 1024              # PSUM tile free size (2 banks)
NMT = MT // MTILE         # m-tiles per row-chunk (8)
MMF = 512                 # fp32 matmul max moving free size
NHALF = 2                 # split M into halves for SBUF working buffers
MHALF = MT // NHALF       # 4096
JPH = NMT // NHALF        # m-tiles per half (4)
KP = 67                   # padded contraction size (groups at 0/32/64)

_prog_cache = {}


def build_program():
    nc = bacc.Bacc("TRN2", target_bir_lowering=False, debug=False,
                   num_devices=NCORES)
    bnd = nc.dram_tensor("bnd", [BC, 3, N], F32, kind="ExternalInput").ap()
    tgt = nc.dram_tensor("tgt", [3, MT], F32, kind="ExternalInput").ap()
    out = nc.dram_tensor("out", [BC, 1], F32, kind="ExternalOutput").ap()

    with tile.TileContext(nc) as tc, ExitStack() as ctx:
        consts = ctx.enter_context(tc.tile_pool(name="consts", bufs=1))
        work = ctx.enter_context(tc.tile_pool(name="work", bufs=1))
        tcp = ctx.enter_context(tc.tile_pool(name="tcp", bufs=2))
        psum = ctx.enter_context(tc.tile_pool(name="psum", bufs=3, space="PSUM"))
        psum2 = ctx.enter_context(tc.tile_pool(name="psum2", bufs=2, space="PSUM"))
        dpool = ctx.enter_context(tc.tile_pool(name="dpool", bufs=1, space="DRAM"))

        # --- build rhs_pad / lhsT_pad with V ops only (DMAs staged).
        # fp32r matmul operands must come from fp32r-rounding producers,
        # so zero/one fills go through copies/tensor_scalar, not memset. ---
        ys = consts.tile([3, MT], F32)
        nc.sync.dma_start(out=ys[:, :], in_=tgt[:, :])
        rhs_pad = consts.tile([KP, MT], F32R)
        lhsTs = [consts.tile([KP, N], F32R, name=f"lhsT_pad{b}")
                 for b in range(BC)]
        with tc.tile_pool(name="zscr", bufs=1) as zscr:
            zKP = zscr.tile([KP, MT], F32)
            nc.vector.memset(zKP[:, :], 0.0)
            nc.vector.tensor_copy(rhs_pad[:, :], zKP[:, :])
            nc.vector.tensor_scalar_mul(rhs_pad[0:3, :], ys[:, :], -2.0)
            nc.vector.tensor_mul(rhs_pad[64:67, :], ys[:, :], ys[:, :])
            nc.vector.tensor_scalar(rhs_pad[32:35, :], ys[:, :], 0.0, 1.0,
                                    OP.mult, OP.add)
            for b in range(BC):
                xs = consts.tile([3, N], F32, name=f"xs{b}")
                nc.sync.dma_start(out=xs[:, :], in_=bnd[b, :, :])
                lhsT_pad = lhsTs[b]
                nc.vector.tensor_copy(lhsT_pad[:, :], zKP[:, 0:N])
                nc.vector.tensor_copy(lhsT_pad[0:3, :], xs[:, :])
                nc.vector.tensor_mul(lhsT_pad[32:35, :], xs[:, :], xs[:, :])
                nc.vector.tensor_scalar(lhsT_pad[64:67, :], xs[:, :], 0.0, 1.0,
                                        OP.mult, OP.add)

        three1 = consts.tile([P, 1], F32)
        nc.vector.memset(three1, CLASH)
        ones128 = consts.tile([P, 1], F32)
        nc.vector.memset(ones128, 1.0)
        ident = consts.tile([P, P], F32)
        make_identity(nc, ident)

        waste_ts = work.tile([P, N], F32)    # rank pass elementwise out (unused)

        for b in range(BC):
            lhsT = lhsTs[b]
            mdB = work.tile([P, NCHUNK], F32, name=f"mdB{b}")      # min d2
            dcsB = work.tile([P, NCHUNK * NHALF], F32, name=f"dcsB{b}")
            stcB = work.tile([P, NCHUNK], F32, name=f"stcB{b}")

            for c in range(NCHUNK):
                lc = lhsT[:, c * P:(c + 1) * P]
                md8 = work.tile([P, NMT], F32, name="md8")
                stc8 = work.tile([P, NMT], F32, name="stc8")
                for h in range(NHALF):
                    tcl = tcp.tile([P, MHALF], F32, name="tcl", tag="tcl")
                    tcb = tcp.tile([P, MHALF], F32, name="tcb", tag="tcb")
                    for j4 in range(JPH):
                        j = h * JPH + j4
                        sl = slice(j4 * MTILE, (j4 + 1) * MTILE)
                        ps = psum.tile([P, MTILE], F32, name="ps", tag="ps")
                        for q in range(MTILE // MMF):
                            nc.tensor.matmul(
                                ps[:, q * MMF:(q + 1) * MMF], lc,
                                rhs_pad[:, j * MTILE + q * MMF:
                                        j * MTILE + (q + 1) * MMF],
                                start=True, stop=True)
                        # pass 1: lower clamp + fused true-min accumulate
                        nc.vector.tensor_scalar(
                            tcl[:, sl], ps[:, :], 0.0, 3.4e38,
                            OP.max, OP.min, accum_out=md8[:, j:j + 1])
                        # pass 2: upper clamp + fused sum accumulate (2x mode)
                        nc.vector.tensor_scalar(
                            tcb[:, sl], tcl[:, sl], CLASH2, 0.0,
                            OP.min, OP.add, accum_out=stc8[:, j:j + 1])
                    # sqrt of clamped d2; only the row-sum accumulator is used:
                    # repel_row = 9*M - 6*sum(dc) + sum(tc)
                    nc.scalar.activation(tcl, tcb, AF.Sqrt,
                                         accum_out=dcsB[:, c * NHALF + h:
                                                        c * NHALF + h + 1])
                nc.vector.tensor_reduce(mdB[:, c:c + 1], md8, AX.X, OP.min)
                nc.vector.tensor_reduce(stcB[:, c:c + 1], stc8, AX.X, OP.add)

            # ---- per-batch epilogue ----
            vB = work.tile([P, NCHUNK], F32, name=f"vB{b}")   # min dists
            nc.scalar.activation(vB, mdB, AF.Sqrt)

            # vT[c, q] = vB[q, c] (PE transpose), flatten to [1, N], then
            # partition-broadcast to [128, N] -- each step is one DMA so
            # downstream consumers wait on a single producer.
            vT = psum2.tile([NCHUNK, P], F32, name="vT", tag="ep")
            nc.tensor.transpose(vT, vB, ident)
            vTs = work.tile([NCHUNK, P], F32, name=f"vTs{b}")
            nc.scalar.copy(vTs, vT)
            vfl = dpool.tile([1, N], F32, name=f"vfl{b}")
            nc.sync.dma_start(
                out=vfl[0:1, :].rearrange("p (c q) -> p c q", c=NCHUNK),
                in_=vTs[:, :])
            vrep = work.tile([P, N], F32, name=f"vrep{b}")
            vfl_bcast = bass.AP(tensor=vfl.tensor, offset=vfl.offset,
                                ap=[[0, P], vfl.ap[-1]])
            nc.sync.dma_start(out=vrep[:, :], in_=vfl_bcast)

            rank8 = work.tile([P, NCHUNK], F32, name=f"rank8{b}")
            for c in range(NCHUNK):
                nc.vector.tensor_scalar(waste_ts, vrep, vB[:, c:c + 1], 0.0,
                                        OP.is_lt, OP.add,
                                        accum_out=rank8[:, c:c + 1])
            sel8 = work.tile([P, NCHUNK], F32, name=f"sel8{b}")
            nc.vector.tensor_scalar(sel8, rank8, float(TOPK), None, OP.is_lt)

            stack2 = work.tile([P, 2], F32, name=f"stack2{b}")
            prod8 = work.tile([P, NCHUNK], F32, name=f"prod8{b}")
            nc.vector.tensor_mul(prod8, sel8, vB)
            nc.vector.tensor_reduce(stack2[:, 0:1], prod8, AX.X, OP.add)
            # per-row repel: 9*M - 6*sum(dc) + sum(tc)
            tdc = work.tile([P, 1], F32, name=f"tdc{b}")
            nc.vector.tensor_reduce(tdc, dcsB, AX.X, OP.add)
            tst = work.tile([P, 1], F32, name=f"tst{b}")
            nc.vector.tensor_reduce(tst, stcB, AX.X, OP.add)
            tdc2 = work.tile([P, 1], F32, name=f"tdc2{b}")
            nc.vector.tensor_scalar(tdc2, tdc, -6.0, float(9 * MT * NCHUNK),
                                    OP.mult, OP.add)
            nc.vector.tensor_add(stack2[:, 1:2], tdc2, tst)

            fin = psum2.tile([1, 2], F32, name="fin", tag="ep")
            nc.tensor.matmul(fin, ones128, stack2, start=True, stop=True)
            en = work.tile([1, 2], F32, name=f"en{b}")
            nc.vector.tensor_scalar_mul(en[0:1, 0:1], fin[0:1, 0:1],
                                        ATTRACT_SCALE / TOPK)
            nc.vector.tensor_scalar_mul(en[0:1, 1:2], fin[0:1, 1:2],
                                        REPEL_SCALE)
            en2 = work.tile([1, 1], F32, name=f"en2{b}")
            nc.vector.tensor_add(en2, en[0:1, 0:1], en[0:1, 1:2])
            nc.sync.dma_start(out=out[b:b + 1, 0:1], in_=en2[0:1, 0:1])

    nc.compile()
    return nc


def _get_program():
    if "nc" not in _prog_cache:
        _prog_cache["nc"] = build_program()
    return _prog_cache["nc"]


def make_in_maps(binder_trans, target_coords):
    x = np.ascontiguousarray(
        np.asarray(binder_trans, dtype=np.float32).transpose(0, 2, 1))
    y = np.ascontiguousarray(np.asarray(target_coords, dtype=np.float32).T)
    return [{"bnd": np.ascontiguousarray(x[c * BC:(c + 1) * BC]), "tgt": y}
            for c in range(NCORES)]


def kernel(binder_trans, target_coords):
    nc = _get_program()
    in_maps = make_in_maps(binder_trans, target_coords)
    res = run_bass_kernel_spmd(nc, in_maps, list(range(NCORES)))
    outs = [np.asarray(res.results[c]["out"], dtype=np.float32).reshape(BC)
            for c in range(NCORES)]
    return np.concatenate(outs).astype(np.float32)



# revision 2
# speedup vs baseline: 1.4674x; 1.4674x over previous
"""Trainium2 Bass kernel for BinderEnergyGuidance (retrieval_knn).

Math (per batch b of 16):
  d[b,n,m]   = ||binder[b,n] - target[m]||           (N=1024, M=8192)
  attract[b] = mean of the k=204 smallest per-row min-distances
  repel[b]   = sum relu(3 - d)^2
  out[b]     = 10*attract[b] + 5*repel[b]

Strategy: data-parallel over batch, 2 batches per core.  Host packs the
inputs so the device contraction is K=5:
  lhsT rows: [x0 x1 x2 ; ||x||^2 ; 1]        (per batch, [5, N])
  rhs  rows: [-2y0 -2y1 -2y2 ; 1 ; sum(y^2)+eps]   ([5, M])
so one fp32r matmul emits d2 + eps directly into PSUM.

Per 2048-wide m-tile (64 per core):
  PE : 4x 512-col matmuls -> PSUM fp32 d2 (2 PSUM tiles = all 8 banks)
  Act: dc = Sqrt(d2) PSUM->SBUF bf16 (eps pre-added keeps d2 > 0)
  DVE tsA: w = min(dc,3), accum(min) -> per-row min dist (attract).
  repel integrand (3-w)^2, summed per row, via 3 balanced routes:
    'A': Act Square(w - 3) + accum          (Act only)
    'D': vc = w-3 (ts); vc*vc (tt); ts accum (DVE only)
    'P': vc = w-3 (ts); vc*vc on gpsimd tt; ts accum (Pool + DVE)
  Route counts chosen so Act/DVE/Pool all land at ~140us busy.

Note: tensor_scalar with accum_out applies only op0 to the written
output; op1 becomes the accumulation op.  All op1 choices here are
no-ops on the output.

Epilogue (per batch): rowmins [128,8] -> +3 -> bf16 pad to [128,128] ->
XBAR DMA transpose -> flatten -> gpsimd partition_broadcast -> top-k by
rank (count of strictly-smaller values) -> select/dot; final partition
sums via gpsimd partition_all_reduce (no PSUM use outside the matmuls).

Self-contained: hardcodes shapes for binder[16,1024,3], target[8192,3].
"""

import numpy as np
from contextlib import ExitStack

import concourse.bass as bass
import concourse.bacc as bacc
import concourse.tile as tile
from concourse import mybir, bass_isa
from concourse.bass_utils import run_bass_kernel_spmd

F32 = mybir.dt.float32
F32R = mybir.dt.float32r
BF16 = mybir.dt.bfloat16
AF = mybir.ActivationFunctionType
OP = mybir.AluOpType
AX = mybir.AxisListType
RED = bass_isa.ReduceOp

B, N, MT = 16, 1024, 8192
NCORES = 8
BC = B // NCORES          # batches per core
TOPK = 204                # int(0.2 * N)
CLASH = 3.0
EPS = 1e-3                # guards sqrt against fp-rounding-negative d2
ATTRACT_SCALE, REPEL_SCALE = 10.0, 5.0

P = 128                   # SBUF partitions
NCHUNK = N // P           # 8 row-chunks per batch
MTILE = 2048              # PSUM tile free size (4 banks)
JPC = MT // MTILE         # m-tiles per chunk (4)
MMF = 512                 # fp32 matmul max moving free size
KP = 5                    # packed contraction size

# square-route assignment per m-tile index k (64 per core):
# 'P' gpsimd, 'A' scalar-engine Square, 'D' vector engine
N_POOL, N_ACT = 32, 10


def _routes():
    r = []
    for k in range(64):
        if k % 2 == 0 and r.count("P") < N_POOL:
            r.append("P")
        elif (k % 8 == 1 or k % 16 == 3) and r.count("A") < N_ACT:
            r.append("A")
        else:
            r.append("D")
    return r


ROUTES = _routes()

_prog_cache = {}


def build_program():
    nc = bacc.Bacc("TRN2", target_bir_lowering=False, debug=False,
                   num_devices=NCORES)
    bnd = nc.dram_tensor("bnd", [BC, KP, N], F32R, kind="ExternalInput").ap()
    tgt = nc.dram_tensor("tgt", [KP, MT], F32R, kind="ExternalInput").ap()
    out = nc.dram_tensor("out", [BC, 1], F32, kind="ExternalOutput").ap()

    with tile.TileContext(nc) as tc, ExitStack() as ctx:
        consts = ctx.enter_context(tc.tile_pool(name="consts", bufs=1))
        work = ctx.enter_context(tc.tile_pool(name="work", bufs=1))
        dcp = ctx.enter_context(tc.tile_pool(name="dcp", bufs=6))
        wp = ctx.enter_context(tc.tile_pool(name="wp", bufs=6))
        vcd = ctx.enter_context(tc.tile_pool(name="vcd", bufs=3))
        vcp = ctx.enter_context(tc.tile_pool(name="vcp", bufs=8))
        wap = ctx.enter_context(tc.tile_pool(name="wap", bufs=3))
        v2d = ctx.enter_context(tc.tile_pool(name="v2d", bufs=3))
        v2pp = ctx.enter_context(tc.tile_pool(name="v2pp", bufs=6))
        psum = ctx.enter_context(tc.tile_pool(name="psum", bufs=2, space="PSUM"))

        # ---- inputs: pure DMA, no engine preamble.  lhsT0 + first rhs
        # chunk gate the first matmul, so issue them first on separate
        # queues. ----
        rhs = consts.tile([KP, MT], F32R)
        lhsTs = [consts.tile([KP, N], F32R, name=f"lhsT{b}") for b in range(BC)]
        nc.sync.dma_start(out=lhsTs[0][:, :], in_=bnd[0, :, :])
        nc.scalar.dma_start(out=rhs[:, 0:2048], in_=tgt[:, 0:2048])
        nc.sync.dma_start(out=rhs[:, 2048:4096], in_=tgt[:, 2048:4096])
        nc.scalar.dma_start(out=rhs[:, 4096:6144], in_=tgt[:, 4096:6144])
        nc.sync.dma_start(out=rhs[:, 6144:8192], in_=tgt[:, 6144:8192])
        nc.scalar.dma_start(out=lhsTs[1][:, :], in_=bnd[1, :, :])

        # shared elementwise-output scratch (values never read)
        waste1k = work.tile([P, N], BF16)
        neg3 = consts.tile([P, 1], F32)
        nc.vector.memset(neg3, -CLASH)

        # Engine queues are strict FIFO, so an op that waits on a slow
        # producer (gpsimd square, epilogue DMA chain) must not be emitted
        # right after it or it head-of-line-blocks the whole engine.
        # Deferred emission queues provide the lag.
        pend_sum = []     # P-route row-sum ts ops waiting for gpsimd
        pend_act = []     # A-route Square ops, lag one tile
        pend_epi = []     # previous batch's epilogue part 2

        def flush(q, keep=0):
            while len(q) > keep:
                q.pop(0)()

        for b in range(BC):
            lhsT = lhsTs[b]
            sw2B = work.tile([P, NCHUNK * JPC], F32, name=f"sw2B{b}")
            mnB = work.tile([P, NCHUNK * JPC], F32, name=f"mnB{b}")

            for c in range(NCHUNK):
                lc = lhsT[:, c * P:(c + 1) * P]
                for j in range(JPC):
                    k = c * JPC + j
                    route = ROUTES[k]
                    ps = psum.tile([P, MTILE], F32, name="ps", tag="ps")
                    for q in range(MTILE // MMF):
                        nc.tensor.matmul(
                            ps[:, q * MMF:(q + 1) * MMF], lc,
                            rhs[:, j * MTILE + q * MMF:
                                j * MTILE + (q + 1) * MMF],
                            start=True, stop=True)
                    dc = dcp.tile([P, MTILE], BF16, name="dc", tag="dc")
                    nc.scalar.activation(dc, ps, AF.Sqrt)
                    # w = min(dc, 3); accum(min) -> rowmin (attract)
                    w = wp.tile([P, MTILE], BF16, name="w", tag="w")
                    nc.vector.tensor_scalar(
                        w, dc, CLASH, 3.4e38, OP.min, OP.min,
                        accum_out=mnB[:, k:k + 1])
                    if route == "A":
                        def mk_act(w=w, sw2B=sw2B, k=k):
                            def go():
                                wa = wap.tile([P, MTILE], BF16, name="wa",
                                              tag="wa")
                                nc.scalar.activation(
                                    wa, w, AF.Square, bias=neg3[:, 0:1],
                                    scale=1.0, accum_out=sw2B[:, k:k + 1])
                            return go
                        pend_act.append(mk_act())
                    else:
                        # vc = w - 3 = -relu(3-d)
                        pool_ = vcp if route == "P" else vcd
                        vc = pool_.tile([P, MTILE], BF16, name="vc", tag="vc")
                        nc.vector.tensor_scalar(
                            vc, w, CLASH, 0.0, OP.subtract, OP.min)
                        if route == "P":
                            v2 = v2pp.tile([P, MTILE], BF16, name="v2",
                                           tag="v2")
                            nc.gpsimd.tensor_tensor(v2, vc, vc, OP.mult)
                            def mk_sum(v2=v2, sw2B=sw2B, k=k):
                                def go():
                                    nc.vector.tensor_scalar(
                                        v2, v2, 3.4e38, 0.0, OP.min, OP.add,
                                        accum_out=sw2B[:, k:k + 1])
                                return go
                            pend_sum.append(mk_sum())
                        else:
                            v2 = v2d.tile([P, MTILE], BF16, name="v2",
                                          tag="v2")
                            nc.vector.tensor_tensor(v2, vc, vc, OP.mult)
                            nc.vector.tensor_scalar(
                                v2, v2, 3.4e38, 0.0, OP.min, OP.add,
                                accum_out=sw2B[:, k:k + 1])
                    flush(pend_act, keep=3)
                    flush(pend_sum, keep=6)
                if c == 1:
                    flush(pend_epi)
            flush(pend_act)
            flush(pend_sum)

            # ---- per-batch epilogue, part 1: reduces + the transpose/
            # broadcast DMA chain (off the compute engines) ----
            vBm = work.tile([P, NCHUNK], F32, name=f"vBm{b}")
            nc.vector.tensor_reduce(
                vBm, mnB.rearrange("p (c j) -> p c j", c=NCHUNK), AX.X, OP.min)
            stack2 = work.tile([P, 2], F32, name=f"stack2{b}")
            nc.vector.tensor_reduce(stack2[:, 1:2], sw2B, AX.X, OP.add)
            vBb = work.tile([P, P], BF16, name=f"vBb{b}")
            nc.vector.memset(vBb, CLASH)
            nc.vector.tensor_copy(vBb[:, 0:NCHUNK], vBm)
            vT = work.tile([P, P], BF16, name=f"vT{b}")
            nc.sync.dma_start(out=vT, in_=vBb, transpose=True)
            vfl = work.tile([1, N], BF16, name=f"vfl{b}")
            nc.sync.dma_start(
                out=vfl[0:1, :].rearrange("p (c q) -> p c q", c=NCHUNK),
                in_=vT[0:NCHUNK, :])
            vrep = work.tile([P, N], BF16, name=f"vrep{b}")
            nc.gpsimd.partition_broadcast(vrep, vfl[0:1, :], P)

            # part 2: rank selection + final combine; deferred into the
            # next batch's main loop so the DMA latency above is hidden
            def mk_epi(b=b, vBm=vBm, vrep=vrep, stack2=stack2):
                def go():
                    rank8 = work.tile([P, NCHUNK], F32, name=f"rank8{b}")
                    for c in range(NCHUNK):
                        nc.vector.tensor_scalar(
                            waste1k, vrep, vBm[:, c:c + 1], 0.0,
                            OP.is_lt, OP.add, accum_out=rank8[:, c:c + 1])
                    sel8 = work.tile([P, NCHUNK], F32, name=f"sel8{b}")
                    nc.vector.tensor_scalar(sel8, rank8, float(TOPK), None,
                                            OP.is_lt)
                    prod8 = work.tile([P, NCHUNK], F32, name=f"prod8{b}")
                    nc.vector.tensor_mul(prod8, sel8, vBm)
                    # sum of selected (rowmin-3); add 3 per selected row
                    # via the count
                    nc.vector.tensor_reduce(stack2[:, 0:1], prod8, AX.X,
                                            OP.add)
                    cnt8 = work.tile([P, 1], F32, name=f"cnt8{b}")
                    nc.vector.tensor_reduce(cnt8, sel8, AX.X, OP.add)
                    fin = work.tile([P, 2], F32, name=f"fin{b}")
                    nc.gpsimd.partition_all_reduce(fin, stack2, P, RED.add)
                    fcnt = work.tile([P, 1], F32, name=f"fcnt{b}")
                    nc.gpsimd.partition_all_reduce(fcnt, cnt8, P, RED.add)
                    en = work.tile([1, 2], F32, name=f"en{b}")
                    nc.vector.tensor_scalar(
                        en[0:1, 0:1], fcnt[0:1, 0:1],
                        CLASH * ATTRACT_SCALE / TOPK, None, OP.mult)
                    t0 = work.tile([1, 1], F32, name=f"t0{b}")
                    nc.vector.tensor_scalar(t0, fin[0:1, 0:1],
                                            ATTRACT_SCALE / TOPK, None,
                                            OP.mult)
                    nc.vector.tensor_scalar(en[0:1, 1:2], fin[0:1, 1:2],
                                            REPEL_SCALE, None, OP.mult)
                    en2 = work.tile([1, 1], F32, name=f"en2{b}")
                    nc.vector.tensor_add(en2, en[0:1, 0:1], en[0:1, 1:2])
                    en3 = work.tile([1, 1], F32, name=f"en3{b}")
                    nc.vector.tensor_add(en3, en2, t0)
                    nc.sync.dma_start(out=out[b:b + 1, 0:1],
                                      in_=en3[0:1, 0:1])
                return go
            pend_epi.append(mk_epi())
        flush(pend_epi)

    nc.compile()
    return nc


def _get_program():
    if "nc" not in _prog_cache:
        _prog_cache["nc"] = build_program()
    return _prog_cache["nc"]


def make_in_maps(binder_trans, target_coords):
    x = np.asarray(binder_trans, dtype=np.float32)          # [B, N, 3]
    y = np.asarray(target_coords, dtype=np.float32)         # [M, 3]
    # lhsT per batch: [x0 x1 x2 ; ||x||^2 ; 1]  -> [B, 5, N]
    xT = x.transpose(0, 2, 1)                               # [B, 3, N]
    xsq = (x * x).sum(-1)[:, None, :]                       # [B, 1, N]
    ones_n = np.ones((B, 1, N), dtype=np.float32)
    lhs = np.ascontiguousarray(
        np.concatenate([xT, xsq, ones_n], axis=1))          # [B, 5, N]
    # rhs: [-2y0 -2y1 -2y2 ; 1 ; sum(y^2)+eps] -> [5, M]
    yT = -2.0 * y.T                                         # [3, M]
    ones_m = np.ones((1, MT), dtype=np.float32)
    ysq = (y * y).sum(-1)[None, :] + np.float32(EPS)        # [1, M]
    rhs = np.ascontiguousarray(
        np.concatenate([yT, ones_m, ysq], axis=0))          # [5, M]
    return [{"bnd": np.ascontiguousarray(lhs[c * BC:(c + 1) * BC]),
             "tgt": rhs}
            for c in range(NCORES)]


def kernel(binder_trans, target_coords):
    nc = _get_program()
    in_maps = make_in_maps(binder_trans, target_coords)
    res = run_bass_kernel_spmd(nc, in_maps, list(range(NCORES)))
    outs = [np.asarray(res.results[c]["out"], dtype=np.float32).reshape(BC)
            for c in range(NCORES)]
    return np.concatenate(outs).astype(np.float32)


# revision 3
# speedup vs baseline: 1.7128x; 1.1672x over previous
"""Trainium2 Bass kernel for BinderEnergyGuidance (retrieval_knn).

Math (per batch b of 16):
  d[b,n,m]   = ||binder[b,n] - target[m]||           (N=1024, M=8192)
  attract[b] = mean of the k=204 smallest per-row min-distances
  repel[b]   = sum relu(3 - d)^2
  out[b]     = 10*attract[b] + 5*repel[b]

Strategy: data-parallel over batch, 2 batches per core.  Host packs the
inputs so the device contraction is K=5:
  lhsT rows: [x0 x1 x2 ; ||x||^2 ; 1]        (per batch, [5, N])
  rhs  rows: [-2y0 -2y1 -2y2 ; 1 ; sum(y^2)+eps]   ([5, M])
so one fp32r matmul emits d2 + eps directly into PSUM.

Per 2048-wide m-tile (64 per core):
  PE : 4x 512-col matmuls -> PSUM fp32 d2 (2 PSUM tiles = all 8 banks)
  Act: dc = Sqrt(d2) PSUM->SBUF bf16 (eps pre-added keeps d2 > 0)
  DVE tsA: w = min(dc,3), accum(min) -> per-row min dist (attract).
  repel integrand (3-w)^2, summed per row, via 3 balanced routes:
    'A': Act Square(w - 3) + accum          (Act only)
    'D': vc = w-3 (ts); vc*vc (tt); ts accum (DVE only)
    'P': vc = w-3 (ts); vc*vc on gpsimd tt; ts accum (Pool + DVE)
  Route counts chosen so Act/DVE/Pool all land at ~140us busy.

Note: tensor_scalar with accum_out applies only op0 to the written
output; op1 becomes the accumulation op.  All op1 choices here are
no-ops on the output.

Epilogue (per batch): rowmins [128,8] -> +3 -> bf16 pad to [128,128] ->
XBAR DMA transpose -> flatten -> gpsimd partition_broadcast -> top-k by
rank (count of strictly-smaller values) -> select/dot; final partition
sums via gpsimd partition_all_reduce (no PSUM use outside the matmuls).

Self-contained: hardcodes shapes for binder[16,1024,3], target[8192,3].
"""

import numpy as np
from contextlib import ExitStack

import concourse.bass as bass
import concourse.bacc as bacc
import concourse.tile as tile
from concourse import mybir, bass_isa
from concourse.bass_utils import run_bass_kernel_spmd

F32 = mybir.dt.float32
F32R = mybir.dt.float32r
BF16 = mybir.dt.bfloat16
AF = mybir.ActivationFunctionType
OP = mybir.AluOpType
AX = mybir.AxisListType
RED = bass_isa.ReduceOp

B, N, MT = 16, 1024, 8192
NCORES = 8
BC = B // NCORES          # batches per core
TOPK = 204                # int(0.2 * N)
CLASH = 3.0
EPS = 1e-3                # guards sqrt against fp-rounding-negative d2
ATTRACT_SCALE, REPEL_SCALE = 10.0, 5.0

P = 128                   # SBUF partitions
NCHUNK = N // P           # 8 row-chunks per batch
MTILE = 2048              # PSUM tile free size (4 banks)
JPC = MT // MTILE         # m-tiles per chunk (4)
MMF = 512                 # fp32 matmul max moving free size
KP = 5                    # packed contraction size

# square-route assignment per (batch, m-tile) — 32 tiles per batch.
# 'P' gpsimd, 'A' scalar-engine Square, 'D' vector engine.
# Batch 1 keeps its last chunks gpsimd-free so Pool drains before the
# final epilogue instead of 25us after the last sqrt.


def _routes():
    b0 = []
    for k in range(32):
        if k % 2 == 0:
            b0.append("P")
        elif k % 8 == 1:
            b0.append("A")
        else:
            b0.append("D")
    b1 = []
    for k in range(32):
        if k < 24:
            b1.append("P" if k % 3 != 2 else "D")
        elif k < 30:
            b1.append("A")
        else:
            b1.append("D")
    return [b0, b1]


ROUTES = _routes()

_prog_cache = {}


def build_program():
    nc = bacc.Bacc("TRN2", target_bir_lowering=False, debug=False,
                   num_devices=NCORES)
    bnd = nc.dram_tensor("bnd", [BC, KP, N], F32R, kind="ExternalInput").ap()
    tgt = nc.dram_tensor("tgt", [KP, MT], F32R, kind="ExternalInput").ap()
    out = nc.dram_tensor("out", [BC, 1], F32, kind="ExternalOutput").ap()

    with tile.TileContext(nc) as tc, ExitStack() as ctx:
        consts = ctx.enter_context(tc.tile_pool(name="consts", bufs=1))
        work = ctx.enter_context(tc.tile_pool(name="work", bufs=1))
        dcp = ctx.enter_context(tc.tile_pool(name="dcp", bufs=6))
        wp = ctx.enter_context(tc.tile_pool(name="wp", bufs=6))
        vcd = ctx.enter_context(tc.tile_pool(name="vcd", bufs=3))
        vcp = ctx.enter_context(tc.tile_pool(name="vcp", bufs=8))
        wap = ctx.enter_context(tc.tile_pool(name="wap", bufs=3))
        v2d = ctx.enter_context(tc.tile_pool(name="v2d", bufs=3))
        v2pp = ctx.enter_context(tc.tile_pool(name="v2pp", bufs=6))
        psum = ctx.enter_context(tc.tile_pool(name="psum", bufs=2, space="PSUM"))
        dpool = ctx.enter_context(tc.tile_pool(name="dpool", bufs=1, space="DRAM"))

        # ---- inputs: pure DMA, no engine preamble.  lhsT0 + first rhs
        # chunk gate the first matmul, so issue them first on separate
        # queues. ----
        rhs = consts.tile([KP, MT], F32R)
        lhsTs = [consts.tile([KP, N], F32R, name=f"lhsT{b}") for b in range(BC)]
        nc.sync.dma_start(out=lhsTs[0][:, :], in_=bnd[0, :, :])
        nc.scalar.dma_start(out=rhs[:, 0:512], in_=tgt[:, 0:512])
        nc.sync.dma_start(out=rhs[:, 512:2048], in_=tgt[:, 512:2048])
        nc.scalar.dma_start(out=rhs[:, 2048:4096], in_=tgt[:, 2048:4096])
        nc.sync.dma_start(out=rhs[:, 4096:6144], in_=tgt[:, 4096:6144])
        nc.scalar.dma_start(out=rhs[:, 6144:8192], in_=tgt[:, 6144:8192])
        nc.sync.dma_start(out=lhsTs[1][:, :], in_=bnd[1, :, :])

        # shared elementwise-output scratch (values never read)
        waste1k = work.tile([P, N], BF16)
        neg3 = consts.tile([P, 1], F32)
        nc.vector.memset(neg3, -CLASH)
        ones128 = consts.tile([P, 1], F32)
        nc.vector.memset(ones128, 1.0)

        # Engine queues are strict FIFO, so an op that waits on a slow
        # producer (gpsimd square, epilogue DMA chain) must not be emitted
        # right after it or it head-of-line-blocks the whole engine.
        # Deferred emission queues provide the lag.
        pend_sum = []     # P-route row-sum ts ops waiting for gpsimd
        pend_act = []     # A-route Square ops, lag one tile
        pend_epi = []     # previous batch's epilogue part 2

        def flush(q, keep=0):
            while len(q) > keep:
                q.pop(0)()

        for b in range(BC):
            lhsT = lhsTs[b]
            sw2B = work.tile([P, NCHUNK * JPC], F32, name=f"sw2B{b}")
            mnB = work.tile([P, NCHUNK * JPC], F32, name=f"mnB{b}")

            for c in range(NCHUNK):
                lc = lhsT[:, c * P:(c + 1) * P]
                for j in range(JPC):
                    k = c * JPC + j
                    route = ROUTES[b][k]
                    ps = psum.tile([P, MTILE], F32, name="ps", tag="ps")
                    for q in range(MTILE // MMF):
                        nc.tensor.matmul(
                            ps[:, q * MMF:(q + 1) * MMF], lc,
                            rhs[:, j * MTILE + q * MMF:
                                j * MTILE + (q + 1) * MMF],
                            start=True, stop=True)
                    dc = dcp.tile([P, MTILE], BF16, name="dc", tag="dc")
                    nc.scalar.activation(dc, ps, AF.Sqrt)
                    # w = min(dc, 3); accum(min) -> rowmin (attract)
                    w = wp.tile([P, MTILE], BF16, name="w", tag="w")
                    nc.vector.tensor_scalar(
                        w, dc, CLASH, 3.4e38, OP.min, OP.min,
                        accum_out=mnB[:, k:k + 1])
                    if route == "A":
                        def mk_act(w=w, sw2B=sw2B, k=k):
                            def go():
                                wa = wap.tile([P, MTILE], BF16, name="wa",
                                              tag="wa")
                                nc.scalar.activation(
                                    wa, w, AF.Square, bias=neg3[:, 0:1],
                                    scale=1.0, accum_out=sw2B[:, k:k + 1])
                            return go
                        pend_act.append(mk_act())
                    else:
                        # vc = w - 3 = -relu(3-d)
                        pool_ = vcp if route == "P" else vcd
                        vc = pool_.tile([P, MTILE], BF16, name="vc", tag="vc")
                        nc.vector.tensor_scalar(
                            vc, w, CLASH, 0.0, OP.subtract, OP.min)
                        if route == "P":
                            v2 = v2pp.tile([P, MTILE], BF16, name="v2",
                                           tag="v2")
                            nc.gpsimd.tensor_tensor(v2, vc, vc, OP.mult)
                            def mk_sum(v2=v2, sw2B=sw2B, k=k):
                                def go():
                                    nc.vector.tensor_scalar(
                                        v2, v2, 3.4e38, 0.0, OP.min, OP.add,
                                        accum_out=sw2B[:, k:k + 1])
                                return go
                            pend_sum.append(mk_sum())
                        else:
                            v2 = v2d.tile([P, MTILE], BF16, name="v2",
                                          tag="v2")
                            nc.vector.tensor_tensor(v2, vc, vc, OP.mult)
                            nc.vector.tensor_scalar(
                                v2, v2, 3.4e38, 0.0, OP.min, OP.add,
                                accum_out=sw2B[:, k:k + 1])
                    flush(pend_act, keep=3)
                    flush(pend_sum, keep=6)
                if c == 1:
                    flush(pend_epi)
            flush(pend_act)
            flush(pend_sum)

            # ---- per-batch epilogue, part 1: reduces + the transpose/
            # broadcast DMA chain (off the compute engines) ----
            vBm = work.tile([P, NCHUNK], F32, name=f"vBm{b}")
            nc.vector.tensor_reduce(
                vBm, mnB.rearrange("p (c j) -> p c j", c=NCHUNK), AX.X, OP.min)
            stack2 = work.tile([P, 2], F32, name=f"stack2{b}")
            nc.vector.tensor_reduce(stack2[:, 1:2], sw2B, AX.X, OP.add)
            # rank selection ignores element order, so flatten [128, 8]
            # partition-major straight to DRAM (no transpose needed) and
            # broadcast-read it back with a zero partition stride.
            vBb = work.tile([P, NCHUNK], BF16, name=f"vBb{b}")
            nc.vector.tensor_copy(vBb, vBm)
            vfd = dpool.tile([1, N], BF16, name=f"vfd{b}")
            nc.sync.dma_start(
                out=vfd[0:1, :].rearrange("o (p c) -> o p c", p=P),
                in_=vBb)
            vrep = work.tile([P, N], BF16, name=f"vrep{b}")
            vfd_bcast = bass.AP(tensor=vfd.tensor, offset=vfd.offset,
                                ap=[[0, P], vfd.ap[-1]])
            nc.sync.dma_start(out=vrep, in_=vfd_bcast)

            # part 2: rank selection + final combine; deferred into the
            # next batch's main loop so the DMA latency above is hidden
            def mk_epi(b=b, vBm=vBm, vrep=vrep, stack2=stack2):
                def go():
                    rank8 = work.tile([P, NCHUNK], F32, name=f"rank8{b}")
                    for c in range(NCHUNK):
                        nc.vector.tensor_scalar(
                            waste1k, vrep, vBm[:, c:c + 1], 0.0,
                            OP.is_lt, OP.add, accum_out=rank8[:, c:c + 1])
                    sel8 = work.tile([P, NCHUNK], F32, name=f"sel8{b}")
                    nc.vector.tensor_scalar(sel8, rank8, float(TOPK), None,
                                            OP.is_lt)
                    prod8 = work.tile([P, NCHUNK], F32, name=f"prod8{b}")
                    nc.vector.tensor_mul(prod8, sel8, vBm)
                    # sum of selected (rowmin-3); add 3 per selected row
                    # via the count
                    nc.vector.tensor_reduce(stack2[:, 0:1], prod8, AX.X,
                                            OP.add)
                    cnt8 = work.tile([P, 1], F32, name=f"cnt8{b}")
                    nc.vector.tensor_reduce(cnt8, sel8, AX.X, OP.add)
                    if b == 0:
                        fin = work.tile([P, 2], F32, name=f"fin{b}")
                        nc.gpsimd.partition_all_reduce(fin, stack2, P,
                                                       RED.add)
                        fcnt = work.tile([P, 1], F32, name=f"fcnt{b}")
                        nc.gpsimd.partition_all_reduce(fcnt, cnt8, P,
                                                       RED.add)
                    else:
                        # PE ones-matmul: PSUM is free after the main loop
                        st3 = work.tile([P, 3], F32, name=f"st3{b}")
                        nc.vector.tensor_copy(st3[:, 0:2], stack2)
                        nc.vector.tensor_copy(st3[:, 2:3], cnt8)
                        finp = psum.tile([1, 3], F32, name="finp", tag="ps")
                        nc.tensor.matmul(finp, ones128, st3, start=True,
                                         stop=True)
                        fin = work.tile([1, 2], F32, name=f"fin{b}")
                        nc.vector.tensor_copy(fin, finp[0:1, 0:2])
                        fcnt = work.tile([1, 1], F32, name=f"fcnt{b}")
                        nc.vector.tensor_copy(fcnt, finp[0:1, 2:3])
                    en = work.tile([1, 2], F32, name=f"en{b}")
                    nc.vector.tensor_scalar(
                        en[0:1, 0:1], fcnt[0:1, 0:1],
                        CLASH * ATTRACT_SCALE / TOPK, None, OP.mult)
                    t0 = work.tile([1, 1], F32, name=f"t0{b}")
                    nc.vector.tensor_scalar(t0, fin[0:1, 0:1],
                                            ATTRACT_SCALE / TOPK, None,
                                            OP.mult)
                    nc.vector.tensor_scalar(en[0:1, 1:2], fin[0:1, 1:2],
                                            REPEL_SCALE, None, OP.mult)
                    en2 = work.tile([1, 1], F32, name=f"en2{b}")
                    nc.vector.tensor_add(en2, en[0:1, 0:1], en[0:1, 1:2])
                    en3 = work.tile([1, 1], F32, name=f"en3{b}")
                    nc.vector.tensor_add(en3, en2, t0)
                    nc.sync.dma_start(out=out[b:b + 1, 0:1],
                                      in_=en3[0:1, 0:1])
                return go
            pend_epi.append(mk_epi())
        flush(pend_epi)

    nc.compile()
    return nc


def _get_program():
    if "nc" not in _prog_cache:
        _prog_cache["nc"] = build_program()
    return _prog_cache["nc"]


def make_in_maps(binder_trans, target_coords):
    x = np.asarray(binder_trans, dtype=np.float32)          # [B, N, 3]
    y = np.asarray(target_coords, dtype=np.float32)         # [M, 3]
    # lhsT per batch: [x0 x1 x2 ; ||x||^2 ; 1]  -> [B, 5, N]
    xT = x.transpose(0, 2, 1)                               # [B, 3, N]
    xsq = (x * x).sum(-1)[:, None, :]                       # [B, 1, N]
    ones_n = np.ones((B, 1, N), dtype=np.float32)
    lhs = np.ascontiguousarray(
        np.concatenate([xT, xsq, ones_n], axis=1))          # [B, 5, N]
    # rhs: [-2y0 -2y1 -2y2 ; 1 ; sum(y^2)+eps] -> [5, M]
    yT = -2.0 * y.T                                         # [3, M]
    ones_m = np.ones((1, MT), dtype=np.float32)
    ysq = (y * y).sum(-1)[None, :] + np.float32(EPS)        # [1, M]
    rhs = np.ascontiguousarray(
        np.concatenate([yT, ones_m, ysq], axis=0))          # [5, M]
    return [{"bnd": np.ascontiguousarray(lhs[c * BC:(c + 1) * BC]),
             "tgt": rhs}
            for c in range(NCORES)]


def kernel(binder_trans, target_coords):
    nc = _get_program()
    in_maps = make_in_maps(binder_trans, target_coords)
    res = run_bass_kernel_spmd(nc, in_maps, list(range(NCORES)))
    outs = [np.asarray(res.results[c]["out"], dtype=np.float32).reshape(BC)
            for c in range(NCORES)]
    return np.concatenate(outs).astype(np.float32)


# revision 4
# speedup vs baseline: 1.7399x; 1.0158x over previous
"""Trainium2 Bass kernel for BinderEnergyGuidance (retrieval_knn).

Math (per batch b of 16):
  d[b,n,m]   = ||binder[b,n] - target[m]||           (N=1024, M=8192)
  attract[b] = mean of the k=204 smallest per-row min-distances
  repel[b]   = sum relu(3 - d)^2
  out[b]     = 10*attract[b] + 5*repel[b]

Strategy: data-parallel over batch, 2 batches per core.  Host packs the
inputs so the device contraction is K=5:
  lhsT rows: [x0 x1 x2 ; ||x||^2 ; 1]        (per batch, [5, N])
  rhs  rows: [-2y0 -2y1 -2y2 ; 1 ; sum(y^2)+eps]   ([5, M])
so one fp32r matmul emits d2 + eps directly into PSUM.

Per 2048-wide m-tile (64 per core):
  PE : 4x 512-col matmuls -> PSUM fp32 d2 (2 PSUM tiles = all 8 banks)
  Act: dc = Sqrt(d2) PSUM->SBUF bf16 (eps pre-added keeps d2 > 0)
  DVE tsA: w = min(dc,3), accum(min) -> per-row min dist (attract).
  repel integrand (3-w)^2, summed per row, via 3 balanced routes:
    'A': Act Square(w - 3) + accum          (Act only)
    'D': vc = w-3 (ts); vc*vc (tt); ts accum (DVE only)
    'P': vc = w-3 (ts); vc*vc on gpsimd tt; ts accum (Pool + DVE)
  Route counts chosen so Act/DVE/Pool all land at ~140us busy.

Note: tensor_scalar with accum_out applies only op0 to the written
output; op1 becomes the accumulation op.  All op1 choices here are
no-ops on the output.

Epilogue (per batch): rowmins [128,8] -> +3 -> bf16 pad to [128,128] ->
XBAR DMA transpose -> flatten -> gpsimd partition_broadcast -> top-k by
rank (count of strictly-smaller values) -> select/dot; final partition
sums via gpsimd partition_all_reduce (no PSUM use outside the matmuls).

Self-contained: hardcodes shapes for binder[16,1024,3], target[8192,3].
"""

import numpy as np
from contextlib import ExitStack

import concourse.bass as bass
import concourse.bacc as bacc
import concourse.tile as tile
from concourse import mybir, bass_isa
from concourse.bass_utils import run_bass_kernel_spmd

F32 = mybir.dt.float32
F32R = mybir.dt.float32r
BF16 = mybir.dt.bfloat16
AF = mybir.ActivationFunctionType
OP = mybir.AluOpType
AX = mybir.AxisListType
RED = bass_isa.ReduceOp

B, N, MT = 16, 1024, 8192
NCORES = 8
BC = B // NCORES          # batches per core
TOPK = 204                # int(0.2 * N)
CLASH = 3.0
EPS = 1e-3                # guards sqrt against fp-rounding-negative d2
ATTRACT_SCALE, REPEL_SCALE = 10.0, 5.0

P = 128                   # SBUF partitions
NCHUNK = N // P           # 8 row-chunks per batch
MTILE = 2048              # PSUM tile free size (4 banks)
JPC = MT // MTILE         # m-tiles per chunk (4)
MMF = 512                 # fp32 matmul max moving free size
KP = 5                    # packed contraction size

# square-route assignment per (batch, m-tile) — 32 tiles per batch.
# 'P' gpsimd, 'A' scalar-engine Square, 'D' vector engine.
# Batch 1 keeps its last chunks gpsimd-free so Pool drains before the
# final epilogue instead of 25us after the last sqrt.


def _routes():
    b0 = []
    for k in range(32):
        if k % 2 == 0:
            b0.append("P")
        elif k % 8 == 1:
            b0.append("A")
        else:
            b0.append("D")
    b1 = []
    for k in range(32):
        if k < 20:
            b1.append("P" if k % 5 != 4 else "D")
        elif k < 24:
            b1.append("D")
        elif k < 30:
            b1.append("A")
        else:
            b1.append("D")
    return [b0, b1]


ROUTES = _routes()

_prog_cache = {}


def build_program():
    nc = bacc.Bacc("TRN2", target_bir_lowering=False, debug=False,
                   num_devices=NCORES)
    bnd = nc.dram_tensor("bnd", [BC, KP, N], F32R, kind="ExternalInput").ap()
    tgt = nc.dram_tensor("tgt", [KP, MT], F32R, kind="ExternalInput").ap()
    out = nc.dram_tensor("out", [BC, 1], F32, kind="ExternalOutput").ap()

    with tile.TileContext(nc) as tc, ExitStack() as ctx:
        consts = ctx.enter_context(tc.tile_pool(name="consts", bufs=1))
        work = ctx.enter_context(tc.tile_pool(name="work", bufs=1))
        dcp = ctx.enter_context(tc.tile_pool(name="dcp", bufs=6))
        wp = ctx.enter_context(tc.tile_pool(name="wp", bufs=6))
        vcd = ctx.enter_context(tc.tile_pool(name="vcd", bufs=3))
        vcp = ctx.enter_context(tc.tile_pool(name="vcp", bufs=8))
        wap = ctx.enter_context(tc.tile_pool(name="wap", bufs=3))
        v2d = ctx.enter_context(tc.tile_pool(name="v2d", bufs=3))
        v2pp = ctx.enter_context(tc.tile_pool(name="v2pp", bufs=6))
        psum = ctx.enter_context(tc.tile_pool(name="psum", bufs=2, space="PSUM"))
        dpool = ctx.enter_context(tc.tile_pool(name="dpool", bufs=1, space="DRAM"))

        # ---- inputs: pure DMA, no engine preamble.  lhsT0 + first rhs
        # chunk gate the first matmul, so issue them first on separate
        # queues. ----
        rhs = consts.tile([KP, MT], F32R)
        lhsTs = [consts.tile([KP, N], F32R, name=f"lhsT{b}") for b in range(BC)]
        nc.sync.dma_start(out=lhsTs[0][:, :], in_=bnd[0, :, :])
        nc.scalar.dma_start(out=rhs[:, 0:512], in_=tgt[:, 0:512])
        nc.sync.dma_start(out=rhs[:, 512:2048], in_=tgt[:, 512:2048])
        nc.scalar.dma_start(out=rhs[:, 2048:4096], in_=tgt[:, 2048:4096])
        nc.sync.dma_start(out=rhs[:, 4096:6144], in_=tgt[:, 4096:6144])
        nc.scalar.dma_start(out=rhs[:, 6144:8192], in_=tgt[:, 6144:8192])
        nc.sync.dma_start(out=lhsTs[1][:, :], in_=bnd[1, :, :])

        # shared elementwise-output scratch (values never read)
        waste1k = work.tile([P, N], BF16)
        neg3 = consts.tile([P, 1], F32)
        nc.vector.memset(neg3, -CLASH)
        ones128 = consts.tile([P, 1], F32)
        nc.vector.memset(ones128, 1.0)

        # Engine queues are strict FIFO, so an op that waits on a slow
        # producer (gpsimd square, epilogue DMA chain) must not be emitted
        # right after it or it head-of-line-blocks the whole engine.
        # Deferred emission queues provide the lag.
        pend_sum = []     # P-route row-sum ts ops waiting for gpsimd
        pend_act = []     # A-route Square ops, lag one tile
        pend_epi = []     # previous batch's epilogue part 2
        pend_fin = []     # final partition-sum + output, all batches

        def flush(q, keep=0):
            while len(q) > keep:
                q.pop(0)()

        for b in range(BC):
            lhsT = lhsTs[b]
            sw2B = work.tile([P, NCHUNK * JPC], F32, name=f"sw2B{b}")
            mnB = work.tile([P, NCHUNK * JPC], F32, name=f"mnB{b}")

            for c in range(NCHUNK):
                lc = lhsT[:, c * P:(c + 1) * P]
                for j in range(JPC):
                    k = c * JPC + j
                    route = ROUTES[b][k]
                    ps = psum.tile([P, MTILE], F32, name="ps", tag="ps")
                    for q in range(MTILE // MMF):
                        nc.tensor.matmul(
                            ps[:, q * MMF:(q + 1) * MMF], lc,
                            rhs[:, j * MTILE + q * MMF:
                                j * MTILE + (q + 1) * MMF],
                            start=True, stop=True)
                    dc = dcp.tile([P, MTILE], BF16, name="dc", tag="dc")
                    nc.scalar.activation(dc, ps, AF.Sqrt)
                    # w = min(dc, 3); accum(min) -> rowmin (attract)
                    w = wp.tile([P, MTILE], BF16, name="w", tag="w")
                    nc.vector.tensor_scalar(
                        w, dc, CLASH, 3.4e38, OP.min, OP.min,
                        accum_out=mnB[:, k:k + 1])
                    if route == "A":
                        def mk_act(w=w, sw2B=sw2B, k=k):
                            def go():
                                wa = wap.tile([P, MTILE], BF16, name="wa",
                                              tag="wa")
                                nc.scalar.activation(
                                    wa, w, AF.Square, bias=neg3[:, 0:1],
                                    scale=1.0, accum_out=sw2B[:, k:k + 1])
                            return go
                        pend_act.append(mk_act())
                    else:
                        # vc = w - 3 = -relu(3-d)
                        pool_ = vcp if route == "P" else vcd
                        vc = pool_.tile([P, MTILE], BF16, name="vc", tag="vc")
                        nc.vector.tensor_scalar(
                            vc, w, CLASH, 0.0, OP.subtract, OP.min)
                        if route == "P":
                            v2 = v2pp.tile([P, MTILE], BF16, name="v2",
                                           tag="v2")
                            nc.gpsimd.tensor_tensor(v2, vc, vc, OP.mult)
                            def mk_sum(v2=v2, sw2B=sw2B, k=k):
                                def go():
                                    nc.vector.tensor_scalar(
                                        v2, v2, 3.4e38, 0.0, OP.min, OP.add,
                                        accum_out=sw2B[:, k:k + 1])
                                return go
                            pend_sum.append(mk_sum())
                        else:
                            v2 = v2d.tile([P, MTILE], BF16, name="v2",
                                          tag="v2")
                            nc.vector.tensor_tensor(v2, vc, vc, OP.mult)
                            nc.vector.tensor_scalar(
                                v2, v2, 3.4e38, 0.0, OP.min, OP.add,
                                accum_out=sw2B[:, k:k + 1])
                    flush(pend_act, keep=5)
                    flush(pend_sum, keep=6)
                if c == 1:
                    flush(pend_epi)
            flush(pend_act)
            flush(pend_sum)

            # ---- per-batch epilogue, part 1: reduces + the transpose/
            # broadcast DMA chain (off the compute engines) ----
            vBm = work.tile([P, NCHUNK], F32, name=f"vBm{b}")
            nc.vector.tensor_reduce(
                vBm, mnB.rearrange("p (c j) -> p c j", c=NCHUNK), AX.X, OP.min)
            stack2 = work.tile([P, 2], F32, name=f"stack2{b}")
            nc.vector.tensor_reduce(stack2[:, 1:2], sw2B, AX.X, OP.add)
            # rank selection ignores element order, so flatten [128, 8]
            # partition-major straight to DRAM (no transpose needed) and
            # broadcast-read it back with a zero partition stride.
            vBb = work.tile([P, NCHUNK], BF16, name=f"vBb{b}")
            nc.vector.tensor_copy(vBb, vBm)
            vfd = dpool.tile([1, N], BF16, name=f"vfd{b}")
            nc.sync.dma_start(
                out=vfd[0:1, :].rearrange("o (p c) -> o p c", p=P),
                in_=vBb)
            vrep = work.tile([P, N], BF16, name=f"vrep{b}")
            vfd_bcast = bass.AP(tensor=vfd.tensor, offset=vfd.offset,
                                ap=[[0, P], vfd.ap[-1]])
            nc.sync.dma_start(out=vrep, in_=vfd_bcast)

            # part 2: rank selection + final combine; deferred into the
            # next batch's main loop so the DMA latency above is hidden
            def mk_epi(b=b, vBm=vBm, vrep=vrep, stack2=stack2):
                def go():
                    rank8 = work.tile([P, NCHUNK], F32, name=f"rank8{b}")
                    for c in range(NCHUNK):
                        nc.vector.tensor_scalar(
                            waste1k, vrep, vBm[:, c:c + 1], 0.0,
                            OP.is_lt, OP.add, accum_out=rank8[:, c:c + 1])
                    sel8 = work.tile([P, NCHUNK], F32, name=f"sel8{b}")
                    nc.vector.tensor_scalar(sel8, rank8, float(TOPK), None,
                                            OP.is_lt)
                    prod8 = work.tile([P, NCHUNK], F32, name=f"prod8{b}")
                    nc.vector.tensor_mul(prod8, sel8, vBm)
                    # sum of selected (rowmin-3); add 3 per selected row
                    # via the count
                    nc.vector.tensor_reduce(stack2[:, 0:1], prod8, AX.X,
                                            OP.add)
                    cnt8 = work.tile([P, 1], F32, name=f"cnt8{b}")
                    nc.vector.tensor_reduce(cnt8, sel8, AX.X, OP.add)
                    st3 = work.tile([P, 3], F32, name=f"st3{b}")
                    nc.vector.tensor_copy(st3[:, 0:2], stack2)
                    nc.vector.tensor_copy(st3[:, 2:3], cnt8)

                    # PE ones-matmul + final combine: deferred to after the
                    # main loops, when PE/PSUM are idle (a mid-stream fin
                    # matmul head-of-line-blocks the PE FIFO)
                    def fin_go(b=b, st3=st3):
                        finp = psum.tile([1, 3], F32, name="finp", tag="ps")
                        nc.tensor.matmul(finp, ones128, st3, start=True,
                                         stop=True)
                        fin = work.tile([1, 2], F32, name=f"fin{b}")
                        nc.vector.tensor_copy(fin, finp[0:1, 0:2])
                        fcnt = work.tile([1, 1], F32, name=f"fcnt{b}")
                        nc.vector.tensor_copy(fcnt, finp[0:1, 2:3])
                        en = work.tile([1, 2], F32, name=f"en{b}")
                        nc.vector.tensor_scalar(
                            en[0:1, 0:1], fcnt[0:1, 0:1],
                            CLASH * ATTRACT_SCALE / TOPK, None, OP.mult)
                        t0 = work.tile([1, 1], F32, name=f"t0{b}")
                        nc.vector.tensor_scalar(t0, fin[0:1, 0:1],
                                                ATTRACT_SCALE / TOPK, None,
                                                OP.mult)
                        nc.vector.tensor_scalar(en[0:1, 1:2], fin[0:1, 1:2],
                                                REPEL_SCALE, None, OP.mult)
                        en2 = work.tile([1, 1], F32, name=f"en2{b}")
                        nc.vector.tensor_add(en2, en[0:1, 0:1], en[0:1, 1:2])
                        en3 = work.tile([1, 1], F32, name=f"en3{b}")
                        nc.vector.tensor_add(en3, en2, t0)
                        nc.sync.dma_start(out=out[b:b + 1, 0:1],
                                          in_=en3[0:1, 0:1])
                    pend_fin.append(fin_go)
                return go
            pend_epi.append(mk_epi())
        flush(pend_epi)
        flush(pend_fin)

    nc.compile()
    return nc


def _get_program():
    if "nc" not in _prog_cache:
        _prog_cache["nc"] = build_program()
    return _prog_cache["nc"]


def make_in_maps(binder_trans, target_coords):
    x = np.asarray(binder_trans, dtype=np.float32)          # [B, N, 3]
    y = np.asarray(target_coords, dtype=np.float32)         # [M, 3]
    # lhsT per batch: [x0 x1 x2 ; ||x||^2 ; 1]  -> [B, 5, N]
    xT = x.transpose(0, 2, 1)                               # [B, 3, N]
    xsq = (x * x).sum(-1)[:, None, :]                       # [B, 1, N]
    ones_n = np.ones((B, 1, N), dtype=np.float32)
    lhs = np.ascontiguousarray(
        np.concatenate([xT, xsq, ones_n], axis=1))          # [B, 5, N]
    # rhs: [-2y0 -2y1 -2y2 ; 1 ; sum(y^2)+eps] -> [5, M]
    yT = -2.0 * y.T                                         # [3, M]
    ones_m = np.ones((1, MT), dtype=np.float32)
    ysq = (y * y).sum(-1)[None, :] + np.float32(EPS)        # [1, M]
    rhs = np.ascontiguousarray(
        np.concatenate([yT, ones_m, ysq], axis=0))          # [5, M]
    return [{"bnd": np.ascontiguousarray(lhs[c * BC:(c + 1) * BC]),
             "tgt": rhs}
            for c in range(NCORES)]


def kernel(binder_trans, target_coords):
    nc = _get_program()
    in_maps = make_in_maps(binder_trans, target_coords)
    res = run_bass_kernel_spmd(nc, in_maps, list(range(NCORES)))
    outs = [np.asarray(res.results[c]["out"], dtype=np.float32).reshape(BC)
            for c in range(NCORES)]
    return np.concatenate(outs).astype(np.float32)


# revision 5
# speedup vs baseline: 1.7688x; 1.0166x over previous
"""Trainium2 Bass kernel for BinderEnergyGuidance (retrieval_knn).

Math (per batch b of 16):
  d[b,n,m]   = ||binder[b,n] - target[m]||           (N=1024, M=8192)
  attract[b] = mean of the k=204 smallest per-row min-distances
  repel[b]   = sum relu(3 - d)^2
  out[b]     = 10*attract[b] + 5*repel[b]

Strategy: data-parallel over batch, 2 batches per core.  Host packs the
inputs so the device contraction is K=5:
  lhsT rows: [x0 x1 x2 ; ||x||^2 ; 1]        (per batch, [5, N])
  rhs  rows: [-2y0 -2y1 -2y2 ; 1 ; sum(y^2)+eps]   ([5, M])
so one fp32r matmul emits d2 + eps directly into PSUM.

Per 2048-wide m-tile (64 per core):
  PE : 4x 512-col matmuls -> PSUM fp32 d2 (2 PSUM tiles = all 8 banks)
  Act: dc = Sqrt(d2) PSUM->SBUF bf16 (eps pre-added keeps d2 > 0)
  DVE tsA: w = min(dc,3), accum(min) -> per-row min dist (attract).
  repel integrand (3-w)^2, summed per row, via 3 balanced routes:
    'A': Act Square(w - 3) + accum          (Act only)
    'D': vc = w-3 (ts); vc*vc (tt); ts accum (DVE only)
    'P': vc = w-3 (ts); vc*vc on gpsimd tt; ts accum (Pool + DVE)
  Route counts chosen so Act/DVE/Pool all land at ~140us busy.

Note: tensor_scalar with accum_out applies only op0 to the written
output; op1 becomes the accumulation op.  All op1 choices here are
no-ops on the output.

Epilogue (per batch): rowmins [128,8] -> +3 -> bf16 pad to [128,128] ->
XBAR DMA transpose -> flatten -> gpsimd partition_broadcast -> top-k by
rank (count of strictly-smaller values) -> select/dot; final partition
sums via gpsimd partition_all_reduce (no PSUM use outside the matmuls).

Self-contained: hardcodes shapes for binder[16,1024,3], target[8192,3].
"""

import numpy as np
from contextlib import ExitStack

import concourse.bass as bass
import concourse.bacc as bacc
import concourse.tile as tile
from concourse import mybir, bass_isa
from concourse.bass_utils import run_bass_kernel_spmd

F32 = mybir.dt.float32
F32R = mybir.dt.float32r
BF16 = mybir.dt.bfloat16
AF = mybir.ActivationFunctionType
OP = mybir.AluOpType
AX = mybir.AxisListType
RED = bass_isa.ReduceOp

B, N, MT = 16, 1024, 8192
NCORES = 8
BC = B // NCORES          # batches per core
TOPK = 204                # int(0.2 * N)
CLASH = 3.0
EPS = 1e-3                # guards sqrt against fp-rounding-negative d2
ATTRACT_SCALE, REPEL_SCALE = 10.0, 5.0

P = 128                   # SBUF partitions
NCHUNK = N // P           # 8 row-chunks per batch
MTILE = 2048              # PSUM tile free size (4 banks)
JPC = MT // MTILE         # m-tiles per chunk (4)
MMF = 512                 # fp32 matmul max moving free size
KP = 5                    # packed contraction size

# square-route assignment per (batch, m-tile) — 32 tiles per batch.
# 'P' gpsimd, 'A' scalar-engine Square, 'D' vector engine.
# Batch 1 keeps its last chunks gpsimd-free so Pool drains before the
# final epilogue instead of 25us after the last sqrt.


def _routes():
    b0 = []
    for k in range(32):
        if k % 2 == 0:
            b0.append("P")
        elif k in (9, 25):
            b0.append("A")
        else:
            b0.append("D")
    b1 = []
    for k in range(32):
        if k < 20:
            b1.append("P" if k % 5 != 4 else "D")
        elif k < 24:
            b1.append("D")
        elif k < 30:
            b1.append("A")
        else:
            b1.append("D")
    return [b0, b1]


ROUTES = _routes()

_prog_cache = {}


def build_program():
    nc = bacc.Bacc("TRN2", target_bir_lowering=False, debug=False,
                   num_devices=NCORES)
    bnd = nc.dram_tensor("bnd", [BC, KP, N], F32R, kind="ExternalInput").ap()
    tgt = nc.dram_tensor("tgt", [KP, MT], F32R, kind="ExternalInput").ap()
    out = nc.dram_tensor("out", [BC, 1], F32, kind="ExternalOutput").ap()

    with tile.TileContext(nc) as tc, ExitStack() as ctx:
        consts = ctx.enter_context(tc.tile_pool(name="consts", bufs=1))
        work = ctx.enter_context(tc.tile_pool(name="work", bufs=1))
        dcp = ctx.enter_context(tc.tile_pool(name="dcp", bufs=6))
        wp = ctx.enter_context(tc.tile_pool(name="wp", bufs=6))
        vcd = ctx.enter_context(tc.tile_pool(name="vcd", bufs=3))
        vcp = ctx.enter_context(tc.tile_pool(name="vcp", bufs=8))
        wap = ctx.enter_context(tc.tile_pool(name="wap", bufs=3))
        v2d = ctx.enter_context(tc.tile_pool(name="v2d", bufs=3))
        v2pp = ctx.enter_context(tc.tile_pool(name="v2pp", bufs=6))
        psum = ctx.enter_context(tc.tile_pool(name="psum", bufs=2, space="PSUM"))
        dpool = ctx.enter_context(tc.tile_pool(name="dpool", bufs=1, space="DRAM"))

        # ---- inputs: pure DMA, no engine preamble.  lhsT0 + first rhs
        # chunk gate the first matmul, so issue them first on separate
        # queues. ----
        rhs = consts.tile([KP, MT], F32R)
        lhsTs = [consts.tile([KP, N], F32R, name=f"lhsT{b}") for b in range(BC)]
        nc.sync.dma_start(out=lhsTs[0][:, :], in_=bnd[0, :, :])
        nc.scalar.dma_start(out=rhs[:, 0:512], in_=tgt[:, 0:512])
        nc.sync.dma_start(out=rhs[:, 512:2048], in_=tgt[:, 512:2048])
        nc.scalar.dma_start(out=rhs[:, 2048:4096], in_=tgt[:, 2048:4096])
        nc.sync.dma_start(out=rhs[:, 4096:6144], in_=tgt[:, 4096:6144])
        nc.scalar.dma_start(out=rhs[:, 6144:8192], in_=tgt[:, 6144:8192])
        nc.sync.dma_start(out=lhsTs[1][:, :], in_=bnd[1, :, :])

        # shared elementwise-output scratch (values never read)
        waste1k = work.tile([P, N], BF16)
        neg3 = consts.tile([P, 1], F32)
        nc.vector.memset(neg3, -CLASH)
        ones128 = consts.tile([P, 1], F32)
        nc.vector.memset(ones128, 1.0)

        # Engine queues are strict FIFO, so an op that waits on a slow
        # producer (gpsimd square, epilogue DMA chain) must not be emitted
        # right after it or it head-of-line-blocks the whole engine.
        # Deferred emission queues provide the lag.
        pend_sum = []     # P-route row-sum ts ops waiting for gpsimd
        pend_act = []     # A-route Square ops, lag one tile
        pend_epi = []     # previous batch's epilogue part 2
        pend_fin = []     # final partition-sum + output, all batches

        def flush(q, keep=0):
            while len(q) > keep:
                q.pop(0)()

        for b in range(BC):
            lhsT = lhsTs[b]
            sw2B = work.tile([P, NCHUNK * JPC], F32, name=f"sw2B{b}")
            mnB = work.tile([P, NCHUNK * JPC], F32, name=f"mnB{b}")

            for c in range(NCHUNK):
                lc = lhsT[:, c * P:(c + 1) * P]
                for j in range(JPC):
                    k = c * JPC + j
                    route = ROUTES[b][k]
                    ps = psum.tile([P, MTILE], F32, name="ps", tag="ps")
                    for q in range(MTILE // MMF):
                        nc.tensor.matmul(
                            ps[:, q * MMF:(q + 1) * MMF], lc,
                            rhs[:, j * MTILE + q * MMF:
                                j * MTILE + (q + 1) * MMF],
                            start=True, stop=True)
                    dc = dcp.tile([P, MTILE], BF16, name="dc", tag="dc")
                    nc.scalar.activation(dc, ps, AF.Sqrt)
                    # w = min(dc, 3); accum(min) -> rowmin (attract)
                    w = wp.tile([P, MTILE], BF16, name="w", tag="w")
                    nc.vector.tensor_scalar(
                        w, dc, CLASH, 3.4e38, OP.min, OP.min,
                        accum_out=mnB[:, k:k + 1])
                    if route == "A":
                        def mk_act(w=w, sw2B=sw2B, k=k):
                            def go():
                                wa = wap.tile([P, MTILE], BF16, name="wa",
                                              tag="wa")
                                nc.scalar.activation(
                                    wa, w, AF.Square, bias=neg3[:, 0:1],
                                    scale=1.0, accum_out=sw2B[:, k:k + 1])
                            return go
                        pend_act.append(mk_act())
                    else:
                        # vc = w - 3 = -relu(3-d)
                        pool_ = vcp if route == "P" else vcd
                        vc = pool_.tile([P, MTILE], BF16, name="vc", tag="vc")
                        nc.vector.tensor_scalar(
                            vc, w, CLASH, 0.0, OP.subtract, OP.min)
                        if route == "P":
                            v2 = v2pp.tile([P, MTILE], BF16, name="v2",
                                           tag="v2")
                            nc.gpsimd.tensor_tensor(v2, vc, vc, OP.mult)
                            def mk_sum(v2=v2, sw2B=sw2B, k=k):
                                def go():
                                    nc.vector.tensor_scalar(
                                        v2, v2, 3.4e38, 0.0, OP.min, OP.add,
                                        accum_out=sw2B[:, k:k + 1])
                                return go
                            pend_sum.append(mk_sum())
                        else:
                            v2 = v2d.tile([P, MTILE], BF16, name="v2",
                                          tag="v2")
                            nc.vector.tensor_tensor(v2, vc, vc, OP.mult)
                            nc.vector.tensor_scalar(
                                v2, v2, 3.4e38, 0.0, OP.min, OP.add,
                                accum_out=sw2B[:, k:k + 1])
                    flush(pend_act, keep=5)
                    flush(pend_sum, keep=6)
                if c == 1:
                    flush(pend_epi)
            flush(pend_act)
            flush(pend_sum)

            # ---- per-batch epilogue, part 1: reduces + the transpose/
            # broadcast DMA chain (off the compute engines) ----
            vBm = work.tile([P, NCHUNK], F32, name=f"vBm{b}")
            nc.vector.tensor_reduce(
                vBm, mnB.rearrange("p (c j) -> p c j", c=NCHUNK), AX.X, OP.min)
            stack2 = work.tile([P, 2], F32, name=f"stack2{b}")
            nc.vector.tensor_reduce(stack2[:, 1:2], sw2B, AX.X, OP.add)
            # rank selection ignores element order, so flatten [128, 8]
            # partition-major straight to DRAM (no transpose needed) and
            # broadcast-read it back with a zero partition stride.
            vBb = work.tile([P, NCHUNK], BF16, name=f"vBb{b}")
            nc.vector.tensor_copy(vBb, vBm)
            vfd = dpool.tile([1, N], BF16, name=f"vfd{b}")
            nc.sync.dma_start(
                out=vfd[0:1, :].rearrange("o (p c) -> o p c", p=P),
                in_=vBb)
            vrep = work.tile([P, N], BF16, name=f"vrep{b}")
            vfd_bcast = bass.AP(tensor=vfd.tensor, offset=vfd.offset,
                                ap=[[0, P], vfd.ap[-1]])
            nc.sync.dma_start(out=vrep, in_=vfd_bcast)

            # part 2: rank selection + final combine; deferred into the
            # next batch's main loop so the DMA latency above is hidden
            def mk_epi(b=b, vBm=vBm, vrep=vrep, stack2=stack2):
                def go():
                    rank8 = work.tile([P, NCHUNK], F32, name=f"rank8{b}")
                    for c in range(NCHUNK):
                        nc.vector.tensor_scalar(
                            waste1k, vrep, vBm[:, c:c + 1], 0.0,
                            OP.is_lt, OP.add, accum_out=rank8[:, c:c + 1])
                    sel8 = work.tile([P, NCHUNK], F32, name=f"sel8{b}")
                    nc.vector.tensor_scalar(sel8, rank8, float(TOPK), None,
                                            OP.is_lt)
                    prod8 = work.tile([P, NCHUNK], F32, name=f"prod8{b}")
                    nc.vector.tensor_mul(prod8, sel8, vBm)
                    # sum of selected (rowmin-3); add 3 per selected row
                    # via the count
                    nc.vector.tensor_reduce(stack2[:, 0:1], prod8, AX.X,
                                            OP.add)
                    cnt8 = work.tile([P, 1], F32, name=f"cnt8{b}")
                    nc.vector.tensor_reduce(cnt8, sel8, AX.X, OP.add)
                    st3 = work.tile([P, 3], F32, name=f"st3{b}")
                    nc.vector.tensor_copy(st3[:, 0:2], stack2)
                    nc.vector.tensor_copy(st3[:, 2:3], cnt8)

                    # PE ones-matmul + final combine: deferred to after the
                    # main loops, when PE/PSUM are idle (a mid-stream fin
                    # matmul head-of-line-blocks the PE FIFO)
                    def fin_go(b=b, st3=st3):
                        finp = psum.tile([1, 3], F32, name="finp", tag="ps")
                        nc.tensor.matmul(finp, ones128, st3, start=True,
                                         stop=True)
                        fin = work.tile([1, 2], F32, name=f"fin{b}")
                        nc.vector.tensor_copy(fin, finp[0:1, 0:2])
                        fcnt = work.tile([1, 1], F32, name=f"fcnt{b}")
                        nc.vector.tensor_copy(fcnt, finp[0:1, 2:3])
                        en = work.tile([1, 2], F32, name=f"en{b}")
                        nc.vector.tensor_scalar(
                            en[0:1, 0:1], fcnt[0:1, 0:1],
                            CLASH * ATTRACT_SCALE / TOPK, None, OP.mult)
                        t0 = work.tile([1, 1], F32, name=f"t0{b}")
                        nc.vector.tensor_scalar(t0, fin[0:1, 0:1],
                                                ATTRACT_SCALE / TOPK, None,
                                                OP.mult)
                        nc.vector.tensor_scalar(en[0:1, 1:2], fin[0:1, 1:2],
                                                REPEL_SCALE, None, OP.mult)
                        en2 = work.tile([1, 1], F32, name=f"en2{b}")
                        nc.vector.tensor_add(en2, en[0:1, 0:1], en[0:1, 1:2])
                        en3 = work.tile([1, 1], F32, name=f"en3{b}")
                        nc.vector.tensor_add(en3, en2, t0)
                        nc.sync.dma_start(out=out[b:b + 1, 0:1],
                                          in_=en3[0:1, 0:1])
                    pend_fin.append(fin_go)
                return go
            pend_epi.append(mk_epi())
        flush(pend_epi)
        flush(pend_fin)

    nc.compile()
    return nc


def _get_program():
    if "nc" not in _prog_cache:
        _prog_cache["nc"] = build_program()
    return _prog_cache["nc"]


def make_in_maps(binder_trans, target_coords):
    x = np.asarray(binder_trans, dtype=np.float32)          # [B, N, 3]
    y = np.asarray(target_coords, dtype=np.float32)         # [M, 3]
    # lhsT per batch: [x0 x1 x2 ; ||x||^2 ; 1]  -> [B, 5, N]
    xT = x.transpose(0, 2, 1)                               # [B, 3, N]
    xsq = (x * x).sum(-1)[:, None, :]                       # [B, 1, N]
    ones_n = np.ones((B, 1, N), dtype=np.float32)
    lhs = np.ascontiguousarray(
        np.concatenate([xT, xsq, ones_n], axis=1))          # [B, 5, N]
    # rhs: [-2y0 -2y1 -2y2 ; 1 ; sum(y^2)+eps] -> [5, M]
    yT = -2.0 * y.T                                         # [3, M]
    ones_m = np.ones((1, MT), dtype=np.float32)
    ysq = (y * y).sum(-1)[None, :] + np.float32(EPS)        # [1, M]
    rhs = np.ascontiguousarray(
        np.concatenate([yT, ones_m, ysq], axis=0))          # [5, M]
    return [{"bnd": np.ascontiguousarray(lhs[c * BC:(c + 1) * BC]),
             "tgt": rhs}
            for c in range(NCORES)]


def kernel(binder_trans, target_coords):
    nc = _get_program()
    in_maps = make_in_maps(binder_trans, target_coords)
    res = run_bass_kernel_spmd(nc, in_maps, list(range(NCORES)))
    outs = [np.asarray(res.results[c]["out"], dtype=np.float32).reshape(BC)
            for c in range(NCORES)]
    return np.concatenate(outs).astype(np.float32)


# revision 6
# speedup vs baseline: 1.8090x; 1.0227x over previous
"""Trainium2 Bass kernel for BinderEnergyGuidance (retrieval_knn).

Math (per batch b of 16):
  d[b,n,m]   = ||binder[b,n] - target[m]||           (N=1024, M=8192)
  attract[b] = mean of the k=204 smallest per-row min-distances
  repel[b]   = sum relu(3 - d)^2
  out[b]     = 10*attract[b] + 5*repel[b]

Strategy: data-parallel over batch, 2 batches per core.  Host packs the
inputs so the device contraction is K=5:
  lhsT rows: [x0 x1 x2 ; ||x||^2 ; 1]        (per batch, [5, N])
  rhs  rows: [-2y0 -2y1 -2y2 ; 1 ; sum(y^2)+eps]   ([5, M])
so one fp32r matmul emits d2 + eps directly into PSUM.

Per 2048-wide m-tile (64 per core):
  PE : 4x 512-col matmuls -> PSUM fp32 d2 (2 PSUM tiles = all 8 banks)
  Act: dc = Sqrt(d2) PSUM->SBUF bf16 (eps pre-added keeps d2 > 0)
  DVE tsA: w = min(dc,3), accum(min) -> per-row min dist (attract).
  repel integrand (3-w)^2, summed per row, via 3 balanced routes:
    'A': Act Square(w - 3) + accum          (Act only)
    'D': vc = w-3 (ts); vc*vc (tt); ts accum (DVE only)
    'P': vc = w-3 (ts); vc*vc on gpsimd tt; ts accum (Pool + DVE)
  Route counts chosen so Act/DVE/Pool all land at ~140us busy.

Note: tensor_scalar with accum_out applies only op0 to the written
output; op1 becomes the accumulation op.  All op1 choices here are
no-ops on the output.

Epilogue (per batch): rowmins [128,8] -> +3 -> bf16 pad to [128,128] ->
XBAR DMA transpose -> flatten -> gpsimd partition_broadcast -> top-k by
rank (count of strictly-smaller values) -> select/dot; final partition
sums via gpsimd partition_all_reduce (no PSUM use outside the matmuls).

Self-contained: hardcodes shapes for binder[16,1024,3], target[8192,3].
"""

import numpy as np
from contextlib import ExitStack

import concourse.bass as bass
import concourse.bacc as bacc
import concourse.tile as tile
from concourse import mybir, bass_isa
from concourse.masks import make_identity
from concourse.bass_utils import run_bass_kernel_spmd

F32 = mybir.dt.float32
F32R = mybir.dt.float32r
BF16 = mybir.dt.bfloat16
AF = mybir.ActivationFunctionType
OP = mybir.AluOpType
AX = mybir.AxisListType
RED = bass_isa.ReduceOp

B, N, MT = 16, 1024, 8192
NCORES = 8
BC = B // NCORES          # batches per core
TOPK = 204                # int(0.2 * N)
CLASH = 3.0
EPS = 1e-3                # guards sqrt against fp-rounding-negative d2
ATTRACT_SCALE, REPEL_SCALE = 10.0, 5.0

P = 128                   # SBUF partitions
NCHUNK = N // P           # 8 row-chunks per batch
MTILE = 2048              # PSUM tile free size (4 banks)
JPC = MT // MTILE         # m-tiles per chunk (4)
MMF = 512                 # fp32 matmul max moving free size
KP = 5                    # packed contraction size

# square-route assignment per (batch, m-tile) — 32 tiles per batch.
# 'P' gpsimd, 'A' scalar-engine Square, 'D' vector engine.
# Batch 1 keeps its last chunks gpsimd-free so Pool drains before the
# final epilogue instead of 25us after the last sqrt.


def _routes():
    b0 = []
    for k in range(32):
        if k % 2 == 0:
            b0.append("P")
        elif k in (9, 25):
            b0.append("A")
        else:
            b0.append("D")
    b1 = []
    for k in range(32):
        if k < 20:
            b1.append("P" if k % 5 != 4 else "D")
        elif k < 24:
            b1.append("D")
        elif k < 30:
            b1.append("A")
        else:
            b1.append("D")
    return [b0, b1]


ROUTES = _routes()

_prog_cache = {}


def build_program():
    nc = bacc.Bacc("TRN2", target_bir_lowering=False, debug=False,
                   num_devices=NCORES)
    bnd = nc.dram_tensor("bnd", [BC, KP, N], F32R, kind="ExternalInput").ap()
    tgt = nc.dram_tensor("tgt", [KP, MT], F32R, kind="ExternalInput").ap()
    out = nc.dram_tensor("out", [BC, 1], F32, kind="ExternalOutput").ap()

    with tile.TileContext(nc) as tc, ExitStack() as ctx:
        consts = ctx.enter_context(tc.tile_pool(name="consts", bufs=1))
        work = ctx.enter_context(tc.tile_pool(name="work", bufs=1))
        dcp = ctx.enter_context(tc.tile_pool(name="dcp", bufs=6))
        wp = ctx.enter_context(tc.tile_pool(name="wp", bufs=6))
        vcd = ctx.enter_context(tc.tile_pool(name="vcd", bufs=3))
        vcp = ctx.enter_context(tc.tile_pool(name="vcp", bufs=8))
        wap = ctx.enter_context(tc.tile_pool(name="wap", bufs=3))
        v2d = ctx.enter_context(tc.tile_pool(name="v2d", bufs=3))
        v2pp = ctx.enter_context(tc.tile_pool(name="v2pp", bufs=6))
        psum = ctx.enter_context(tc.tile_pool(name="psum", bufs=2, space="PSUM"))
        dpool = ctx.enter_context(tc.tile_pool(name="dpool", bufs=1, space="DRAM"))

        # ---- inputs: pure DMA, no engine preamble.  lhsT0 + first rhs
        # chunk gate the first matmul, so issue them first on separate
        # queues. ----
        rhs = consts.tile([KP, MT], F32R)
        lhsTs = [consts.tile([KP, N], F32R, name=f"lhsT{b}") for b in range(BC)]
        nc.sync.dma_start(out=lhsTs[0][:, :], in_=bnd[0, :, :])
        nc.scalar.dma_start(out=rhs[:, 0:512], in_=tgt[:, 0:512])
        nc.sync.dma_start(out=rhs[:, 512:2048], in_=tgt[:, 512:2048])
        nc.scalar.dma_start(out=rhs[:, 2048:4096], in_=tgt[:, 2048:4096])
        nc.sync.dma_start(out=rhs[:, 4096:6144], in_=tgt[:, 4096:6144])
        nc.scalar.dma_start(out=rhs[:, 6144:8192], in_=tgt[:, 6144:8192])
        nc.sync.dma_start(out=lhsTs[1][:, :], in_=bnd[1, :, :])

        # shared elementwise-output scratch (values never read)
        waste1k = work.tile([P, N], BF16)
        neg3 = consts.tile([P, 1], F32)
        nc.vector.memset(neg3, -CLASH)
        ones128 = consts.tile([P, 1], F32)
        nc.vector.memset(ones128, 1.0)
        ident = consts.tile([P, P], F32)
        make_identity(nc, ident)

        # Engine queues are strict FIFO, so an op that waits on a slow
        # producer (gpsimd square, epilogue DMA chain) must not be emitted
        # right after it or it head-of-line-blocks the whole engine.
        # Deferred emission queues provide the lag.
        pend_sum = []     # P-route row-sum ts ops waiting for gpsimd
        pend_act = []     # A-route Square ops, lag one tile
        pend_epi = []     # previous batch's epilogue part 2
        pend_fin = []     # final partition-sum + output, all batches
        pend_rank = []    # b1 split-rank: early ranks vs chunks 0-6

        def flush(q, keep=0):
            while len(q) > keep:
                q.pop(0)()

        for b in range(BC):
            lhsT = lhsTs[b]
            sw2B = work.tile([P, NCHUNK * JPC], F32, name=f"sw2B{b}")
            mnB = work.tile([P, NCHUNK * JPC], F32, name=f"mnB{b}")

            NE = (NCHUNK - 1) * P          # 896: rows in chunks 0-6
            vBm8 = work.tile([P, NCHUNK], F32, name=f"vBm8{b}")
            rankE = work.tile([P, NCHUNK], F32, name=f"rankE{b}")
            vrepE = work.tile([P, NE], BF16, name=f"vrepE{b}")
            for c in range(NCHUNK):
                if b == 1 and c == NCHUNK - 1:
                    # ---- split-rank early piece: chunks 0-6 are final;
                    # flatten+broadcast their rowmins and rank against
                    # them while chunk 7 computes ----
                    nc.vector.tensor_reduce(
                        vBm8[:, 0:NCHUNK - 1],
                        mnB[:, 0:(NCHUNK - 1) * JPC].rearrange(
                            "p (c j) -> p c j", c=NCHUNK - 1),
                        AX.X, OP.min)
                    vBbE = work.tile([P, NCHUNK - 1], BF16, name="vBbE")
                    nc.vector.tensor_copy(vBbE, vBm8[:, 0:NCHUNK - 1])
                    vfdE = dpool.tile([1, NE], BF16, name="vfdE")
                    nc.sync.dma_start(
                        out=vfdE[0:1, :].rearrange("o (p c) -> o p c", p=P),
                        in_=vBbE)
                    vfdE_b = bass.AP(tensor=vfdE.tensor, offset=vfdE.offset,
                                     ap=[[0, P], vfdE.ap[-1]])
                    nc.sync.dma_start(out=vrepE, in_=vfdE_b)
                    for cc in range(NCHUNK - 1):
                        def mk_rank(cc=cc):
                            def go():
                                nc.vector.tensor_scalar(
                                    waste1k[:, 0:NE], vrepE,
                                    vBm8[:, cc:cc + 1], 0.0,
                                    OP.is_lt, OP.add,
                                    accum_out=rankE[:, cc:cc + 1])
                            return go
                        pend_rank.append(mk_rank())
                lc = lhsT[:, c * P:(c + 1) * P]
                for j in range(JPC):
                    if b == 1 and c == NCHUNK - 1 and j >= 2:
                        flush(pend_rank, keep=(4 if j == 2 else 0))
                    k = c * JPC + j
                    route = ROUTES[b][k]
                    ps = psum.tile([P, MTILE], F32, name="ps", tag="ps")
                    for q in range(MTILE // MMF):
                        nc.tensor.matmul(
                            ps[:, q * MMF:(q + 1) * MMF], lc,
                            rhs[:, j * MTILE + q * MMF:
                                j * MTILE + (q + 1) * MMF],
                            start=True, stop=True)
                    dc = dcp.tile([P, MTILE], BF16, name="dc", tag="dc")
                    nc.scalar.activation(dc, ps, AF.Sqrt)
                    # w = min(dc, 3); accum(min) -> rowmin (attract)
                    w = wp.tile([P, MTILE], BF16, name="w", tag="w")
                    nc.vector.tensor_scalar(
                        w, dc, CLASH, 3.4e38, OP.min, OP.min,
                        accum_out=mnB[:, k:k + 1])
                    if route == "A":
                        def mk_act(w=w, sw2B=sw2B, k=k):
                            def go():
                                wa = wap.tile([P, MTILE], BF16, name="wa",
                                              tag="wa")
                                nc.scalar.activation(
                                    wa, w, AF.Square, bias=neg3[:, 0:1],
                                    scale=1.0, accum_out=sw2B[:, k:k + 1])
                            return go
                        pend_act.append(mk_act())
                    else:
                        # vc = w - 3 = -relu(3-d)
                        pool_ = vcp if route == "P" else vcd
                        vc = pool_.tile([P, MTILE], BF16, name="vc", tag="vc")
                        nc.vector.tensor_scalar(
                            vc, w, CLASH, 0.0, OP.subtract, OP.min)
                        if route == "P":
                            v2 = v2pp.tile([P, MTILE], BF16, name="v2",
                                           tag="v2")
                            nc.gpsimd.tensor_tensor(v2, vc, vc, OP.mult)
                            def mk_sum(v2=v2, sw2B=sw2B, k=k):
                                def go():
                                    nc.vector.tensor_scalar(
                                        v2, v2, 3.4e38, 0.0, OP.min, OP.add,
                                        accum_out=sw2B[:, k:k + 1])
                                return go
                            pend_sum.append(mk_sum())
                        else:
                            v2 = v2d.tile([P, MTILE], BF16, name="v2",
                                          tag="v2")
                            nc.vector.tensor_tensor(v2, vc, vc, OP.mult)
                            nc.vector.tensor_scalar(
                                v2, v2, 3.4e38, 0.0, OP.min, OP.add,
                                accum_out=sw2B[:, k:k + 1])
                    flush(pend_act, keep=5)
                    flush(pend_sum, keep=6)
                if c == 1:
                    flush(pend_epi)
            flush(pend_act)
            flush(pend_sum)

            # ---- per-batch epilogue, part 1 ----
            stack2 = work.tile([P, 2], F32, name=f"stack2{b}")
            nc.vector.tensor_reduce(stack2[:, 1:2], sw2B, AX.X, OP.add)
            vrep = None
            if b == 0:
                vBm = work.tile([P, NCHUNK], F32, name=f"vBm{b}")
                nc.vector.tensor_reduce(
                    vBm, mnB.rearrange("p (c j) -> p c j", c=NCHUNK),
                    AX.X, OP.min)
                # rank selection ignores element order, so flatten [128, 8]
                # partition-major straight to DRAM and broadcast-read it
                # back with a zero partition stride.
                vBb = work.tile([P, NCHUNK], BF16, name=f"vBb{b}")
                nc.vector.tensor_copy(vBb, vBm)
                vfd = dpool.tile([1, N], BF16, name=f"vfd{b}")
                nc.sync.dma_start(
                    out=vfd[0:1, :].rearrange("o (p c) -> o p c", p=P),
                    in_=vBb)
                vrep = work.tile([P, N], BF16, name=f"vrep{b}")
                vfd_bcast = bass.AP(tensor=vfd.tensor, offset=vfd.offset,
                                    ap=[[0, P], vfd.ap[-1]])
                nc.sync.dma_start(out=vrep, in_=vfd_bcast)
            else:
                vBm = vBm8

            # part 2: rank selection + final combine; deferred into the
            # next batch's main loop so the DMA latency above is hidden
            def mk_epi(b=b, vBm=vBm, vrep=vrep, stack2=stack2,
                       vBm8=vBm8, rankE=rankE, vrepE=vrepE):
                def go():
                    rank8 = work.tile([P, NCHUNK], F32, name=f"rank8{b}")
                    if b == 0:
                        for c in range(NCHUNK):
                            nc.vector.tensor_scalar(
                                waste1k, vrep, vBm[:, c:c + 1], 0.0,
                                OP.is_lt, OP.add, accum_out=rank8[:, c:c + 1])
                    else:
                        NE = (NCHUNK - 1) * P
                        # chunk 7 rowmins land in vBm8[:, 7]
                        nc.vector.tensor_reduce(
                            vBm8[:, NCHUNK - 1:NCHUNK],
                            mnB[:, (NCHUNK - 1) * JPC:], AX.X, OP.min)
                        # its rank vs chunks 0-6
                        nc.vector.tensor_scalar(
                            waste1k[:, 0:NE], vrepE,
                            vBm8[:, NCHUNK - 1:NCHUNK], 0.0,
                            OP.is_lt, OP.add,
                            accum_out=rankE[:, NCHUNK - 1:NCHUNK])
                        # chunk-7 j-side: PE transpose [128,1]->[1,128]
                        # (PSUM is free now) + gpsimd broadcast
                        vtp = psum.tile([1, P], F32, name="vtp", tag="ps")
                        nc.tensor.transpose(vtp, vBm8[:, NCHUNK - 1:NCHUNK],
                                            ident)
                        vfl2 = work.tile([1, P], F32, name="vfl2")
                        nc.vector.tensor_copy(vfl2, vtp)
                        vrep2 = work.tile([P, P], F32, name="vrep2")
                        nc.gpsimd.partition_broadcast(vrep2, vfl2[0:1, :], P)
                        rankL = work.tile([P, NCHUNK], F32, name="rankL")
                        for c in range(NCHUNK):
                            nc.vector.tensor_scalar(
                                waste1k[:, 0:P], vrep2, vBm8[:, c:c + 1],
                                0.0, OP.is_lt, OP.add,
                                accum_out=rankL[:, c:c + 1])
                        nc.vector.tensor_add(rank8, rankE, rankL)
                    sel8 = work.tile([P, NCHUNK], F32, name=f"sel8{b}")
                    nc.vector.tensor_scalar(sel8, rank8, float(TOPK), None,
                                            OP.is_lt)
                    prod8 = work.tile([P, NCHUNK], F32, name=f"prod8{b}")
                    nc.vector.tensor_mul(prod8, sel8, vBm)
                    # sum of selected (rowmin-3); add 3 per selected row
                    # via the count
                    nc.vector.tensor_reduce(stack2[:, 0:1], prod8, AX.X,
                                            OP.add)
                    cnt8 = work.tile([P, 1], F32, name=f"cnt8{b}")
                    nc.vector.tensor_reduce(cnt8, sel8, AX.X, OP.add)
                    st3 = work.tile([P, 3], F32, name=f"st3{b}")
                    nc.vector.tensor_copy(st3[:, 0:2], stack2)
                    nc.vector.tensor_copy(st3[:, 2:3], cnt8)

                    # PE ones-matmul + final combine: deferred to after the
                    # main loops, when PE/PSUM are idle (a mid-stream fin
                    # matmul head-of-line-blocks the PE FIFO)
                    def fin_go(b=b, st3=st3):
                        finp = psum.tile([1, 3], F32, name="finp", tag="ps")
                        nc.tensor.matmul(finp, ones128, st3, start=True,
                                         stop=True)
                        fin = work.tile([1, 2], F32, name=f"fin{b}")
                        nc.vector.tensor_copy(fin, finp[0:1, 0:2])
                        fcnt = work.tile([1, 1], F32, name=f"fcnt{b}")
                        nc.vector.tensor_copy(fcnt, finp[0:1, 2:3])
                        en = work.tile([1, 2], F32, name=f"en{b}")
                        nc.vector.tensor_scalar(
                            en[0:1, 0:1], fcnt[0:1, 0:1],
                            CLASH * ATTRACT_SCALE / TOPK, None, OP.mult)
                        t0 = work.tile([1, 1], F32, name=f"t0{b}")
                        nc.vector.tensor_scalar(t0, fin[0:1, 0:1],
                                                ATTRACT_SCALE / TOPK, None,
                                                OP.mult)
                        nc.vector.tensor_scalar(en[0:1, 1:2], fin[0:1, 1:2],
                                                REPEL_SCALE, None, OP.mult)
                        en2 = work.tile([1, 1], F32, name=f"en2{b}")
                        nc.vector.tensor_add(en2, en[0:1, 0:1], en[0:1, 1:2])
                        en3 = work.tile([1, 1], F32, name=f"en3{b}")
                        nc.vector.tensor_add(en3, en2, t0)
                        nc.sync.dma_start(out=out[b:b + 1, 0:1],
                                          in_=en3[0:1, 0:1])
                    pend_fin.append(fin_go)
                return go
            pend_epi.append(mk_epi())
        flush(pend_epi)
        flush(pend_fin)

    nc.compile()
    return nc


def _get_program():
    if "nc" not in _prog_cache:
        _prog_cache["nc"] = build_program()
    return _prog_cache["nc"]


def make_in_maps(binder_trans, target_coords):
    x = np.asarray(binder_trans, dtype=np.float32)          # [B, N, 3]
    y = np.asarray(target_coords, dtype=np.float32)         # [M, 3]
    # lhsT per batch: [x0 x1 x2 ; ||x||^2 ; 1]  -> [B, 5, N]
    xT = x.transpose(0, 2, 1)                               # [B, 3, N]
    xsq = (x * x).sum(-1)[:, None, :]                       # [B, 1, N]
    ones_n = np.ones((B, 1, N), dtype=np.float32)
    lhs = np.ascontiguousarray(
        np.concatenate([xT, xsq, ones_n], axis=1))          # [B, 5, N]
    # rhs: [-2y0 -2y1 -2y2 ; 1 ; sum(y^2)+eps] -> [5, M]
    yT = -2.0 * y.T                                         # [3, M]
    ones_m = np.ones((1, MT), dtype=np.float32)
    ysq = (y * y).sum(-1)[None, :] + np.float32(EPS)        # [1, M]
    rhs = np.ascontiguousarray(
        np.concatenate([yT, ones_m, ysq], axis=0))          # [5, M]
    return [{"bnd": np.ascontiguousarray(lhs[c * BC:(c + 1) * BC]),
             "tgt": rhs}
            for c in range(NCORES)]


def kernel(binder_trans, target_coords):
    nc = _get_program()
    in_maps = make_in_maps(binder_trans, target_coords)
    res = run_bass_kernel_spmd(nc, in_maps, list(range(NCORES)))
    outs = [np.asarray(res.results[c]["out"], dtype=np.float32).reshape(BC)
            for c in range(NCORES)]
    return np.concatenate(outs).astype(np.float32)


# revision 7
# speedup vs baseline: 1.8120x; 1.0017x over previous
"""Trainium2 Bass kernel for BinderEnergyGuidance (retrieval_knn).

Math (per batch b of 16):
  d[b,n,m]   = ||binder[b,n] - target[m]||           (N=1024, M=8192)
  attract[b] = mean of the k=204 smallest per-row min-distances
  repel[b]   = sum relu(3 - d)^2
  out[b]     = 10*attract[b] + 5*repel[b]

Strategy: data-parallel over batch, 2 batches per core.  Host packs the
inputs so the device contraction is K=5:
  lhsT rows: [x0 x1 x2 ; ||x||^2 ; 1]        (per batch, [5, N])
  rhs  rows: [-2y0 -2y1 -2y2 ; 1 ; sum(y^2)+eps]   ([5, M])
so one fp32r matmul emits d2 + eps directly into PSUM.

Per 2048-wide m-tile (64 per core):
  PE : 4x 512-col matmuls -> PSUM fp32 d2 (2 PSUM tiles = all 8 banks)
  Act: dc = Sqrt(d2) PSUM->SBUF bf16 (eps pre-added keeps d2 > 0)
  DVE tsA: w = min(dc,3), accum(min) -> per-row min dist (attract).
  repel integrand (3-w)^2, summed per row, via 3 balanced routes:
    'A': Act Square(w - 3) + accum          (Act only)
    'D': vc = w-3 (ts); vc*vc (tt); ts accum (DVE only)
    'P': vc = w-3 (ts); vc*vc on gpsimd tt; ts accum (Pool + DVE)
  Route counts chosen so Act/DVE/Pool all land at ~140us busy.

Note: tensor_scalar with accum_out applies only op0 to the written
output; op1 becomes the accumulation op.  All op1 choices here are
no-ops on the output.

Epilogue (per batch): rowmins [128,8] -> +3 -> bf16 pad to [128,128] ->
XBAR DMA transpose -> flatten -> gpsimd partition_broadcast -> top-k by
rank (count of strictly-smaller values) -> select/dot; final partition
sums via gpsimd partition_all_reduce (no PSUM use outside the matmuls).

Self-contained: hardcodes shapes for binder[16,1024,3], target[8192,3].
"""

import numpy as np
from contextlib import ExitStack

import concourse.bass as bass
import concourse.bacc as bacc
import concourse.tile as tile
from concourse import mybir, bass_isa
from concourse.masks import make_identity
from concourse.bass_utils import run_bass_kernel_spmd

F32 = mybir.dt.float32
F32R = mybir.dt.float32r
BF16 = mybir.dt.bfloat16
AF = mybir.ActivationFunctionType
OP = mybir.AluOpType
AX = mybir.AxisListType
RED = bass_isa.ReduceOp

B, N, MT = 16, 1024, 8192
NCORES = 8
BC = B // NCORES          # batches per core
TOPK = 204                # int(0.2 * N)
CLASH = 3.0
EPS = 1e-3                # guards sqrt against fp-rounding-negative d2
ATTRACT_SCALE, REPEL_SCALE = 10.0, 5.0

P = 128                   # SBUF partitions
NCHUNK = N // P           # 8 row-chunks per batch
MTILE = 2048              # PSUM tile free size (4 banks)
JPC = MT // MTILE         # m-tiles per chunk (4)
MMF = 512                 # fp32 matmul max moving free size
KP = 5                    # packed contraction size

# square-route assignment per (batch, m-tile) — 32 tiles per batch.
# 'P' gpsimd, 'A' scalar-engine Square, 'D' vector engine.
# Batch 1 keeps its last chunks gpsimd-free so Pool drains before the
# final epilogue instead of 25us after the last sqrt.


def _routes():
    b0 = []
    for k in range(32):
        if k % 2 == 0:
            b0.append("P")
        elif k in (9, 25):
            b0.append("A")
        else:
            b0.append("D")
    b1 = []
    for k in range(32):
        if k < 20:
            b1.append("P" if k % 5 != 4 else "D")
        elif k < 24:
            b1.append("D")
        elif k < 30 or k == 31:
            b1.append("A")
        else:
            b1.append("D")
    return [b0, b1]


ROUTES = _routes()

_prog_cache = {}


def build_program():
    nc = bacc.Bacc("TRN2", target_bir_lowering=False, debug=False,
                   num_devices=NCORES)
    bnd = nc.dram_tensor("bnd", [BC, KP, N], F32R, kind="ExternalInput").ap()
    tgt = nc.dram_tensor("tgt", [KP, MT], F32R, kind="ExternalInput").ap()
    out = nc.dram_tensor("out", [BC, 1], F32, kind="ExternalOutput").ap()

    with tile.TileContext(nc) as tc, ExitStack() as ctx:
        consts = ctx.enter_context(tc.tile_pool(name="consts", bufs=1))
        work = ctx.enter_context(tc.tile_pool(name="work", bufs=1))
        dcp = ctx.enter_context(tc.tile_pool(name="dcp", bufs=6))
        wp = ctx.enter_context(tc.tile_pool(name="wp", bufs=6))
        vcd = ctx.enter_context(tc.tile_pool(name="vcd", bufs=3))
        vcp = ctx.enter_context(tc.tile_pool(name="vcp", bufs=8))
        wap = ctx.enter_context(tc.tile_pool(name="wap", bufs=3))
        v2d = ctx.enter_context(tc.tile_pool(name="v2d", bufs=3))
        v2pp = ctx.enter_context(tc.tile_pool(name="v2pp", bufs=6))
        psum = ctx.enter_context(tc.tile_pool(name="psum", bufs=2, space="PSUM"))
        dpool = ctx.enter_context(tc.tile_pool(name="dpool", bufs=1, space="DRAM"))

        # ---- inputs: pure DMA, no engine preamble.  lhsT0 + first rhs
        # chunk gate the first matmul, so issue them first on separate
        # queues. ----
        rhs = consts.tile([KP, MT], F32R)
        lhsTs = [consts.tile([KP, N], F32R, name=f"lhsT{b}") for b in range(BC)]
        nc.sync.dma_start(out=lhsTs[0][:, :], in_=bnd[0, :, :])
        nc.scalar.dma_start(out=rhs[:, 0:512], in_=tgt[:, 0:512])
        nc.sync.dma_start(out=rhs[:, 512:2048], in_=tgt[:, 512:2048])
        nc.scalar.dma_start(out=rhs[:, 2048:4096], in_=tgt[:, 2048:4096])
        nc.sync.dma_start(out=rhs[:, 4096:6144], in_=tgt[:, 4096:6144])
        nc.scalar.dma_start(out=rhs[:, 6144:8192], in_=tgt[:, 6144:8192])
        nc.sync.dma_start(out=lhsTs[1][:, :], in_=bnd[1, :, :])

        # shared elementwise-output scratch (values never read)
        waste1k = work.tile([P, N], BF16)
        neg3 = consts.tile([P, 1], F32)
        nc.vector.memset(neg3, -CLASH)
        ones128 = consts.tile([P, 1], F32)
        nc.vector.memset(ones128, 1.0)
        ident = consts.tile([P, P], F32)
        make_identity(nc, ident)

        # Engine queues are strict FIFO, so an op that waits on a slow
        # producer (gpsimd square, epilogue DMA chain) must not be emitted
        # right after it or it head-of-line-blocks the whole engine.
        # Deferred emission queues provide the lag.
        pend_sum = []     # P-route row-sum ts ops waiting for gpsimd
        pend_act = []     # A-route Square ops, lag one tile
        pend_epi = []     # previous batch's epilogue part 2
        pend_fin = []     # final partition-sum + output, all batches
        pend_rank = []    # b1 split-rank: early ranks vs chunks 0-6

        def flush(q, keep=0):
            while len(q) > keep:
                q.pop(0)()

        for b in range(BC):
            lhsT = lhsTs[b]
            sw2B = work.tile([P, NCHUNK * JPC], F32, name=f"sw2B{b}")
            mnB = work.tile([P, NCHUNK * JPC], F32, name=f"mnB{b}")

            NE = (NCHUNK - 1) * P          # 896: rows in chunks 0-6
            vBm8 = work.tile([P, NCHUNK], F32, name=f"vBm8{b}")
            rankE = work.tile([P, NCHUNK], F32, name=f"rankE{b}")
            vrepE = work.tile([P, NE], BF16, name=f"vrepE{b}")
            for c in range(NCHUNK):
                if b == 1 and c == NCHUNK - 1:
                    # ---- split-rank early piece: chunks 0-6 are final;
                    # flatten+broadcast their rowmins and rank against
                    # them while chunk 7 computes ----
                    nc.vector.tensor_reduce(
                        vBm8[:, 0:NCHUNK - 1],
                        mnB[:, 0:(NCHUNK - 1) * JPC].rearrange(
                            "p (c j) -> p c j", c=NCHUNK - 1),
                        AX.X, OP.min)
                    vBbE = work.tile([P, NCHUNK - 1], BF16, name="vBbE")
                    nc.vector.tensor_copy(vBbE, vBm8[:, 0:NCHUNK - 1])
                    vfdE = dpool.tile([1, NE], BF16, name="vfdE")
                    nc.sync.dma_start(
                        out=vfdE[0:1, :].rearrange("o (p c) -> o p c", p=P),
                        in_=vBbE)
                    vfdE_b = bass.AP(tensor=vfdE.tensor, offset=vfdE.offset,
                                     ap=[[0, P], vfdE.ap[-1]])
                    nc.sync.dma_start(out=vrepE, in_=vfdE_b)
                    for cc in range(NCHUNK - 1):
                        def mk_rank(cc=cc):
                            def go():
                                nc.vector.tensor_scalar(
                                    waste1k[:, 0:NE], vrepE,
                                    vBm8[:, cc:cc + 1], 0.0,
                                    OP.is_lt, OP.add,
                                    accum_out=rankE[:, cc:cc + 1])
                            return go
                        pend_rank.append(mk_rank())
                lc = lhsT[:, c * P:(c + 1) * P]
                for j in range(JPC):
                    if b == 1 and c == NCHUNK - 1 and j >= 2:
                        flush(pend_rank, keep=(4 if j == 2 else 0))
                    k = c * JPC + j
                    route = ROUTES[b][k]
                    ps = psum.tile([P, MTILE], F32, name="ps", tag="ps")
                    for q in range(MTILE // MMF):
                        nc.tensor.matmul(
                            ps[:, q * MMF:(q + 1) * MMF], lc,
                            rhs[:, j * MTILE + q * MMF:
                                j * MTILE + (q + 1) * MMF],
                            start=True, stop=True)
                    dc = dcp.tile([P, MTILE], BF16, name="dc", tag="dc")
                    nc.scalar.activation(dc, ps, AF.Sqrt)
                    # w = min(dc, 3); accum(min) -> rowmin (attract)
                    w = wp.tile([P, MTILE], BF16, name="w", tag="w")
                    nc.vector.tensor_scalar(
                        w, dc, CLASH, 3.4e38, OP.min, OP.min,
                        accum_out=mnB[:, k:k + 1])
                    if route == "A":
                        def mk_act(w=w, sw2B=sw2B, k=k):
                            def go():
                                wa = wap.tile([P, MTILE], BF16, name="wa",
                                              tag="wa")
                                nc.scalar.activation(
                                    wa, w, AF.Square, bias=neg3[:, 0:1],
                                    scale=1.0, accum_out=sw2B[:, k:k + 1])
                            return go
                        pend_act.append(mk_act())
                    else:
                        # vc = w - 3 = -relu(3-d)
                        pool_ = vcp if route == "P" else vcd
                        vc = pool_.tile([P, MTILE], BF16, name="vc", tag="vc")
                        nc.vector.tensor_scalar(
                            vc, w, CLASH, 0.0, OP.subtract, OP.min)
                        if route == "P":
                            v2 = v2pp.tile([P, MTILE], BF16, name="v2",
                                           tag="v2")
                            nc.gpsimd.tensor_tensor(v2, vc, vc, OP.mult)
                            def mk_sum(v2=v2, sw2B=sw2B, k=k):
                                def go():
                                    nc.vector.tensor_scalar(
                                        v2, v2, 3.4e38, 0.0, OP.min, OP.add,
                                        accum_out=sw2B[:, k:k + 1])
                                return go
                            pend_sum.append(mk_sum())
                        else:
                            v2 = v2d.tile([P, MTILE], BF16, name="v2",
                                          tag="v2")
                            nc.vector.tensor_tensor(v2, vc, vc, OP.mult)
                            nc.vector.tensor_scalar(
                                v2, v2, 3.4e38, 0.0, OP.min, OP.add,
                                accum_out=sw2B[:, k:k + 1])
                    flush(pend_act, keep=5)
                    flush(pend_sum, keep=6)
                if c == 1:
                    flush(pend_epi)
            flush(pend_act)
            flush(pend_sum)

            # ---- per-batch epilogue, part 1 ----
            stack2 = work.tile([P, 2], F32, name=f"stack2{b}")
            nc.vector.tensor_reduce(stack2[:, 1:2], sw2B, AX.X, OP.add)
            vrep = None
            if b == 0:
                vBm = work.tile([P, NCHUNK], F32, name=f"vBm{b}")
                nc.vector.tensor_reduce(
                    vBm, mnB.rearrange("p (c j) -> p c j", c=NCHUNK),
                    AX.X, OP.min)
                # rank selection ignores element order, so flatten [128, 8]
                # partition-major straight to DRAM and broadcast-read it
                # back with a zero partition stride.
                vBb = work.tile([P, NCHUNK], BF16, name=f"vBb{b}")
                nc.vector.tensor_copy(vBb, vBm)
                vfd = dpool.tile([1, N], BF16, name=f"vfd{b}")
                nc.sync.dma_start(
                    out=vfd[0:1, :].rearrange("o (p c) -> o p c", p=P),
                    in_=vBb)
                vrep = work.tile([P, N], BF16, name=f"vrep{b}")
                vfd_bcast = bass.AP(tensor=vfd.tensor, offset=vfd.offset,
                                    ap=[[0, P], vfd.ap[-1]])
                nc.sync.dma_start(out=vrep, in_=vfd_bcast)
            else:
                vBm = vBm8

            # part 2: rank selection + final combine; deferred into the
            # next batch's main loop so the DMA latency above is hidden
            def mk_epi(b=b, vBm=vBm, vrep=vrep, stack2=stack2,
                       vBm8=vBm8, rankE=rankE, vrepE=vrepE):
                def go():
                    rank8 = work.tile([P, NCHUNK], F32, name=f"rank8{b}")
                    if b == 0:
                        for c in range(NCHUNK):
                            nc.vector.tensor_scalar(
                                waste1k, vrep, vBm[:, c:c + 1], 0.0,
                                OP.is_lt, OP.add, accum_out=rank8[:, c:c + 1])
                    else:
                        NE = (NCHUNK - 1) * P
                        # chunk 7 rowmins land in vBm8[:, 7]
                        nc.vector.tensor_reduce(
                            vBm8[:, NCHUNK - 1:NCHUNK],
                            mnB[:, (NCHUNK - 1) * JPC:], AX.X, OP.min)
                        # its rank vs chunks 0-6
                        nc.vector.tensor_scalar(
                            waste1k[:, 0:NE], vrepE,
                            vBm8[:, NCHUNK - 1:NCHUNK], 0.0,
                            OP.is_lt, OP.add,
                            accum_out=rankE[:, NCHUNK - 1:NCHUNK])
                        # chunk-7 j-side: PE transpose [128,1]->[1,128]
                        # (PSUM is free now) + gpsimd broadcast
                        vtp = psum.tile([1, P], F32, name="vtp", tag="ps")
                        nc.tensor.transpose(vtp, vBm8[:, NCHUNK - 1:NCHUNK],
                                            ident)
                        vfl2 = work.tile([1, P], F32, name="vfl2")
                        nc.vector.tensor_copy(vfl2, vtp)
                        vrep2 = work.tile([P, P], F32, name="vrep2")
                        nc.gpsimd.partition_broadcast(vrep2, vfl2[0:1, :], P)
                        rankL = work.tile([P, NCHUNK], F32, name="rankL")
                        for c in range(NCHUNK):
                            nc.vector.tensor_scalar(
                                waste1k[:, 0:P], vrep2, vBm8[:, c:c + 1],
                                0.0, OP.is_lt, OP.add,
                                accum_out=rankL[:, c:c + 1])
                        nc.vector.tensor_add(rank8, rankE, rankL)
                    sel8 = work.tile([P, NCHUNK], F32, name=f"sel8{b}")
                    nc.vector.tensor_scalar(sel8, rank8, float(TOPK), None,
                                            OP.is_lt)
                    prod8 = work.tile([P, NCHUNK], F32, name=f"prod8{b}")
                    nc.vector.tensor_mul(prod8, sel8, vBm)
                    # sum of selected (rowmin-3); add 3 per selected row
                    # via the count
                    nc.vector.tensor_reduce(stack2[:, 0:1], prod8, AX.X,
                                            OP.add)
                    cnt8 = work.tile([P, 1], F32, name=f"cnt8{b}")
                    nc.vector.tensor_reduce(cnt8, sel8, AX.X, OP.add)
                    st3 = work.tile([P, 3], F32, name=f"st3{b}")
                    nc.vector.tensor_copy(st3[:, 0:2], stack2)
                    nc.vector.tensor_copy(st3[:, 2:3], cnt8)

                    # PE ones-matmul + final combine: deferred to after the
                    # main loops, when PE/PSUM are idle (a mid-stream fin
                    # matmul head-of-line-blocks the PE FIFO)
                    def fin_go(b=b, st3=st3):
                        finp = psum.tile([1, 3], F32, name="finp", tag="ps")
                        nc.tensor.matmul(finp, ones128, st3, start=True,
                                         stop=True)
                        fin = work.tile([1, 2], F32, name=f"fin{b}")
                        nc.vector.tensor_copy(fin, finp[0:1, 0:2])
                        fcnt = work.tile([1, 1], F32, name=f"fcnt{b}")
                        nc.vector.tensor_copy(fcnt, finp[0:1, 2:3])
                        en = work.tile([1, 2], F32, name=f"en{b}")
                        nc.vector.tensor_scalar(
                            en[0:1, 0:1], fcnt[0:1, 0:1],
                            CLASH * ATTRACT_SCALE / TOPK, None, OP.mult)
                        t0 = work.tile([1, 1], F32, name=f"t0{b}")
                        nc.vector.tensor_scalar(t0, fin[0:1, 0:1],
                                                ATTRACT_SCALE / TOPK, None,
                                                OP.mult)
                        nc.vector.tensor_scalar(en[0:1, 1:2], fin[0:1, 1:2],
                                                REPEL_SCALE, None, OP.mult)
                        en2 = work.tile([1, 1], F32, name=f"en2{b}")
                        nc.vector.tensor_add(en2, en[0:1, 0:1], en[0:1, 1:2])
                        en3 = work.tile([1, 1], F32, name=f"en3{b}")
                        nc.vector.tensor_add(en3, en2, t0)
                        nc.sync.dma_start(out=out[b:b + 1, 0:1],
                                          in_=en3[0:1, 0:1])
                    pend_fin.append(fin_go)
                return go
            pend_epi.append(mk_epi())
        flush(pend_epi)
        flush(pend_fin)

    nc.compile()
    return nc


def _get_program():
    if "nc" not in _prog_cache:
        _prog_cache["nc"] = build_program()
    return _prog_cache["nc"]


def make_in_maps(binder_trans, target_coords):
    x = np.asarray(binder_trans, dtype=np.float32)          # [B, N, 3]
    y = np.asarray(target_coords, dtype=np.float32)         # [M, 3]
    # lhsT per batch: [x0 x1 x2 ; ||x||^2 ; 1]  -> [B, 5, N]
    xT = x.transpose(0, 2, 1)                               # [B, 3, N]
    xsq = (x * x).sum(-1)[:, None, :]                       # [B, 1, N]
    ones_n = np.ones((B, 1, N), dtype=np.float32)
    lhs = np.ascontiguousarray(
        np.concatenate([xT, xsq, ones_n], axis=1))          # [B, 5, N]
    # rhs: [-2y0 -2y1 -2y2 ; 1 ; sum(y^2)+eps] -> [5, M]
    yT = -2.0 * y.T                                         # [3, M]
    ones_m = np.ones((1, MT), dtype=np.float32)
    ysq = (y * y).sum(-1)[None, :] + np.float32(EPS)        # [1, M]
    rhs = np.ascontiguousarray(
        np.concatenate([yT, ones_m, ysq], axis=0))          # [5, M]
    return [{"bnd": np.ascontiguousarray(lhs[c * BC:(c + 1) * BC]),
             "tgt": rhs}
            for c in range(NCORES)]


def kernel(binder_trans, target_coords):
    nc = _get_program()
    in_maps = make_in_maps(binder_trans, target_coords)
    res = run_bass_kernel_spmd(nc, in_maps, list(range(NCORES)))
    outs = [np.asarray(res.results[c]["out"], dtype=np.float32).reshape(BC)
            for c in range(NCORES)]
    return np.concatenate(outs).astype(np.float32)


# revision 8
# speedup vs baseline: 1.8366x; 1.0136x over previous
"""Trainium2 Bass kernel for BinderEnergyGuidance (retrieval_knn).

Math (per batch b of 16):
  d[b,n,m]   = ||binder[b,n] - target[m]||           (N=1024, M=8192)
  attract[b] = mean of the k=204 smallest per-row min-distances
  repel[b]   = sum relu(3 - d)^2
  out[b]     = 10*attract[b] + 5*repel[b]

Strategy: data-parallel over batch, 2 batches per core.  Host packs the
inputs so the device contraction is K=5:
  lhsT rows: [x0 x1 x2 ; ||x||^2 ; 1]        (per batch, [5, N])
  rhs  rows: [-2y0 -2y1 -2y2 ; 1 ; sum(y^2)+eps]   ([5, M])
so one fp32r matmul emits d2 + eps directly into PSUM.

Per 2048-wide m-tile (64 per core):
  PE : 4x 512-col matmuls -> PSUM fp32 d2 (2 PSUM tiles = all 8 banks)
  Act: dc = Sqrt(d2) PSUM->SBUF bf16 (eps pre-added keeps d2 > 0)
  DVE tsA: w = min(dc,3), accum(min) -> per-row min dist (attract).
  repel integrand (3-w)^2, summed per row, via 3 balanced routes:
    'A': Act Square(w - 3) + accum          (Act only)
    'D': vc = w-3 (ts); vc*vc (tt); ts accum (DVE only)
    'P': vc = w-3 (ts); vc*vc on gpsimd tt; ts accum (Pool + DVE)
  Route counts chosen so Act/DVE/Pool all land at ~140us busy.

Note: tensor_scalar with accum_out applies only op0 to the written
output; op1 becomes the accumulation op.  All op1 choices here are
no-ops on the output.

Epilogue (per batch): rowmins [128,8] -> +3 -> bf16 pad to [128,128] ->
XBAR DMA transpose -> flatten -> gpsimd partition_broadcast -> top-k by
rank (count of strictly-smaller values) -> select/dot; final partition
sums via gpsimd partition_all_reduce (no PSUM use outside the matmuls).

Self-contained: hardcodes shapes for binder[16,1024,3], target[8192,3].
"""

import numpy as np
from contextlib import ExitStack

import concourse.bass as bass
import concourse.bacc as bacc
import concourse.tile as tile
from concourse import mybir, bass_isa
from concourse.masks import make_identity
from concourse.bass_utils import run_bass_kernel_spmd

F32 = mybir.dt.float32
F32R = mybir.dt.float32r
BF16 = mybir.dt.bfloat16
AF = mybir.ActivationFunctionType
OP = mybir.AluOpType
AX = mybir.AxisListType
RED = bass_isa.ReduceOp

B, N, MT = 16, 1024, 8192
NCORES = 8
BC = B // NCORES          # batches per core
TOPK = 204                # int(0.2 * N)
CLASH = 3.0
EPS = 1e-3                # guards sqrt against fp-rounding-negative d2
ATTRACT_SCALE, REPEL_SCALE = 10.0, 5.0

P = 128                   # SBUF partitions
NCHUNK = N // P           # 8 row-chunks per batch
MTILE = 2048              # PSUM tile free size (4 banks)
JPC = MT // MTILE         # m-tiles per chunk (4)
MMF = 512                 # fp32 matmul max moving free size
KP = 5                    # packed contraction size

# square-route assignment per (batch, m-tile) — 32 tiles per batch.
# 'P' gpsimd, 'A' scalar-engine Square, 'D' vector engine.
# Batch 1 keeps its last chunks gpsimd-free so Pool drains before the
# final epilogue instead of 25us after the last sqrt.


def _routes():
    b0 = []
    for k in range(32):
        if k % 2 == 0:
            b0.append("P")
        elif k in (9, 25):
            b0.append("A")
        else:
            b0.append("D")
    b1 = []
    for k in range(32):
        if k < 20:
            b1.append("P" if k % 5 != 4 else "D")
        elif k < 24:
            b1.append("D")
        elif k < 30 or k == 31:
            b1.append("A")
        else:
            b1.append("D")
    return [b0, b1]


ROUTES = _routes()

_prog_cache = {}


def build_program():
    nc = bacc.Bacc("TRN2", target_bir_lowering=False, debug=False,
                   num_devices=NCORES)
    bnd = nc.dram_tensor("bnd", [BC, KP, N], F32R, kind="ExternalInput").ap()
    tgt = nc.dram_tensor("tgt", [KP, MT], F32R, kind="ExternalInput").ap()
    out = nc.dram_tensor("out", [BC, 1], F32, kind="ExternalOutput").ap()

    with tile.TileContext(nc) as tc, ExitStack() as ctx:
        consts = ctx.enter_context(tc.tile_pool(name="consts", bufs=1))
        work = ctx.enter_context(tc.tile_pool(name="work", bufs=1))
        dcp = ctx.enter_context(tc.tile_pool(name="dcp", bufs=8))
        wp = ctx.enter_context(tc.tile_pool(name="wp", bufs=6))
        vcd = ctx.enter_context(tc.tile_pool(name="vcd", bufs=3))
        vcp = ctx.enter_context(tc.tile_pool(name="vcp", bufs=8))
        wap = ctx.enter_context(tc.tile_pool(name="wap", bufs=3))
        v2d = ctx.enter_context(tc.tile_pool(name="v2d", bufs=3))
        v2pp = ctx.enter_context(tc.tile_pool(name="v2pp", bufs=6))
        psum = ctx.enter_context(tc.tile_pool(name="psum", bufs=2, space="PSUM"))
        dpool = ctx.enter_context(tc.tile_pool(name="dpool", bufs=1, space="DRAM"))

        # ---- inputs: pure DMA, no engine preamble.  lhsT0 + first rhs
        # chunk gate the first matmul, so issue them first on separate
        # queues. ----
        rhs = consts.tile([KP, MT], F32R)
        lhsTs = [consts.tile([KP, N], F32R, name=f"lhsT{b}") for b in range(BC)]
        nc.sync.dma_start(out=lhsTs[0][:, :], in_=bnd[0, :, :])
        nc.scalar.dma_start(out=rhs[:, 0:512], in_=tgt[:, 0:512])
        nc.sync.dma_start(out=rhs[:, 512:2048], in_=tgt[:, 512:2048])
        nc.scalar.dma_start(out=rhs[:, 2048:4096], in_=tgt[:, 2048:4096])
        nc.sync.dma_start(out=rhs[:, 4096:6144], in_=tgt[:, 4096:6144])
        nc.scalar.dma_start(out=rhs[:, 6144:8192], in_=tgt[:, 6144:8192])
        nc.sync.dma_start(out=lhsTs[1][:, :], in_=bnd[1, :, :])

        # shared elementwise-output scratch (values never read)
        waste1k = work.tile([P, N], BF16)
        neg3 = consts.tile([P, 1], F32)
        nc.vector.memset(neg3, -CLASH)
        ones128 = consts.tile([P, 1], F32)
        nc.vector.memset(ones128, 1.0)
        ident = consts.tile([P, P], F32)
        make_identity(nc, ident)

        # Engine queues are strict FIFO, so an op that waits on a slow
        # producer (gpsimd square, epilogue DMA chain) must not be emitted
        # right after it or it head-of-line-blocks the whole engine.
        # Deferred emission queues provide the lag.
        pend_sum = []     # P-route row-sum ts ops waiting for gpsimd
        pend_act = []     # A-route Square ops, lag one tile
        pend_epi = []     # previous batch's epilogue part 2
        pend_fin = []     # final partition-sum + output, all batches
        pend_rank = []    # b1 split-rank: early ranks vs chunks 0-6

        def flush(q, keep=0):
            while len(q) > keep:
                q.pop(0)()

        for b in range(BC):
            lhsT = lhsTs[b]
            sw2B = work.tile([P, NCHUNK * JPC], F32, name=f"sw2B{b}")
            mnB = work.tile([P, NCHUNK * JPC], F32, name=f"mnB{b}")

            NE = (NCHUNK - 1) * P          # 896: rows in chunks 0-6
            vBm8 = work.tile([P, NCHUNK], F32, name=f"vBm8{b}")
            rankE = work.tile([P, NCHUNK], F32, name=f"rankE{b}")
            vrepE = work.tile([P, NE], BF16, name=f"vrepE{b}")
            for c in range(NCHUNK):
                if b == 1 and c == NCHUNK - 1:
                    # ---- split-rank early piece: chunks 0-6 are final;
                    # flatten+broadcast their rowmins and rank against
                    # them while chunk 7 computes ----
                    nc.vector.tensor_reduce(
                        vBm8[:, 0:NCHUNK - 1],
                        mnB[:, 0:(NCHUNK - 1) * JPC].rearrange(
                            "p (c j) -> p c j", c=NCHUNK - 1),
                        AX.X, OP.min)
                    vBbE = work.tile([P, NCHUNK - 1], BF16, name="vBbE")
                    nc.vector.tensor_copy(vBbE, vBm8[:, 0:NCHUNK - 1])
                    vfdE = dpool.tile([1, NE], BF16, name="vfdE")
                    nc.sync.dma_start(
                        out=vfdE[0:1, :].rearrange("o (p c) -> o p c", p=P),
                        in_=vBbE)
                    vfdE_b = bass.AP(tensor=vfdE.tensor, offset=vfdE.offset,
                                     ap=[[0, P], vfdE.ap[-1]])
                    nc.sync.dma_start(out=vrepE, in_=vfdE_b)
                    for cc in range(NCHUNK - 1):
                        def mk_rank(cc=cc):
                            def go():
                                nc.vector.tensor_scalar(
                                    waste1k[:, 0:NE], vrepE,
                                    vBm8[:, cc:cc + 1], 0.0,
                                    OP.is_lt, OP.add,
                                    accum_out=rankE[:, cc:cc + 1])
                            return go
                        pend_rank.append(mk_rank())
                lc = lhsT[:, c * P:(c + 1) * P]
                for j in range(JPC):
                    if b == 1 and c == NCHUNK - 1 and j >= 2:
                        flush(pend_rank, keep=(4 if j == 2 else 0))
                    k = c * JPC + j
                    route = ROUTES[b][k]
                    ps = psum.tile([P, MTILE], F32, name="ps", tag="ps")
                    for q in range(MTILE // MMF):
                        nc.tensor.matmul(
                            ps[:, q * MMF:(q + 1) * MMF], lc,
                            rhs[:, j * MTILE + q * MMF:
                                j * MTILE + (q + 1) * MMF],
                            start=True, stop=True)
                    dc = dcp.tile([P, MTILE], BF16, name="dc", tag="dc")
                    nc.scalar.activation(dc, ps, AF.Sqrt)
                    # w = min(dc, 3); accum(min) -> rowmin (attract)
                    w = wp.tile([P, MTILE], BF16, name="w", tag="w")
                    nc.vector.tensor_scalar(
                        w, dc, CLASH, 3.4e38, OP.min, OP.min,
                        accum_out=mnB[:, k:k + 1])
                    if route == "A":
                        def mk_act(w=w, sw2B=sw2B, k=k):
                            def go():
                                wa = wap.tile([P, MTILE], BF16, name="wa",
                                              tag="wa")
                                nc.scalar.activation(
                                    wa, w, AF.Square, bias=neg3[:, 0:1],
                                    scale=1.0, accum_out=sw2B[:, k:k + 1])
                            return go
                        pend_act.append(mk_act())
                    else:
                        # vc = w - 3 = -relu(3-d)
                        pool_ = vcp if route == "P" else vcd
                        vc = pool_.tile([P, MTILE], BF16, name="vc", tag="vc")
                        nc.vector.tensor_scalar(
                            vc, w, CLASH, 0.0, OP.subtract, OP.min)
                        if route == "P":
                            v2 = v2pp.tile([P, MTILE], BF16, name="v2",
                                           tag="v2")
                            nc.gpsimd.tensor_tensor(v2, vc, vc, OP.mult)
                            def mk_sum(v2=v2, sw2B=sw2B, k=k):
                                def go():
                                    nc.vector.tensor_scalar(
                                        v2, v2, 3.4e38, 0.0, OP.min, OP.add,
                                        accum_out=sw2B[:, k:k + 1])
                                return go
                            pend_sum.append(mk_sum())
                        else:
                            v2 = v2d.tile([P, MTILE], BF16, name="v2",
                                          tag="v2")
                            nc.vector.tensor_tensor(v2, vc, vc, OP.mult)
                            nc.vector.tensor_scalar(
                                v2, v2, 3.4e38, 0.0, OP.min, OP.add,
                                accum_out=sw2B[:, k:k + 1])
                    flush(pend_act, keep=5)
                    flush(pend_sum, keep=6)
                if c == 1:
                    flush(pend_epi)
            flush(pend_act)
            flush(pend_sum)

            # ---- per-batch epilogue, part 1 ----
            stack2 = work.tile([P, 2], F32, name=f"stack2{b}")
            nc.vector.tensor_reduce(stack2[:, 1:2], sw2B, AX.X, OP.add)
            vrep = None
            if b == 0:
                vBm = work.tile([P, NCHUNK], F32, name=f"vBm{b}")
                nc.vector.tensor_reduce(
                    vBm, mnB.rearrange("p (c j) -> p c j", c=NCHUNK),
                    AX.X, OP.min)
                # rank selection ignores element order, so flatten [128, 8]
                # partition-major straight to DRAM and broadcast-read it
                # back with a zero partition stride.
                vBb = work.tile([P, NCHUNK], BF16, name=f"vBb{b}")
                nc.vector.tensor_copy(vBb, vBm)
                vfd = dpool.tile([1, N], BF16, name=f"vfd{b}")
                nc.sync.dma_start(
                    out=vfd[0:1, :].rearrange("o (p c) -> o p c", p=P),
                    in_=vBb)
                vrep = work.tile([P, N], BF16, name=f"vrep{b}")
                vfd_bcast = bass.AP(tensor=vfd.tensor, offset=vfd.offset,
                                    ap=[[0, P], vfd.ap[-1]])
                nc.sync.dma_start(out=vrep, in_=vfd_bcast)
            else:
                vBm = vBm8

            # part 2: rank selection + final combine; deferred into the
            # next batch's main loop so the DMA latency above is hidden
            def mk_epi(b=b, vBm=vBm, vrep=vrep, stack2=stack2,
                       vBm8=vBm8, rankE=rankE, vrepE=vrepE):
                def go():
                    rank8 = work.tile([P, NCHUNK], F32, name=f"rank8{b}")
                    if b == 0:
                        for c in range(NCHUNK):
                            nc.vector.tensor_scalar(
                                waste1k, vrep, vBm[:, c:c + 1], 0.0,
                                OP.is_lt, OP.add, accum_out=rank8[:, c:c + 1])
                    else:
                        NE = (NCHUNK - 1) * P
                        # chunk 7 rowmins land in vBm8[:, 7]
                        nc.vector.tensor_reduce(
                            vBm8[:, NCHUNK - 1:NCHUNK],
                            mnB[:, (NCHUNK - 1) * JPC:], AX.X, OP.min)
                        # its rank vs chunks 0-6
                        nc.vector.tensor_scalar(
                            waste1k[:, 0:NE], vrepE,
                            vBm8[:, NCHUNK - 1:NCHUNK], 0.0,
                            OP.is_lt, OP.add,
                            accum_out=rankE[:, NCHUNK - 1:NCHUNK])
                        # chunk-7 j-side: PE transpose [128,1]->[1,128]
                        # (PSUM is free now) + gpsimd broadcast
                        vtp = psum.tile([1, P], F32, name="vtp", tag="ps")
                        nc.tensor.transpose(vtp, vBm8[:, NCHUNK - 1:NCHUNK],
                                            ident)
                        vfl2 = work.tile([1, P], F32, name="vfl2")
                        nc.vector.tensor_copy(vfl2, vtp)
                        vrep2 = work.tile([P, P], F32, name="vrep2")
                        nc.gpsimd.partition_broadcast(vrep2, vfl2[0:1, :], P)
                        rankL = work.tile([P, NCHUNK], F32, name="rankL")
                        for c in range(NCHUNK):
                            nc.vector.tensor_scalar(
                                waste1k[:, 0:P], vrep2, vBm8[:, c:c + 1],
                                0.0, OP.is_lt, OP.add,
                                accum_out=rankL[:, c:c + 1])
                        nc.vector.tensor_add(rank8, rankE, rankL)
                    sel8 = work.tile([P, NCHUNK], F32, name=f"sel8{b}")
                    nc.vector.tensor_scalar(sel8, rank8, float(TOPK), None,
                                            OP.is_lt)
                    prod8 = work.tile([P, NCHUNK], F32, name=f"prod8{b}")
                    nc.vector.tensor_mul(prod8, sel8, vBm)
                    # sum of selected (rowmin-3); add 3 per selected row
                    # via the count
                    nc.vector.tensor_reduce(stack2[:, 0:1], prod8, AX.X,
                                            OP.add)
                    cnt8 = work.tile([P, 1], F32, name=f"cnt8{b}")
                    nc.vector.tensor_reduce(cnt8, sel8, AX.X, OP.add)
                    st3 = work.tile([P, 3], F32, name=f"st3{b}")
                    nc.vector.tensor_copy(st3[:, 0:2], stack2)
                    nc.vector.tensor_copy(st3[:, 2:3], cnt8)

                    # PE ones-matmul + final combine: deferred to after the
                    # main loops, when PE/PSUM are idle (a mid-stream fin
                    # matmul head-of-line-blocks the PE FIFO)
                    def fin_go(b=b, st3=st3):
                        finp = psum.tile([1, 3], F32, name="finp", tag="ps")
                        nc.tensor.matmul(finp, ones128, st3, start=True,
                                         stop=True)
                        fin = work.tile([1, 2], F32, name=f"fin{b}")
                        nc.vector.tensor_copy(fin, finp[0:1, 0:2])
                        fcnt = work.tile([1, 1], F32, name=f"fcnt{b}")
                        nc.vector.tensor_copy(fcnt, finp[0:1, 2:3])
                        en = work.tile([1, 2], F32, name=f"en{b}")
                        nc.vector.tensor_scalar(
                            en[0:1, 0:1], fcnt[0:1, 0:1],
                            CLASH * ATTRACT_SCALE / TOPK, None, OP.mult)
                        t0 = work.tile([1, 1], F32, name=f"t0{b}")
                        nc.vector.tensor_scalar(t0, fin[0:1, 0:1],
                                                ATTRACT_SCALE / TOPK, None,
                                                OP.mult)
                        nc.vector.tensor_scalar(en[0:1, 1:2], fin[0:1, 1:2],
                                                REPEL_SCALE, None, OP.mult)
                        en2 = work.tile([1, 1], F32, name=f"en2{b}")
                        nc.vector.tensor_add(en2, en[0:1, 0:1], en[0:1, 1:2])
                        en3 = work.tile([1, 1], F32, name=f"en3{b}")
                        nc.vector.tensor_add(en3, en2, t0)
                        nc.sync.dma_start(out=out[b:b + 1, 0:1],
                                          in_=en3[0:1, 0:1])
                    pend_fin.append(fin_go)
                return go
            pend_epi.append(mk_epi())
        flush(pend_epi)
        flush(pend_fin)

    nc.compile()
    return nc


def _get_program():
    if "nc" not in _prog_cache:
        _prog_cache["nc"] = build_program()
    return _prog_cache["nc"]


def make_in_maps(binder_trans, target_coords):
    x = np.asarray(binder_trans, dtype=np.float32)          # [B, N, 3]
    y = np.asarray(target_coords, dtype=np.float32)         # [M, 3]
    # lhsT per batch: [x0 x1 x2 ; ||x||^2 ; 1]  -> [B, 5, N]
    xT = x.transpose(0, 2, 1)                               # [B, 3, N]
    xsq = (x * x).sum(-1)[:, None, :]                       # [B, 1, N]
    ones_n = np.ones((B, 1, N), dtype=np.float32)
    lhs = np.ascontiguousarray(
        np.concatenate([xT, xsq, ones_n], axis=1))          # [B, 5, N]
    # rhs: [-2y0 -2y1 -2y2 ; 1 ; sum(y^2)+eps] -> [5, M]
    yT = -2.0 * y.T                                         # [3, M]
    ones_m = np.ones((1, MT), dtype=np.float32)
    ysq = (y * y).sum(-1)[None, :] + np.float32(EPS)        # [1, M]
    rhs = np.ascontiguousarray(
        np.concatenate([yT, ones_m, ysq], axis=0))          # [5, M]
    return [{"bnd": np.ascontiguousarray(lhs[c * BC:(c + 1) * BC]),
             "tgt": rhs}
            for c in range(NCORES)]


def kernel(binder_trans, target_coords):
    nc = _get_program()
    in_maps = make_in_maps(binder_trans, target_coords)
    res = run_bass_kernel_spmd(nc, in_maps, list(range(NCORES)))
    outs = [np.asarray(res.results[c]["out"], dtype=np.float32).reshape(BC)
            for c in range(NCORES)]
    return np.concatenate(outs).astype(np.float32)


# revision 9
# speedup vs baseline: 1.8634x; 1.0146x over previous
"""Trainium2 Bass kernel for BinderEnergyGuidance (retrieval_knn).

Math (per batch b of 16):
  d[b,n,m]   = ||binder[b,n] - target[m]||           (N=1024, M=8192)
  attract[b] = mean of the k=204 smallest per-row min-distances
  repel[b]   = sum relu(3 - d)^2
  out[b]     = 10*attract[b] + 5*repel[b]

Strategy: data-parallel over batch, 2 batches per core.  Host packs the
inputs so the device contraction is K=5:
  lhsT rows: [x0 x1 x2 ; ||x||^2 ; 1]        (per batch, [5, N])
  rhs  rows: [-2y0 -2y1 -2y2 ; 1 ; sum(y^2)+eps]   ([5, M])
so one fp32r matmul emits d2 + eps directly into PSUM.

Per 2048-wide m-tile (64 per core):
  PE : 4x 512-col matmuls -> PSUM fp32 d2 (2 PSUM tiles = all 8 banks)
  Act: dc = Sqrt(d2) PSUM->SBUF bf16 (eps pre-added keeps d2 > 0)
  DVE tsA: w = min(dc,3), accum(min) -> per-row min dist (attract).
  repel integrand (3-w)^2, summed per row, via 3 balanced routes:
    'A': Act Square(w - 3) + accum          (Act only)
    'D': vc = w-3 (ts); vc*vc (tt); ts accum (DVE only)
    'P': vc = w-3 (ts); vc*vc on gpsimd tt; ts accum (Pool + DVE)
  Route counts chosen so Act/DVE/Pool all land at ~140us busy.

Note: tensor_scalar with accum_out applies only op0 to the written
output; op1 becomes the accumulation op.  All op1 choices here are
no-ops on the output.

Epilogue (per batch): rowmins [128,8] -> +3 -> bf16 pad to [128,128] ->
XBAR DMA transpose -> flatten -> gpsimd partition_broadcast -> top-k by
rank (count of strictly-smaller values) -> select/dot; final partition
sums via gpsimd partition_all_reduce (no PSUM use outside the matmuls).

Self-contained: hardcodes shapes for binder[16,1024,3], target[8192,3].
"""

import numpy as np
from contextlib import ExitStack

import concourse.bass as bass
import concourse.bacc as bacc
import concourse.tile as tile
from concourse import mybir, bass_isa
from concourse.masks import make_identity
from concourse.bass_utils import run_bass_kernel_spmd

F32 = mybir.dt.float32
F32R = mybir.dt.float32r
BF16 = mybir.dt.bfloat16
AF = mybir.ActivationFunctionType
OP = mybir.AluOpType
AX = mybir.AxisListType
RED = bass_isa.ReduceOp

B, N, MT = 16, 1024, 8192
NCORES = 8
BC = B // NCORES          # batches per core
TOPK = 204                # int(0.2 * N)
CLASH = 3.0
EPS = 1e-3                # guards sqrt against fp-rounding-negative d2
ATTRACT_SCALE, REPEL_SCALE = 10.0, 5.0

P = 128                   # SBUF partitions
NCHUNK = N // P           # 8 row-chunks per batch
MTILE = 2048              # PSUM tile free size (4 banks)
JPC = MT // MTILE         # m-tiles per chunk (4)
MMF = 512                 # fp32 matmul max moving free size
KP = 5                    # packed contraction size

# square-route assignment per (batch, m-tile) — 32 tiles per batch.
# 'P' gpsimd, 'A' scalar-engine Square, 'D' vector engine.
# Batch 1 keeps its last chunks gpsimd-free so Pool drains before the
# final epilogue instead of 25us after the last sqrt.


def _routes():
    b0 = []
    for k in range(32):
        if k % 2 == 0:
            b0.append("P")
        elif k in (9, 25):
            b0.append("A")
        else:
            b0.append("D")
    b1 = []
    for k in range(32):
        if k < 20:
            b1.append("P" if k % 5 != 4 else "D")
        elif k < 24:
            b1.append("D")
        elif k < 30 or k == 31:
            b1.append("A")
        else:
            b1.append("D")
    return [b0, b1]


ROUTES = _routes()

_prog_cache = {}


def build_program():
    nc = bacc.Bacc("TRN2", target_bir_lowering=False, debug=False,
                   num_devices=NCORES)
    bnd = nc.dram_tensor("bnd", [BC, KP, N], F32R, kind="ExternalInput").ap()
    tgt = nc.dram_tensor("tgt", [KP, MT], F32R, kind="ExternalInput").ap()
    out = nc.dram_tensor("out", [BC, 1], F32, kind="ExternalOutput").ap()

    with tile.TileContext(nc) as tc, ExitStack() as ctx:
        consts = ctx.enter_context(tc.tile_pool(name="consts", bufs=1))
        work = ctx.enter_context(tc.tile_pool(name="work", bufs=1))
        dcp = ctx.enter_context(tc.tile_pool(name="dcp", bufs=8))
        wp = ctx.enter_context(tc.tile_pool(name="wp", bufs=6))
        vcd = ctx.enter_context(tc.tile_pool(name="vcd", bufs=3))
        vcp = ctx.enter_context(tc.tile_pool(name="vcp", bufs=7))
        wap = ctx.enter_context(tc.tile_pool(name="wap", bufs=3))
        v2d = ctx.enter_context(tc.tile_pool(name="v2d", bufs=3))
        v2pp = ctx.enter_context(tc.tile_pool(name="v2pp", bufs=9))
        psum = ctx.enter_context(tc.tile_pool(name="psum", bufs=2, space="PSUM"))
        dpool = ctx.enter_context(tc.tile_pool(name="dpool", bufs=1, space="DRAM"))

        # ---- inputs: pure DMA, no engine preamble.  lhsT0 + first rhs
        # chunk gate the first matmul, so issue them first on separate
        # queues. ----
        rhs = consts.tile([KP, MT], F32R)
        lhsTs = [consts.tile([KP, N], F32R, name=f"lhsT{b}") for b in range(BC)]
        nc.sync.dma_start(out=lhsTs[0][:, :], in_=bnd[0, :, :])
        nc.scalar.dma_start(out=rhs[:, 0:512], in_=tgt[:, 0:512])
        nc.sync.dma_start(out=rhs[:, 512:2048], in_=tgt[:, 512:2048])
        nc.scalar.dma_start(out=rhs[:, 2048:4096], in_=tgt[:, 2048:4096])
        nc.sync.dma_start(out=rhs[:, 4096:6144], in_=tgt[:, 4096:6144])
        nc.scalar.dma_start(out=rhs[:, 6144:8192], in_=tgt[:, 6144:8192])
        nc.sync.dma_start(out=lhsTs[1][:, :], in_=bnd[1, :, :])

        # shared elementwise-output scratch (values never read)
        waste1k = work.tile([P, N], BF16)
        neg3 = consts.tile([P, 1], F32)
        nc.vector.memset(neg3, -CLASH)
        ones128 = consts.tile([P, 1], F32)
        nc.vector.memset(ones128, 1.0)
        ident = consts.tile([P, P], F32)
        make_identity(nc, ident)

        # Engine queues are strict FIFO, so an op that waits on a slow
        # producer (gpsimd square, epilogue DMA chain) must not be emitted
        # right after it or it head-of-line-blocks the whole engine.
        # Deferred emission queues provide the lag.
        pend_sum = []     # P-route row-sum ts ops waiting for gpsimd
        pend_act = []     # A-route Square ops, lag one tile
        pend_epi = []     # previous batch's epilogue part 2
        pend_fin = []     # final partition-sum + output, all batches
        pend_rank = []    # b1 split-rank: early ranks vs chunks 0-6

        def flush(q, keep=0):
            while len(q) > keep:
                q.pop(0)()

        for b in range(BC):
            lhsT = lhsTs[b]
            sw2B = work.tile([P, NCHUNK * JPC], F32, name=f"sw2B{b}")
            mnB = work.tile([P, NCHUNK * JPC], F32, name=f"mnB{b}")

            NE = (NCHUNK - 1) * P          # 896: rows in chunks 0-6
            vBm8 = work.tile([P, NCHUNK], F32, name=f"vBm8{b}")
            rankE = work.tile([P, NCHUNK], F32, name=f"rankE{b}")
            vrepE = work.tile([P, NE], BF16, name=f"vrepE{b}")
            for c in range(NCHUNK):
                if b == 1 and c == NCHUNK - 1:
                    # ---- split-rank early piece: chunks 0-6 are final;
                    # flatten+broadcast their rowmins and rank against
                    # them while chunk 7 computes ----
                    nc.vector.tensor_reduce(
                        vBm8[:, 0:NCHUNK - 1],
                        mnB[:, 0:(NCHUNK - 1) * JPC].rearrange(
                            "p (c j) -> p c j", c=NCHUNK - 1),
                        AX.X, OP.min)
                    vBbE = work.tile([P, NCHUNK - 1], BF16, name="vBbE")
                    nc.vector.tensor_copy(vBbE, vBm8[:, 0:NCHUNK - 1])
                    vfdE = dpool.tile([1, NE], BF16, name="vfdE")
                    nc.sync.dma_start(
                        out=vfdE[0:1, :].rearrange("o (p c) -> o p c", p=P),
                        in_=vBbE)
                    vfdE_b = bass.AP(tensor=vfdE.tensor, offset=vfdE.offset,
                                     ap=[[0, P], vfdE.ap[-1]])
                    nc.sync.dma_start(out=vrepE, in_=vfdE_b)
                    for cc in range(NCHUNK - 1):
                        def mk_rank(cc=cc):
                            def go():
                                nc.vector.tensor_scalar(
                                    waste1k[:, 0:NE], vrepE,
                                    vBm8[:, cc:cc + 1], 0.0,
                                    OP.is_lt, OP.add,
                                    accum_out=rankE[:, cc:cc + 1])
                            return go
                        pend_rank.append(mk_rank())
                lc = lhsT[:, c * P:(c + 1) * P]
                for j in range(JPC):
                    if b == 1 and c == NCHUNK - 1 and j >= 2:
                        flush(pend_rank, keep=(4 if j == 2 else 0))
                    k = c * JPC + j
                    route = ROUTES[b][k]
                    ps = psum.tile([P, MTILE], F32, name="ps", tag="ps")
                    for q in range(MTILE // MMF):
                        nc.tensor.matmul(
                            ps[:, q * MMF:(q + 1) * MMF], lc,
                            rhs[:, j * MTILE + q * MMF:
                                j * MTILE + (q + 1) * MMF],
                            start=True, stop=True)
                    dc = dcp.tile([P, MTILE], BF16, name="dc", tag="dc")
                    nc.scalar.activation(dc, ps, AF.Sqrt)
                    # w = min(dc, 3); accum(min) -> rowmin (attract)
                    w = wp.tile([P, MTILE], BF16, name="w", tag="w")
                    nc.vector.tensor_scalar(
                        w, dc, CLASH, 3.4e38, OP.min, OP.min,
                        accum_out=mnB[:, k:k + 1])
                    if route == "A":
                        def mk_act(w=w, sw2B=sw2B, k=k):
                            def go():
                                wa = wap.tile([P, MTILE], BF16, name="wa",
                                              tag="wa")
                                nc.scalar.activation(
                                    wa, w, AF.Square, bias=neg3[:, 0:1],
                                    scale=1.0, accum_out=sw2B[:, k:k + 1])
                            return go
                        pend_act.append(mk_act())
                    else:
                        # vc = w - 3 = -relu(3-d)
                        pool_ = vcp if route == "P" else vcd
                        vc = pool_.tile([P, MTILE], BF16, name="vc", tag="vc")
                        nc.vector.tensor_scalar(
                            vc, w, CLASH, 0.0, OP.subtract, OP.min)
                        if route == "P":
                            v2 = v2pp.tile([P, MTILE], BF16, name="v2",
                                           tag="v2")
                            nc.gpsimd.tensor_tensor(v2, vc, vc, OP.mult)
                            def mk_sum(v2=v2, sw2B=sw2B, k=k):
                                def go():
                                    nc.vector.tensor_scalar(
                                        v2, v2, 3.4e38, 0.0, OP.min, OP.add,
                                        accum_out=sw2B[:, k:k + 1])
                                return go
                            pend_sum.append(mk_sum())
                        else:
                            v2 = v2d.tile([P, MTILE], BF16, name="v2",
                                          tag="v2")
                            nc.vector.tensor_tensor(v2, vc, vc, OP.mult)
                            nc.vector.tensor_scalar(
                                v2, v2, 3.4e38, 0.0, OP.min, OP.add,
                                accum_out=sw2B[:, k:k + 1])
                    flush(pend_act, keep=5)
                    flush(pend_sum, keep=9)
                if c == 1:
                    flush(pend_epi)
            flush(pend_act)
            flush(pend_sum)

            # ---- per-batch epilogue, part 1 ----
            stack2 = work.tile([P, 2], F32, name=f"stack2{b}")
            nc.vector.tensor_reduce(stack2[:, 1:2], sw2B, AX.X, OP.add)
            vrep = None
            if b == 0:
                vBm = work.tile([P, NCHUNK], F32, name=f"vBm{b}")
                nc.vector.tensor_reduce(
                    vBm, mnB.rearrange("p (c j) -> p c j", c=NCHUNK),
                    AX.X, OP.min)
                # rank selection ignores element order, so flatten [128, 8]
                # partition-major straight to DRAM and broadcast-read it
                # back with a zero partition stride.
                vBb = work.tile([P, NCHUNK], BF16, name=f"vBb{b}")
                nc.vector.tensor_copy(vBb, vBm)
                vfd = dpool.tile([1, N], BF16, name=f"vfd{b}")
                nc.sync.dma_start(
                    out=vfd[0:1, :].rearrange("o (p c) -> o p c", p=P),
                    in_=vBb)
                vrep = work.tile([P, N], BF16, name=f"vrep{b}")
                vfd_bcast = bass.AP(tensor=vfd.tensor, offset=vfd.offset,
                                    ap=[[0, P], vfd.ap[-1]])
                nc.sync.dma_start(out=vrep, in_=vfd_bcast)
            else:
                vBm = vBm8

            # part 2: rank selection + final combine; deferred into the
            # next batch's main loop so the DMA latency above is hidden
            def mk_epi(b=b, vBm=vBm, vrep=vrep, stack2=stack2,
                       vBm8=vBm8, rankE=rankE, vrepE=vrepE):
                def go():
                    rank8 = work.tile([P, NCHUNK], F32, name=f"rank8{b}")
                    if b == 0:
                        for c in range(NCHUNK):
                            nc.vector.tensor_scalar(
                                waste1k, vrep, vBm[:, c:c + 1], 0.0,
                                OP.is_lt, OP.add, accum_out=rank8[:, c:c + 1])
                    else:
                        NE = (NCHUNK - 1) * P
                        # chunk 7 rowmins land in vBm8[:, 7]
                        nc.vector.tensor_reduce(
                            vBm8[:, NCHUNK - 1:NCHUNK],
                            mnB[:, (NCHUNK - 1) * JPC:], AX.X, OP.min)
                        # its rank vs chunks 0-6
                        nc.vector.tensor_scalar(
                            waste1k[:, 0:NE], vrepE,
                            vBm8[:, NCHUNK - 1:NCHUNK], 0.0,
                            OP.is_lt, OP.add,
                            accum_out=rankE[:, NCHUNK - 1:NCHUNK])
                        # chunk-7 j-side: PE transpose [128,1]->[1,128]
                        # (PSUM is free now) + gpsimd broadcast
                        vtp = psum.tile([1, P], F32, name="vtp", tag="ps")
                        nc.tensor.transpose(vtp, vBm8[:, NCHUNK - 1:NCHUNK],
                                            ident)
                        vfl2 = work.tile([1, P], F32, name="vfl2")
                        nc.vector.tensor_copy(vfl2, vtp)
                        vrep2 = work.tile([P, P], F32, name="vrep2")
                        nc.gpsimd.partition_broadcast(vrep2, vfl2[0:1, :], P)
                        rankL = work.tile([P, NCHUNK], F32, name="rankL")
                        for c in range(NCHUNK):
                            nc.vector.tensor_scalar(
                                waste1k[:, 0:P], vrep2, vBm8[:, c:c + 1],
                                0.0, OP.is_lt, OP.add,
                                accum_out=rankL[:, c:c + 1])
                        nc.vector.tensor_add(rank8, rankE, rankL)
                    sel8 = work.tile([P, NCHUNK], F32, name=f"sel8{b}")
                    nc.vector.tensor_scalar(sel8, rank8, float(TOPK), None,
                                            OP.is_lt)
                    prod8 = work.tile([P, NCHUNK], F32, name=f"prod8{b}")
                    nc.vector.tensor_mul(prod8, sel8, vBm)
                    # sum of selected (rowmin-3); add 3 per selected row
                    # via the count
                    nc.vector.tensor_reduce(stack2[:, 0:1], prod8, AX.X,
                                            OP.add)
                    cnt8 = work.tile([P, 1], F32, name=f"cnt8{b}")
                    nc.vector.tensor_reduce(cnt8, sel8, AX.X, OP.add)
                    st3 = work.tile([P, 3], F32, name=f"st3{b}")
                    nc.vector.tensor_copy(st3[:, 0:2], stack2)
                    nc.vector.tensor_copy(st3[:, 2:3], cnt8)

                    # PE ones-matmul + final combine: deferred to after the
                    # main loops, when PE/PSUM are idle (a mid-stream fin
                    # matmul head-of-line-blocks the PE FIFO)
                    def fin_go(b=b, st3=st3):
                        finp = psum.tile([1, 3], F32, name="finp", tag="ps")
                        nc.tensor.matmul(finp, ones128, st3, start=True,
                                         stop=True)
                        fin = work.tile([1, 2], F32, name=f"fin{b}")
                        nc.vector.tensor_copy(fin, finp[0:1, 0:2])
                        fcnt = work.tile([1, 1], F32, name=f"fcnt{b}")
                        nc.vector.tensor_copy(fcnt, finp[0:1, 2:3])
                        en = work.tile([1, 2], F32, name=f"en{b}")
                        nc.vector.tensor_scalar(
                            en[0:1, 0:1], fcnt[0:1, 0:1],
                            CLASH * ATTRACT_SCALE / TOPK, None, OP.mult)
                        t0 = work.tile([1, 1], F32, name=f"t0{b}")
                        nc.vector.tensor_scalar(t0, fin[0:1, 0:1],
                                                ATTRACT_SCALE / TOPK, None,
                                                OP.mult)
                        nc.vector.tensor_scalar(en[0:1, 1:2], fin[0:1, 1:2],
                                                REPEL_SCALE, None, OP.mult)
                        en2 = work.tile([1, 1], F32, name=f"en2{b}")
                        nc.vector.tensor_add(en2, en[0:1, 0:1], en[0:1, 1:2])
                        en3 = work.tile([1, 1], F32, name=f"en3{b}")
                        nc.vector.tensor_add(en3, en2, t0)
                        nc.sync.dma_start(out=out[b:b + 1, 0:1],
                                          in_=en3[0:1, 0:1])
                    pend_fin.append(fin_go)
                return go
            pend_epi.append(mk_epi())
        flush(pend_epi)
        flush(pend_fin)

    nc.compile()
    return nc


def _get_program():
    if "nc" not in _prog_cache:
        _prog_cache["nc"] = build_program()
    return _prog_cache["nc"]


def make_in_maps(binder_trans, target_coords):
    x = np.asarray(binder_trans, dtype=np.float32)          # [B, N, 3]
    y = np.asarray(target_coords, dtype=np.float32)         # [M, 3]
    # lhsT per batch: [x0 x1 x2 ; ||x||^2 ; 1]  -> [B, 5, N]
    xT = x.transpose(0, 2, 1)                               # [B, 3, N]
    xsq = (x * x).sum(-1)[:, None, :]                       # [B, 1, N]
    ones_n = np.ones((B, 1, N), dtype=np.float32)
    lhs = np.ascontiguousarray(
        np.concatenate([xT, xsq, ones_n], axis=1))          # [B, 5, N]
    # rhs: [-2y0 -2y1 -2y2 ; 1 ; sum(y^2)+eps] -> [5, M]
    yT = -2.0 * y.T                                         # [3, M]
    ones_m = np.ones((1, MT), dtype=np.float32)
    ysq = (y * y).sum(-1)[None, :] + np.float32(EPS)        # [1, M]
    rhs = np.ascontiguousarray(
        np.concatenate([yT, ones_m, ysq], axis=0))          # [5, M]
    return [{"bnd": np.ascontiguousarray(lhs[c * BC:(c + 1) * BC]),
             "tgt": rhs}
            for c in range(NCORES)]


def kernel(binder_trans, target_coords):
    nc = _get_program()
    in_maps = make_in_maps(binder_trans, target_coords)
    res = run_bass_kernel_spmd(nc, in_maps, list(range(NCORES)))
    outs = [np.asarray(res.results[c]["out"], dtype=np.float32).reshape(BC)
            for c in range(NCORES)]
    return np.concatenate(outs).astype(np.float32)


# revision 10
# speedup vs baseline: 1.8942x; 1.0165x over previous
"""Trainium2 Bass kernel for BinderEnergyGuidance (retrieval_knn).

Math (per batch b of 16):
  d[b,n,m]   = ||binder[b,n] - target[m]||           (N=1024, M=8192)
  attract[b] = mean of the k=204 smallest per-row min-distances
  repel[b]   = sum relu(3 - d)^2
  out[b]     = 10*attract[b] + 5*repel[b]

Strategy: data-parallel over batch, 2 batches per core.  Host packs the
inputs so the device contraction is K=5:
  lhsT rows: [x0 x1 x2 ; ||x||^2 ; 1]        (per batch, [5, N])
  rhs  rows: [-2y0 -2y1 -2y2 ; 1 ; sum(y^2)+eps]   ([5, M])
so one fp32r matmul emits d2 + eps directly into PSUM.

Per 2048-wide m-tile (64 per core):
  PE : 4x 512-col matmuls -> PSUM fp32 d2 (2 PSUM tiles = all 8 banks)
  Act: dc = Sqrt(d2) PSUM->SBUF bf16 (eps pre-added keeps d2 > 0)
  DVE tsA: w = min(dc,3), accum(min) -> per-row min dist (attract).
  repel integrand (3-w)^2, summed per row, via 3 balanced routes:
    'A': Act Square(w - 3) + accum          (Act only)
    'D': vc = w-3 (ts); vc*vc (tt); ts accum (DVE only)
    'P': vc = w-3 (ts); vc*vc on gpsimd tt; ts accum (Pool + DVE)
  Route counts chosen so Act/DVE/Pool all land at ~140us busy.

Note: tensor_scalar with accum_out applies only op0 to the written
output; op1 becomes the accumulation op.  All op1 choices here are
no-ops on the output.

Epilogue (per batch): rowmins [128,8] -> +3 -> bf16 pad to [128,128] ->
XBAR DMA transpose -> flatten -> gpsimd partition_broadcast -> top-k by
rank (count of strictly-smaller values) -> select/dot; final partition
sums via gpsimd partition_all_reduce (no PSUM use outside the matmuls).

Self-contained: hardcodes shapes for binder[16,1024,3], target[8192,3].
"""

import numpy as np
from contextlib import ExitStack

import concourse.bass as bass
import concourse.bacc as bacc
import concourse.tile as tile
from concourse import mybir, bass_isa
from concourse.masks import make_identity
from concourse.bass_utils import run_bass_kernel_spmd

F32 = mybir.dt.float32
F32R = mybir.dt.float32r
BF16 = mybir.dt.bfloat16
AF = mybir.ActivationFunctionType
OP = mybir.AluOpType
AX = mybir.AxisListType
RED = bass_isa.ReduceOp

B, N, MT = 16, 1024, 8192
NCORES = 8
BC = B // NCORES          # batches per core
TOPK = 204                # int(0.2 * N)
CLASH = 3.0
EPS = 1e-3                # guards sqrt against fp-rounding-negative d2
ATTRACT_SCALE, REPEL_SCALE = 10.0, 5.0

P = 128                   # SBUF partitions
NCHUNK = N // P           # 8 row-chunks per batch
MTILE = 2048              # PSUM tile free size (4 banks)
JPC = MT // MTILE         # m-tiles per chunk (4)
MMF = 512                 # fp32 matmul max moving free size
KP = 5                    # packed contraction size

# square-route assignment per (batch, m-tile) — 32 tiles per batch.
# 'P' gpsimd, 'A' scalar-engine Square, 'D' vector engine.
# Batch 1 keeps its last chunks gpsimd-free so Pool drains before the
# final epilogue instead of 25us after the last sqrt.


def _routes():
    b0 = []
    for k in range(32):
        if k % 2 == 0 and k != 0:
            b0.append("P")
        elif k in (0, 9, 25):
            b0.append("A")
        else:
            b0.append("D")
    b1 = []
    for k in range(32):
        if k < 20:
            b1.append("P" if k % 5 != 4 else "D")
        elif k < 24:
            b1.append("D")
        elif k < 30 or k == 31:
            b1.append("A")
        else:
            b1.append("D")
    return [b0, b1]


ROUTES = _routes()

_prog_cache = {}


def build_program():
    nc = bacc.Bacc("TRN2", target_bir_lowering=False, debug=False,
                   num_devices=NCORES)
    bnd = nc.dram_tensor("bnd", [BC, KP, N], F32R, kind="ExternalInput").ap()
    tgt = nc.dram_tensor("tgt", [KP, MT], F32R, kind="ExternalInput").ap()
    out = nc.dram_tensor("out", [BC, 1], F32, kind="ExternalOutput").ap()

    with tile.TileContext(nc) as tc, ExitStack() as ctx:
        consts = ctx.enter_context(tc.tile_pool(name="consts", bufs=1))
        work = ctx.enter_context(tc.tile_pool(name="work", bufs=1))
        dcp = ctx.enter_context(tc.tile_pool(name="dcp", bufs=8))
        wp = ctx.enter_context(tc.tile_pool(name="wp", bufs=6))
        vcd = ctx.enter_context(tc.tile_pool(name="vcd", bufs=3))
        vcp = ctx.enter_context(tc.tile_pool(name="vcp", bufs=7))
        wap = ctx.enter_context(tc.tile_pool(name="wap", bufs=3))
        v2d = ctx.enter_context(tc.tile_pool(name="v2d", bufs=3))
        v2pp = ctx.enter_context(tc.tile_pool(name="v2pp", bufs=9))
        psum = ctx.enter_context(tc.tile_pool(name="psum", bufs=2, space="PSUM"))
        dpool = ctx.enter_context(tc.tile_pool(name="dpool", bufs=1, space="DRAM"))

        # ---- inputs: pure DMA, no engine preamble.  lhsT0 + first rhs
        # chunk gate the first matmul, so issue them first on separate
        # queues. ----
        rhs = consts.tile([KP, MT], F32R)
        lhsTs = [consts.tile([KP, N], F32R, name=f"lhsT{b}") for b in range(BC)]
        nc.sync.dma_start(out=lhsTs[0][:, :], in_=bnd[0, :, :])
        nc.scalar.dma_start(out=rhs[:, 0:512], in_=tgt[:, 0:512])
        nc.sync.dma_start(out=rhs[:, 512:2048], in_=tgt[:, 512:2048])
        nc.scalar.dma_start(out=rhs[:, 2048:4096], in_=tgt[:, 2048:4096])
        nc.sync.dma_start(out=rhs[:, 4096:6144], in_=tgt[:, 4096:6144])
        nc.scalar.dma_start(out=rhs[:, 6144:8192], in_=tgt[:, 6144:8192])
        nc.sync.dma_start(out=lhsTs[1][:, :], in_=bnd[1, :, :])

        # shared elementwise-output scratch (values never read)
        waste1k = work.tile([P, N], BF16)
        neg3 = consts.tile([P, 1], F32)
        nc.vector.memset(neg3, -CLASH)
        ones128 = consts.tile([P, 1], F32)
        nc.vector.memset(ones128, 1.0)
        ident = consts.tile([P, P], F32)
        make_identity(nc, ident)

        # Engine queues are strict FIFO, so an op that waits on a slow
        # producer (gpsimd square, epilogue DMA chain) must not be emitted
        # right after it or it head-of-line-blocks the whole engine.
        # Deferred emission queues provide the lag.
        pend_sum = []     # P-route row-sum ts ops waiting for gpsimd
        pend_act = []     # A-route Square ops, lag one tile
        pend_epi = []     # previous batch's epilogue part 2
        pend_fin = []     # final partition-sum + output, all batches
        pend_rank = []    # b1 split-rank: early ranks vs chunks 0-6

        def flush(q, keep=0):
            while len(q) > keep:
                q.pop(0)()

        for b in range(BC):
            lhsT = lhsTs[b]
            sw2B = work.tile([P, NCHUNK * JPC], F32, name=f"sw2B{b}")
            mnB = work.tile([P, NCHUNK * JPC], F32, name=f"mnB{b}")

            NE = (NCHUNK - 1) * P          # 896: rows in chunks 0-6
            vBm8 = work.tile([P, NCHUNK], F32, name=f"vBm8{b}")
            rankE = work.tile([P, NCHUNK], F32, name=f"rankE{b}")
            vrepE = work.tile([P, NE], BF16, name=f"vrepE{b}")
            for c in range(NCHUNK):
                if b == 1 and c == NCHUNK - 1:
                    # ---- split-rank early piece: chunks 0-6 are final;
                    # flatten+broadcast their rowmins and rank against
                    # them while chunk 7 computes ----
                    nc.vector.tensor_reduce(
                        vBm8[:, 0:NCHUNK - 1],
                        mnB[:, 0:(NCHUNK - 1) * JPC].rearrange(
                            "p (c j) -> p c j", c=NCHUNK - 1),
                        AX.X, OP.min)
                    vBbE = work.tile([P, NCHUNK - 1], BF16, name="vBbE")
                    nc.vector.tensor_copy(vBbE, vBm8[:, 0:NCHUNK - 1])
                    vfdE = dpool.tile([1, NE], BF16, name="vfdE")
                    nc.sync.dma_start(
                        out=vfdE[0:1, :].rearrange("o (p c) -> o p c", p=P),
                        in_=vBbE)
                    vfdE_b = bass.AP(tensor=vfdE.tensor, offset=vfdE.offset,
                                     ap=[[0, P], vfdE.ap[-1]])
                    nc.sync.dma_start(out=vrepE, in_=vfdE_b)
                    for cc in range(NCHUNK - 1):
                        def mk_rank(cc=cc):
                            def go():
                                nc.vector.tensor_scalar(
                                    waste1k[:, 0:NE], vrepE,
                                    vBm8[:, cc:cc + 1], 0.0,
                                    OP.is_lt, OP.add,
                                    accum_out=rankE[:, cc:cc + 1])
                            return go
                        pend_rank.append(mk_rank())
                lc = lhsT[:, c * P:(c + 1) * P]
                for j in range(JPC):
                    if b == 1 and c == NCHUNK - 1 and j >= 2:
                        flush(pend_rank, keep=(4 if j == 2 else 0))
                    k = c * JPC + j
                    route = ROUTES[b][k]
                    ps = psum.tile([P, MTILE], F32, name="ps", tag="ps")
                    for q in range(MTILE // MMF):
                        nc.tensor.matmul(
                            ps[:, q * MMF:(q + 1) * MMF], lc,
                            rhs[:, j * MTILE + q * MMF:
                                j * MTILE + (q + 1) * MMF],
                            start=True, stop=True)
                    dc = dcp.tile([P, MTILE], BF16, name="dc", tag="dc")
                    nc.scalar.activation(dc, ps, AF.Sqrt)
                    # w = min(dc, 3); accum(min) -> rowmin (attract)
                    w = wp.tile([P, MTILE], BF16, name="w", tag="w")
                    nc.vector.tensor_scalar(
                        w, dc, CLASH, 3.4e38, OP.min, OP.min,
                        accum_out=mnB[:, k:k + 1])
                    if route == "A":
                        def mk_act(w=w, sw2B=sw2B, k=k):
                            def go():
                                wa = wap.tile([P, MTILE], BF16, name="wa",
                                              tag="wa")
                                nc.scalar.activation(
                                    wa, w, AF.Square, bias=neg3[:, 0:1],
                                    scale=1.0, accum_out=sw2B[:, k:k + 1])
                            return go
                        pend_act.append(mk_act())
                    else:
                        # vc = w - 3 = -relu(3-d)
                        pool_ = vcp if route == "P" else vcd
                        vc = pool_.tile([P, MTILE], BF16, name="vc", tag="vc")
                        nc.vector.tensor_scalar(
                            vc, w, CLASH, 0.0, OP.subtract, OP.min)
                        if route == "P":
                            v2 = v2pp.tile([P, MTILE], BF16, name="v2",
                                           tag="v2")
                            nc.gpsimd.tensor_tensor(v2, vc, vc, OP.mult)
                            def mk_sum(v2=v2, sw2B=sw2B, k=k):
                                def go():
                                    nc.vector.tensor_scalar(
                                        v2, v2, 3.4e38, 0.0, OP.min, OP.add,
                                        accum_out=sw2B[:, k:k + 1])
                                return go
                            pend_sum.append(mk_sum())
                        else:
                            v2 = v2d.tile([P, MTILE], BF16, name="v2",
                                          tag="v2")
                            nc.vector.tensor_tensor(v2, vc, vc, OP.mult)
                            nc.vector.tensor_scalar(
                                v2, v2, 3.4e38, 0.0, OP.min, OP.add,
                                accum_out=sw2B[:, k:k + 1])
                    flush(pend_act, keep=5)
                    flush(pend_sum, keep=9)
                if c == 1:
                    flush(pend_epi)
            flush(pend_act)
            flush(pend_sum)

            # ---- per-batch epilogue, part 1 ----
            stack2 = work.tile([P, 2], F32, name=f"stack2{b}")
            nc.vector.tensor_reduce(stack2[:, 1:2], sw2B, AX.X, OP.add)
            vrep = None
            if b == 0:
                vBm = work.tile([P, NCHUNK], F32, name=f"vBm{b}")
                nc.vector.tensor_reduce(
                    vBm, mnB.rearrange("p (c j) -> p c j", c=NCHUNK),
                    AX.X, OP.min)
                # rank selection ignores element order, so flatten [128, 8]
                # partition-major straight to DRAM and broadcast-read it
                # back with a zero partition stride.
                vBb = work.tile([P, NCHUNK], BF16, name=f"vBb{b}")
                nc.vector.tensor_copy(vBb, vBm)
                vfd = dpool.tile([1, N], BF16, name=f"vfd{b}")
                nc.sync.dma_start(
                    out=vfd[0:1, :].rearrange("o (p c) -> o p c", p=P),
                    in_=vBb)
                vrep = work.tile([P, N], BF16, name=f"vrep{b}")
                vfd_bcast = bass.AP(tensor=vfd.tensor, offset=vfd.offset,
                                    ap=[[0, P], vfd.ap[-1]])
                nc.sync.dma_start(out=vrep, in_=vfd_bcast)
            else:
                vBm = vBm8

            # part 2: rank selection + final combine; deferred into the
            # next batch's main loop so the DMA latency above is hidden
            def mk_epi(b=b, vBm=vBm, vrep=vrep, stack2=stack2,
                       vBm8=vBm8, rankE=rankE, vrepE=vrepE):
                def go():
                    rank8 = work.tile([P, NCHUNK], F32, name=f"rank8{b}")
                    if b == 0:
                        for c in range(NCHUNK):
                            nc.vector.tensor_scalar(
                                waste1k, vrep, vBm[:, c:c + 1], 0.0,
                                OP.is_lt, OP.add, accum_out=rank8[:, c:c + 1])
                    else:
                        NE = (NCHUNK - 1) * P
                        # chunk 7 rowmins land in vBm8[:, 7]
                        nc.vector.tensor_reduce(
                            vBm8[:, NCHUNK - 1:NCHUNK],
                            mnB[:, (NCHUNK - 1) * JPC:], AX.X, OP.min)
                        # its rank vs chunks 0-6
                        nc.vector.tensor_scalar(
                            waste1k[:, 0:NE], vrepE,
                            vBm8[:, NCHUNK - 1:NCHUNK], 0.0,
                            OP.is_lt, OP.add,
                            accum_out=rankE[:, NCHUNK - 1:NCHUNK])
                        # chunk-7 j-side: PE transpose [128,1]->[1,128]
                        # (PSUM is free now) + gpsimd broadcast
                        vtp = psum.tile([1, P], F32, name="vtp", tag="ps")
                        nc.tensor.transpose(vtp, vBm8[:, NCHUNK - 1:NCHUNK],
                                            ident)
                        vfl2 = work.tile([1, P], F32, name="vfl2")
                        nc.vector.tensor_copy(vfl2, vtp)
                        vrep2 = work.tile([P, P], F32, name="vrep2")
                        nc.gpsimd.partition_broadcast(vrep2, vfl2[0:1, :], P)
                        rankL = work.tile([P, NCHUNK], F32, name="rankL")
                        for c in range(NCHUNK):
                            nc.vector.tensor_scalar(
                                waste1k[:, 0:P], vrep2, vBm8[:, c:c + 1],
                                0.0, OP.is_lt, OP.add,
                                accum_out=rankL[:, c:c + 1])
                        nc.vector.tensor_add(rank8, rankE, rankL)
                    sel8 = work.tile([P, NCHUNK], F32, name=f"sel8{b}")
                    nc.vector.tensor_scalar(sel8, rank8, float(TOPK), None,
                                            OP.is_lt)
                    prod8 = work.tile([P, NCHUNK], F32, name=f"prod8{b}")
                    nc.vector.tensor_mul(prod8, sel8, vBm)
                    # sum of selected (rowmin-3); add 3 per selected row
                    # via the count
                    nc.vector.tensor_reduce(stack2[:, 0:1], prod8, AX.X,
                                            OP.add)
                    cnt8 = work.tile([P, 1], F32, name=f"cnt8{b}")
                    nc.vector.tensor_reduce(cnt8, sel8, AX.X, OP.add)
                    st3 = work.tile([P, 3], F32, name=f"st3{b}")
                    nc.vector.tensor_copy(st3[:, 0:2], stack2)
                    nc.vector.tensor_copy(st3[:, 2:3], cnt8)

                    # PE ones-matmul + final combine: deferred to after the
                    # main loops, when PE/PSUM are idle (a mid-stream fin
                    # matmul head-of-line-blocks the PE FIFO)
                    def fin_go(b=b, st3=st3):
                        finp = psum.tile([1, 3], F32, name="finp", tag="ps")
                        nc.tensor.matmul(finp, ones128, st3, start=True,
                                         stop=True)
                        fin = work.tile([1, 2], F32, name=f"fin{b}")
                        nc.vector.tensor_copy(fin, finp[0:1, 0:2])
                        fcnt = work.tile([1, 1], F32, name=f"fcnt{b}")
                        nc.vector.tensor_copy(fcnt, finp[0:1, 2:3])
                        en = work.tile([1, 2], F32, name=f"en{b}")
                        nc.vector.tensor_scalar(
                            en[0:1, 0:1], fcnt[0:1, 0:1],
                            CLASH * ATTRACT_SCALE / TOPK, None, OP.mult)
                        t0 = work.tile([1, 1], F32, name=f"t0{b}")
                        nc.vector.tensor_scalar(t0, fin[0:1, 0:1],
                                                ATTRACT_SCALE / TOPK, None,
                                                OP.mult)
                        nc.vector.tensor_scalar(en[0:1, 1:2], fin[0:1, 1:2],
                                                REPEL_SCALE, None, OP.mult)
                        en2 = work.tile([1, 1], F32, name=f"en2{b}")
                        nc.vector.tensor_add(en2, en[0:1, 0:1], en[0:1, 1:2])
                        en3 = work.tile([1, 1], F32, name=f"en3{b}")
                        nc.vector.tensor_add(en3, en2, t0)
                        nc.sync.dma_start(out=out[b:b + 1, 0:1],
                                          in_=en3[0:1, 0:1])
                    pend_fin.append(fin_go)
                return go
            pend_epi.append(mk_epi())
        flush(pend_epi)
        flush(pend_fin)

    nc.compile()
    return nc


def _get_program():
    if "nc" not in _prog_cache:
        _prog_cache["nc"] = build_program()
    return _prog_cache["nc"]


def make_in_maps(binder_trans, target_coords):
    x = np.asarray(binder_trans, dtype=np.float32)          # [B, N, 3]
    y = np.asarray(target_coords, dtype=np.float32)         # [M, 3]
    # lhsT per batch: [x0 x1 x2 ; ||x||^2 ; 1]  -> [B, 5, N]
    xT = x.transpose(0, 2, 1)                               # [B, 3, N]
    xsq = (x * x).sum(-1)[:, None, :]                       # [B, 1, N]
    ones_n = np.ones((B, 1, N), dtype=np.float32)
    lhs = np.ascontiguousarray(
        np.concatenate([xT, xsq, ones_n], axis=1))          # [B, 5, N]
    # rhs: [-2y0 -2y1 -2y2 ; 1 ; sum(y^2)+eps] -> [5, M]
    yT = -2.0 * y.T                                         # [3, M]
    ones_m = np.ones((1, MT), dtype=np.float32)
    ysq = (y * y).sum(-1)[None, :] + np.float32(EPS)        # [1, M]
    rhs = np.ascontiguousarray(
        np.concatenate([yT, ones_m, ysq], axis=0))          # [5, M]
    return [{"bnd": np.ascontiguousarray(lhs[c * BC:(c + 1) * BC]),
             "tgt": rhs}
            for c in range(NCORES)]


def kernel(binder_trans, target_coords):
    nc = _get_program()
    in_maps = make_in_maps(binder_trans, target_coords)
    res = run_bass_kernel_spmd(nc, in_maps, list(range(NCORES)))
    outs = [np.asarray(res.results[c]["out"], dtype=np.float32).reshape(BC)
            for c in range(NCORES)]
    return np.concatenate(outs).astype(np.float32)


# revision 11
# speedup vs baseline: 1.9161x; 1.0115x over previous
"""Trainium2 Bass kernel for BinderEnergyGuidance (retrieval_knn).

Math (per batch b of 16):
  d[b,n,m]   = ||binder[b,n] - target[m]||           (N=1024, M=8192)
  attract[b] = mean of the k=204 smallest per-row min-distances
  repel[b]   = sum relu(3 - d)^2
  out[b]     = 10*attract[b] + 5*repel[b]

Strategy: data-parallel over batch, 2 batches per core.  Host packs the
inputs so the device contraction is K=5:
  lhsT rows: [x0 x1 x2 ; ||x||^2 ; 1]        (per batch, [5, N])
  rhs  rows: [-2y0 -2y1 -2y2 ; 1 ; sum(y^2)+eps]   ([5, M])
so one fp32r matmul emits d2 + eps directly into PSUM.

Per 2048-wide m-tile (64 per core):
  PE : 4x 512-col matmuls -> PSUM fp32 d2 (2 PSUM tiles = all 8 banks)
  Act: dc = Sqrt(d2) PSUM->SBUF bf16 (eps pre-added keeps d2 > 0)
  DVE tsA: w = min(dc,3), accum(min) -> per-row min dist (attract).
  repel integrand (3-w)^2, summed per row, via 3 balanced routes:
    'A': Act Square(w - 3) + accum          (Act only)
    'D': vc = w-3 (ts); vc*vc (tt); ts accum (DVE only)
    'P': vc = w-3 (ts); vc*vc on gpsimd tt; ts accum (Pool + DVE)
  Route counts chosen so Act/DVE/Pool all land at ~140us busy.

Note: tensor_scalar with accum_out applies only op0 to the written
output; op1 becomes the accumulation op.  All op1 choices here are
no-ops on the output.

Epilogue (per batch): rowmins [128,8] -> +3 -> bf16 pad to [128,128] ->
XBAR DMA transpose -> flatten -> gpsimd partition_broadcast -> top-k by
rank (count of strictly-smaller values) -> select/dot; final partition
sums via gpsimd partition_all_reduce (no PSUM use outside the matmuls).

Self-contained: hardcodes shapes for binder[16,1024,3], target[8192,3].
"""

import numpy as np
from contextlib import ExitStack

import concourse.bass as bass
import concourse.bacc as bacc
import concourse.tile as tile
from concourse import mybir, bass_isa
from concourse.masks import make_identity
from concourse.bass_utils import run_bass_kernel_spmd

F32 = mybir.dt.float32
F32R = mybir.dt.float32r
BF16 = mybir.dt.bfloat16
AF = mybir.ActivationFunctionType
OP = mybir.AluOpType
AX = mybir.AxisListType
RED = bass_isa.ReduceOp

B, N, MT = 16, 1024, 8192
NCORES = 8
BC = B // NCORES          # batches per core
TOPK = 204                # int(0.2 * N)
CLASH = 3.0
EPS = 1e-3                # guards sqrt against fp-rounding-negative d2
ATTRACT_SCALE, REPEL_SCALE = 10.0, 5.0

P = 128                   # SBUF partitions
NCHUNK = N // P           # 8 row-chunks per batch
MTILE = 2048              # PSUM tile free size (4 banks)
JPC = MT // MTILE         # m-tiles per chunk (4)
MMF = 512                 # fp32 matmul max moving free size
KP = 5                    # packed contraction size

# square-route assignment per (batch, m-tile) — 32 tiles per batch.
# 'P' gpsimd, 'A' scalar-engine Square, 'D' vector engine.
# Batch 1 keeps its last chunks gpsimd-free so Pool drains before the
# final epilogue instead of 25us after the last sqrt.


def _routes():
    b0 = []
    for k in range(32):
        if k % 2 == 0 and k != 0:
            b0.append("P")
        elif k in (0, 9, 25):
            b0.append("A")
        else:
            b0.append("D")
    b1 = []
    for k in range(32):
        if k < 20:
            b1.append("P" if (k % 5 != 4 and k != 18) else "D")
        elif k < 24:
            b1.append("D")
        elif k < 30 or k == 31:
            b1.append("A")
        else:
            b1.append("D")
    return [b0, b1]


ROUTES = _routes()

_prog_cache = {}


def build_program():
    nc = bacc.Bacc("TRN2", target_bir_lowering=False, debug=False,
                   num_devices=NCORES)
    bnd = nc.dram_tensor("bnd", [BC, KP, N], F32R, kind="ExternalInput").ap()
    tgt = nc.dram_tensor("tgt", [KP, MT], F32R, kind="ExternalInput").ap()
    out = nc.dram_tensor("out", [BC, 1], F32, kind="ExternalOutput").ap()

    with tile.TileContext(nc) as tc, ExitStack() as ctx:
        consts = ctx.enter_context(tc.tile_pool(name="consts", bufs=1))
        work = ctx.enter_context(tc.tile_pool(name="work", bufs=1))
        dcp = ctx.enter_context(tc.tile_pool(name="dcp", bufs=8))
        wp = ctx.enter_context(tc.tile_pool(name="wp", bufs=6))
        vcd = ctx.enter_context(tc.tile_pool(name="vcd", bufs=3))
        vcp = ctx.enter_context(tc.tile_pool(name="vcp", bufs=7))
        wap = ctx.enter_context(tc.tile_pool(name="wap", bufs=3))
        v2d = ctx.enter_context(tc.tile_pool(name="v2d", bufs=3))
        v2pp = ctx.enter_context(tc.tile_pool(name="v2pp", bufs=9))
        psum = ctx.enter_context(tc.tile_pool(name="psum", bufs=2, space="PSUM"))
        dpool = ctx.enter_context(tc.tile_pool(name="dpool", bufs=1, space="DRAM"))

        # ---- inputs: pure DMA, no engine preamble.  lhsT0 + first rhs
        # chunk gate the first matmul, so issue them first on separate
        # queues. ----
        rhs = consts.tile([KP, MT], F32R)
        lhsTs = [consts.tile([KP, N], F32R, name=f"lhsT{b}") for b in range(BC)]
        nc.sync.dma_start(out=lhsTs[0][:, :], in_=bnd[0, :, :])
        nc.scalar.dma_start(out=rhs[:, 0:512], in_=tgt[:, 0:512])
        nc.sync.dma_start(out=rhs[:, 512:2048], in_=tgt[:, 512:2048])
        nc.scalar.dma_start(out=rhs[:, 2048:4096], in_=tgt[:, 2048:4096])
        nc.sync.dma_start(out=rhs[:, 4096:6144], in_=tgt[:, 4096:6144])
        nc.scalar.dma_start(out=rhs[:, 6144:8192], in_=tgt[:, 6144:8192])
        nc.sync.dma_start(out=lhsTs[1][:, :], in_=bnd[1, :, :])

        # shared elementwise-output scratch (values never read)
        waste1k = work.tile([P, N], BF16)
        neg3 = consts.tile([P, 1], F32)
        nc.vector.memset(neg3, -CLASH)
        ones128 = consts.tile([P, 1], F32)
        nc.vector.memset(ones128, 1.0)
        ident = consts.tile([P, P], F32)
        make_identity(nc, ident)

        # Engine queues are strict FIFO, so an op that waits on a slow
        # producer (gpsimd square, epilogue DMA chain) must not be emitted
        # right after it or it head-of-line-blocks the whole engine.
        # Deferred emission queues provide the lag.
        pend_sum = []     # P-route row-sum ts ops waiting for gpsimd
        pend_act = []     # A-route Square ops, lag one tile
        pend_epi = []     # previous batch's epilogue part 2
        pend_fin = []     # final partition-sum + output, all batches
        pend_rank = []    # b1 split-rank: early ranks vs chunks 0-6

        def flush(q, keep=0):
            while len(q) > keep:
                q.pop(0)()

        for b in range(BC):
            lhsT = lhsTs[b]
            sw2B = work.tile([P, NCHUNK * JPC], F32, name=f"sw2B{b}")
            mnB = work.tile([P, NCHUNK * JPC], F32, name=f"mnB{b}")

            NE = (NCHUNK - 1) * P          # 896: rows in chunks 0-6
            vBm8 = work.tile([P, NCHUNK], F32, name=f"vBm8{b}")
            rankE = work.tile([P, NCHUNK], F32, name=f"rankE{b}")
            vrepE = work.tile([P, NE], BF16, name=f"vrepE{b}")
            for c in range(NCHUNK):
                if b == 1 and c == NCHUNK - 1:
                    # ---- split-rank early piece: chunks 0-6 are final;
                    # flatten+broadcast their rowmins and rank against
                    # them while chunk 7 computes ----
                    nc.vector.tensor_reduce(
                        vBm8[:, 0:NCHUNK - 1],
                        mnB[:, 0:(NCHUNK - 1) * JPC].rearrange(
                            "p (c j) -> p c j", c=NCHUNK - 1),
                        AX.X, OP.min)
                    vBbE = work.tile([P, NCHUNK - 1], BF16, name="vBbE")
                    nc.vector.tensor_copy(vBbE, vBm8[:, 0:NCHUNK - 1])
                    vfdE = dpool.tile([1, NE], BF16, name="vfdE")
                    nc.sync.dma_start(
                        out=vfdE[0:1, :].rearrange("o (p c) -> o p c", p=P),
                        in_=vBbE)
                    vfdE_b = bass.AP(tensor=vfdE.tensor, offset=vfdE.offset,
                                     ap=[[0, P], vfdE.ap[-1]])
                    nc.sync.dma_start(out=vrepE, in_=vfdE_b)
                    for cc in range(NCHUNK - 1):
                        def mk_rank(cc=cc):
                            def go():
                                nc.vector.tensor_scalar(
                                    waste1k[:, 0:NE], vrepE,
                                    vBm8[:, cc:cc + 1], 0.0,
                                    OP.is_lt, OP.add,
                                    accum_out=rankE[:, cc:cc + 1])
                            return go
                        pend_rank.append(mk_rank())
                lc = lhsT[:, c * P:(c + 1) * P]
                for j in range(JPC):
                    if b == 1 and c == NCHUNK - 1 and j >= 2:
                        flush(pend_rank, keep=(4 if j == 2 else 0))
                    k = c * JPC + j
                    route = ROUTES[b][k]
                    ps = psum.tile([P, MTILE], F32, name="ps", tag="ps")
                    for q in range(MTILE // MMF):
                        nc.tensor.matmul(
                            ps[:, q * MMF:(q + 1) * MMF], lc,
                            rhs[:, j * MTILE + q * MMF:
                                j * MTILE + (q + 1) * MMF],
                            start=True, stop=True)
                    dc = dcp.tile([P, MTILE], BF16, name="dc", tag="dc")
                    nc.scalar.activation(dc, ps, AF.Sqrt)
                    # w = min(dc, 3); accum(min) -> rowmin (attract)
                    w = wp.tile([P, MTILE], BF16, name="w", tag="w")
                    nc.vector.tensor_scalar(
                        w, dc, CLASH, 3.4e38, OP.min, OP.min,
                        accum_out=mnB[:, k:k + 1])
                    if route == "A":
                        def mk_act(w=w, sw2B=sw2B, k=k):
                            def go():
                                wa = wap.tile([P, MTILE], BF16, name="wa",
                                              tag="wa")
                                nc.scalar.activation(
                                    wa, w, AF.Square, bias=neg3[:, 0:1],
                                    scale=1.0, accum_out=sw2B[:, k:k + 1])
                            return go
                        pend_act.append(mk_act())
                    else:
                        # vc = w - 3 = -relu(3-d)
                        pool_ = vcp if route == "P" else vcd
                        vc = pool_.tile([P, MTILE], BF16, name="vc", tag="vc")
                        nc.vector.tensor_scalar(
                            vc, w, CLASH, 0.0, OP.subtract, OP.min)
                        if route == "P":
                            v2 = v2pp.tile([P, MTILE], BF16, name="v2",
                                           tag="v2")
                            nc.gpsimd.tensor_tensor(v2, vc, vc, OP.mult)
                            def mk_sum(v2=v2, sw2B=sw2B, k=k):
                                def go():
                                    nc.vector.tensor_scalar(
                                        v2, v2, 3.4e38, 0.0, OP.min, OP.add,
                                        accum_out=sw2B[:, k:k + 1])
                                return go
                            pend_sum.append(mk_sum())
                        else:
                            v2 = v2d.tile([P, MTILE], BF16, name="v2",
                                          tag="v2")
                            nc.vector.tensor_tensor(v2, vc, vc, OP.mult)
                            nc.vector.tensor_scalar(
                                v2, v2, 3.4e38, 0.0, OP.min, OP.add,
                                accum_out=sw2B[:, k:k + 1])
                    flush(pend_act, keep=5)
                    flush(pend_sum, keep=9)
                if c == 1:
                    flush(pend_epi)
            flush(pend_act)
            flush(pend_sum)

            # ---- per-batch epilogue, part 1 ----
            stack2 = work.tile([P, 2], F32, name=f"stack2{b}")
            nc.vector.tensor_reduce(stack2[:, 1:2], sw2B, AX.X, OP.add)
            vrep = None
            if b == 0:
                vBm = work.tile([P, NCHUNK], F32, name=f"vBm{b}")
                nc.vector.tensor_reduce(
                    vBm, mnB.rearrange("p (c j) -> p c j", c=NCHUNK),
                    AX.X, OP.min)
                # rank selection ignores element order, so flatten [128, 8]
                # partition-major straight to DRAM and broadcast-read it
                # back with a zero partition stride.
                vBb = work.tile([P, NCHUNK], BF16, name=f"vBb{b}")
                nc.vector.tensor_copy(vBb, vBm)
                vfd = dpool.tile([1, N], BF16, name=f"vfd{b}")
                nc.sync.dma_start(
                    out=vfd[0:1, :].rearrange("o (p c) -> o p c", p=P),
                    in_=vBb)
                vrep = work.tile([P, N], BF16, name=f"vrep{b}")
                vfd_bcast = bass.AP(tensor=vfd.tensor, offset=vfd.offset,
                                    ap=[[0, P], vfd.ap[-1]])
                nc.sync.dma_start(out=vrep, in_=vfd_bcast)
            else:
                vBm = vBm8

            # part 2: rank selection + final combine; deferred into the
            # next batch's main loop so the DMA latency above is hidden
            def mk_epi(b=b, vBm=vBm, vrep=vrep, stack2=stack2,
                       vBm8=vBm8, rankE=rankE, vrepE=vrepE):
                def go():
                    rank8 = work.tile([P, NCHUNK], F32, name=f"rank8{b}")
                    if b == 0:
                        for c in range(NCHUNK):
                            nc.vector.tensor_scalar(
                                waste1k, vrep, vBm[:, c:c + 1], 0.0,
                                OP.is_lt, OP.add, accum_out=rank8[:, c:c + 1])
                    else:
                        NE = (NCHUNK - 1) * P
                        # chunk 7 rowmins land in vBm8[:, 7]
                        nc.vector.tensor_reduce(
                            vBm8[:, NCHUNK - 1:NCHUNK],
                            mnB[:, (NCHUNK - 1) * JPC:], AX.X, OP.min)
                        # its rank vs chunks 0-6
                        nc.vector.tensor_scalar(
                            waste1k[:, 0:NE], vrepE,
                            vBm8[:, NCHUNK - 1:NCHUNK], 0.0,
                            OP.is_lt, OP.add,
                            accum_out=rankE[:, NCHUNK - 1:NCHUNK])
                        # chunk-7 j-side: PE transpose [128,1]->[1,128]
                        # (PSUM is free now) + gpsimd broadcast
                        vtp = psum.tile([1, P], F32, name="vtp", tag="ps")
                        nc.tensor.transpose(vtp, vBm8[:, NCHUNK - 1:NCHUNK],
                                            ident)
                        vfl2 = work.tile([1, P], F32, name="vfl2")
                        nc.vector.tensor_copy(vfl2, vtp)
                        vrep2 = work.tile([P, P], F32, name="vrep2")
                        nc.gpsimd.partition_broadcast(vrep2, vfl2[0:1, :], P)
                        rankL = work.tile([P, NCHUNK], F32, name="rankL")
                        for c in range(NCHUNK):
                            nc.vector.tensor_scalar(
                                waste1k[:, 0:P], vrep2, vBm8[:, c:c + 1],
                                0.0, OP.is_lt, OP.add,
                                accum_out=rankL[:, c:c + 1])
                        nc.vector.tensor_add(rank8, rankE, rankL)
                    sel8 = work.tile([P, NCHUNK], F32, name=f"sel8{b}")
                    nc.vector.tensor_scalar(sel8, rank8, float(TOPK), None,
                                            OP.is_lt)
                    prod8 = work.tile([P, NCHUNK], F32, name=f"prod8{b}")
                    nc.vector.tensor_mul(prod8, sel8, vBm)
                    # sum of selected (rowmin-3); add 3 per selected row
                    # via the count
                    nc.vector.tensor_reduce(stack2[:, 0:1], prod8, AX.X,
                                            OP.add)
                    cnt8 = work.tile([P, 1], F32, name=f"cnt8{b}")
                    nc.vector.tensor_reduce(cnt8, sel8, AX.X, OP.add)
                    st3 = work.tile([P, 3], F32, name=f"st3{b}")
                    nc.vector.tensor_copy(st3[:, 0:2], stack2)
                    nc.vector.tensor_copy(st3[:, 2:3], cnt8)

                    # PE ones-matmul + final combine: deferred to after the
                    # main loops, when PE/PSUM are idle (a mid-stream fin
                    # matmul head-of-line-blocks the PE FIFO)
                    def fin_go(b=b, st3=st3):
                        finp = psum.tile([1, 3], F32, name="finp", tag="ps")
                        nc.tensor.matmul(finp, ones128, st3, start=True,
                                         stop=True)
                        fin = work.tile([1, 2], F32, name=f"fin{b}")
                        nc.vector.tensor_copy(fin, finp[0:1, 0:2])
                        fcnt = work.tile([1, 1], F32, name=f"fcnt{b}")
                        nc.vector.tensor_copy(fcnt, finp[0:1, 2:3])
                        en = work.tile([1, 2], F32, name=f"en{b}")
                        nc.vector.tensor_scalar(
                            en[0:1, 0:1], fcnt[0:1, 0:1],
                            CLASH * ATTRACT_SCALE / TOPK, None, OP.mult)
                        t0 = work.tile([1, 1], F32, name=f"t0{b}")
                        nc.vector.tensor_scalar(t0, fin[0:1, 0:1],
                                                ATTRACT_SCALE / TOPK, None,
                                                OP.mult)
                        nc.vector.tensor_scalar(en[0:1, 1:2], fin[0:1, 1:2],
                                                REPEL_SCALE, None, OP.mult)
                        en2 = work.tile([1, 1], F32, name=f"en2{b}")
                        nc.vector.tensor_add(en2, en[0:1, 0:1], en[0:1, 1:2])
                        en3 = work.tile([1, 1], F32, name=f"en3{b}")
                        nc.vector.tensor_add(en3, en2, t0)
                        nc.sync.dma_start(out=out[b:b + 1, 0:1],
                                          in_=en3[0:1, 0:1])
                    pend_fin.append(fin_go)
                return go
            pend_epi.append(mk_epi())
        flush(pend_epi)
        flush(pend_fin)

    nc.compile()
    return nc


def _get_program():
    if "nc" not in _prog_cache:
        _prog_cache["nc"] = build_program()
    return _prog_cache["nc"]


def make_in_maps(binder_trans, target_coords):
    x = np.asarray(binder_trans, dtype=np.float32)          # [B, N, 3]
    y = np.asarray(target_coords, dtype=np.float32)         # [M, 3]
    # lhsT per batch: [x0 x1 x2 ; ||x||^2 ; 1]  -> [B, 5, N]
    xT = x.transpose(0, 2, 1)                               # [B, 3, N]
    xsq = (x * x).sum(-1)[:, None, :]                       # [B, 1, N]
    ones_n = np.ones((B, 1, N), dtype=np.float32)
    lhs = np.ascontiguousarray(
        np.concatenate([xT, xsq, ones_n], axis=1))          # [B, 5, N]
    # rhs: [-2y0 -2y1 -2y2 ; 1 ; sum(y^2)+eps] -> [5, M]
    yT = -2.0 * y.T                                         # [3, M]
    ones_m = np.ones((1, MT), dtype=np.float32)
    ysq = (y * y).sum(-1)[None, :] + np.float32(EPS)        # [1, M]
    rhs = np.ascontiguousarray(
        np.concatenate([yT, ones_m, ysq], axis=0))          # [5, M]
    return [{"bnd": np.ascontiguousarray(lhs[c * BC:(c + 1) * BC]),
             "tgt": rhs}
            for c in range(NCORES)]


def kernel(binder_trans, target_coords):
    nc = _get_program()
    in_maps = make_in_maps(binder_trans, target_coords)
    res = run_bass_kernel_spmd(nc, in_maps, list(range(NCORES)))
    outs = [np.asarray(res.results[c]["out"], dtype=np.float32).reshape(BC)
            for c in range(NCORES)]
    return np.concatenate(outs).astype(np.float32)


# revision 12
# speedup vs baseline: 1.9244x; 1.0044x over previous
"""Trainium2 Bass kernel for BinderEnergyGuidance (retrieval_knn).

Math (per batch b of 16):
  d[b,n,m]   = ||binder[b,n] - target[m]||           (N=1024, M=8192)
  attract[b] = mean of the k=204 smallest per-row min-distances
  repel[b]   = sum relu(3 - d)^2
  out[b]     = 10*attract[b] + 5*repel[b]

Strategy: data-parallel over batch, 2 batches per core.  Host packs the
inputs so the device contraction is K=5:
  lhsT rows: [x0 x1 x2 ; ||x||^2 ; 1]        (per batch, [5, N])
  rhs  rows: [-2y0 -2y1 -2y2 ; 1 ; sum(y^2)+eps]   ([5, M])
so one fp32r matmul emits d2 + eps directly into PSUM.

Per 2048-wide m-tile (64 per core):
  PE : 4x 512-col matmuls -> PSUM fp32 d2 (2 PSUM tiles = all 8 banks)
  Act: dc = Sqrt(d2) PSUM->SBUF bf16 (eps pre-added keeps d2 > 0)
  DVE tsA: w = min(dc,3), accum(min) -> per-row min dist (attract).
  repel integrand (3-w)^2, summed per row, via 3 balanced routes:
    'A': Act Square(w - 3) + accum          (Act only)
    'D': vc = w-3 (ts); vc*vc (tt); ts accum (DVE only)
    'P': vc = w-3 (ts); vc*vc on gpsimd tt; ts accum (Pool + DVE)
  Route counts chosen so Act/DVE/Pool all land at ~140us busy.

Note: tensor_scalar with accum_out applies only op0 to the written
output; op1 becomes the accumulation op.  All op1 choices here are
no-ops on the output.

Epilogue (per batch): rowmins [128,8] -> +3 -> bf16 pad to [128,128] ->
XBAR DMA transpose -> flatten -> gpsimd partition_broadcast -> top-k by
rank (count of strictly-smaller values) -> select/dot; final partition
sums via gpsimd partition_all_reduce (no PSUM use outside the matmuls).

Self-contained: hardcodes shapes for binder[16,1024,3], target[8192,3].
"""

import numpy as np
from contextlib import ExitStack

import concourse.bass as bass
import concourse.bacc as bacc
import concourse.tile as tile
from concourse import mybir, bass_isa
from concourse.masks import make_identity
from concourse.bass_utils import run_bass_kernel_spmd

F32 = mybir.dt.float32
F32R = mybir.dt.float32r
BF16 = mybir.dt.bfloat16
AF = mybir.ActivationFunctionType
OP = mybir.AluOpType
AX = mybir.AxisListType
RED = bass_isa.ReduceOp

B, N, MT = 16, 1024, 8192
NCORES = 8
BC = B // NCORES          # batches per core
TOPK = 204                # int(0.2 * N)
CLASH = 3.0
EPS = 1e-3                # guards sqrt against fp-rounding-negative d2
ATTRACT_SCALE, REPEL_SCALE = 10.0, 5.0

P = 128                   # SBUF partitions
NCHUNK = N // P           # 8 row-chunks per batch
MTILE = 2048              # PSUM tile free size (4 banks)
JPC = MT // MTILE         # m-tiles per chunk (4)
MMF = 512                 # fp32 matmul max moving free size
KP = 5                    # packed contraction size

# square-route assignment per (batch, m-tile) — 32 tiles per batch.
# 'P' gpsimd, 'A' scalar-engine Square, 'D' vector engine.
# Batch 1 keeps its last chunks gpsimd-free so Pool drains before the
# final epilogue instead of 25us after the last sqrt.


def _routes():
    b0 = []
    for k in range(32):
        if k % 2 == 0 and k != 0:
            b0.append("P")
        elif k in (0, 9, 25):
            b0.append("A")
        else:
            b0.append("D")
    b1 = []
    for k in range(32):
        if k < 20:
            b1.append("P" if (k % 5 != 4 and k not in (16, 18)) else "D")
        elif k < 24:
            b1.append("D")
        elif k < 30 or k == 31:
            b1.append("A")
        else:
            b1.append("D")
    return [b0, b1]


ROUTES = _routes()

_prog_cache = {}


def build_program():
    nc = bacc.Bacc("TRN2", target_bir_lowering=False, debug=False,
                   num_devices=NCORES)
    bnd = nc.dram_tensor("bnd", [BC, KP, N], F32R, kind="ExternalInput").ap()
    tgt = nc.dram_tensor("tgt", [KP, MT], F32R, kind="ExternalInput").ap()
    out = nc.dram_tensor("out", [BC, 1], F32, kind="ExternalOutput").ap()

    with tile.TileContext(nc) as tc, ExitStack() as ctx:
        consts = ctx.enter_context(tc.tile_pool(name="consts", bufs=1))
        work = ctx.enter_context(tc.tile_pool(name="work", bufs=1))
        dcp = ctx.enter_context(tc.tile_pool(name="dcp", bufs=8))
        wp = ctx.enter_context(tc.tile_pool(name="wp", bufs=6))
        vcd = ctx.enter_context(tc.tile_pool(name="vcd", bufs=3))
        vcp = ctx.enter_context(tc.tile_pool(name="vcp", bufs=7))
        wap = ctx.enter_context(tc.tile_pool(name="wap", bufs=3))
        v2d = ctx.enter_context(tc.tile_pool(name="v2d", bufs=3))
        v2pp = ctx.enter_context(tc.tile_pool(name="v2pp", bufs=9))
        psum = ctx.enter_context(tc.tile_pool(name="psum", bufs=2, space="PSUM"))
        dpool = ctx.enter_context(tc.tile_pool(name="dpool", bufs=1, space="DRAM"))

        # ---- inputs: pure DMA, no engine preamble.  lhsT0 + first rhs
        # chunk gate the first matmul, so issue them first on separate
        # queues. ----
        rhs = consts.tile([KP, MT], F32R)
        lhsTs = [consts.tile([KP, N], F32R, name=f"lhsT{b}") for b in range(BC)]
        nc.sync.dma_start(out=lhsTs[0][:, :], in_=bnd[0, :, :])
        nc.scalar.dma_start(out=rhs[:, 0:512], in_=tgt[:, 0:512])
        nc.sync.dma_start(out=rhs[:, 512:2048], in_=tgt[:, 512:2048])
        nc.scalar.dma_start(out=rhs[:, 2048:4096], in_=tgt[:, 2048:4096])
        nc.sync.dma_start(out=rhs[:, 4096:6144], in_=tgt[:, 4096:6144])
        nc.scalar.dma_start(out=rhs[:, 6144:8192], in_=tgt[:, 6144:8192])
        nc.sync.dma_start(out=lhsTs[1][:, :], in_=bnd[1, :, :])

        # shared elementwise-output scratch (values never read)
        waste1k = work.tile([P, N], BF16)
        neg3 = consts.tile([P, 1], F32)
        nc.vector.memset(neg3, -CLASH)
        ones128 = consts.tile([P, 1], F32)
        nc.vector.memset(ones128, 1.0)
        ident = consts.tile([P, P], F32)
        make_identity(nc, ident)

        # Engine queues are strict FIFO, so an op that waits on a slow
        # producer (gpsimd square, epilogue DMA chain) must not be emitted
        # right after it or it head-of-line-blocks the whole engine.
        # Deferred emission queues provide the lag.
        pend_sum = []     # P-route row-sum ts ops waiting for gpsimd
        pend_act = []     # A-route Square ops, lag one tile
        pend_epi = []     # previous batch's epilogue part 2
        pend_fin = []     # final partition-sum + output, all batches
        pend_rank = []    # b1 split-rank: early ranks vs chunks 0-6

        def flush(q, keep=0):
            while len(q) > keep:
                q.pop(0)()

        for b in range(BC):
            lhsT = lhsTs[b]
            sw2B = work.tile([P, NCHUNK * JPC], F32, name=f"sw2B{b}")
            mnB = work.tile([P, NCHUNK * JPC], F32, name=f"mnB{b}")

            NE = (NCHUNK - 1) * P          # 896: rows in chunks 0-6
            vBm8 = work.tile([P, NCHUNK], F32, name=f"vBm8{b}")
            rankE = work.tile([P, NCHUNK], F32, name=f"rankE{b}")
            vrepE = work.tile([P, NE], BF16, name=f"vrepE{b}")
            for c in range(NCHUNK):
                if b == 1 and c == NCHUNK - 1:
                    # ---- split-rank early piece: chunks 0-6 are final;
                    # flatten+broadcast their rowmins and rank against
                    # them while chunk 7 computes ----
                    nc.vector.tensor_reduce(
                        vBm8[:, 0:NCHUNK - 1],
                        mnB[:, 0:(NCHUNK - 1) * JPC].rearrange(
                            "p (c j) -> p c j", c=NCHUNK - 1),
                        AX.X, OP.min)
                    vBbE = work.tile([P, NCHUNK - 1], BF16, name="vBbE")
                    nc.vector.tensor_copy(vBbE, vBm8[:, 0:NCHUNK - 1])
                    vfdE = dpool.tile([1, NE], BF16, name="vfdE")
                    nc.sync.dma_start(
                        out=vfdE[0:1, :].rearrange("o (p c) -> o p c", p=P),
                        in_=vBbE)
                    vfdE_b = bass.AP(tensor=vfdE.tensor, offset=vfdE.offset,
                                     ap=[[0, P], vfdE.ap[-1]])
                    nc.sync.dma_start(out=vrepE, in_=vfdE_b)
                    for cc in range(NCHUNK - 1):
                        def mk_rank(cc=cc):
                            def go():
                                nc.vector.tensor_scalar(
                                    waste1k[:, 0:NE], vrepE,
                                    vBm8[:, cc:cc + 1], 0.0,
                                    OP.is_lt, OP.add,
                                    accum_out=rankE[:, cc:cc + 1])
                            return go
                        pend_rank.append(mk_rank())
                lc = lhsT[:, c * P:(c + 1) * P]
                for j in range(JPC):
                    if b == 1 and c == NCHUNK - 1 and j >= 2:
                        flush(pend_rank, keep=(4 if j == 2 else 0))
                    k = c * JPC + j
                    route = ROUTES[b][k]
                    ps = psum.tile([P, MTILE], F32, name="ps", tag="ps")
                    for q in range(MTILE // MMF):
                        nc.tensor.matmul(
                            ps[:, q * MMF:(q + 1) * MMF], lc,
                            rhs[:, j * MTILE + q * MMF:
                                j * MTILE + (q + 1) * MMF],
                            start=True, stop=True)
                    dc = dcp.tile([P, MTILE], BF16, name="dc", tag="dc")
                    nc.scalar.activation(dc, ps, AF.Sqrt)
                    # w = min(dc, 3); accum(min) -> rowmin (attract)
                    w = wp.tile([P, MTILE], BF16, name="w", tag="w")
                    nc.vector.tensor_scalar(
                        w, dc, CLASH, 3.4e38, OP.min, OP.min,
                        accum_out=mnB[:, k:k + 1])
                    if route == "A":
                        def mk_act(w=w, sw2B=sw2B, k=k):
                            def go():
                                wa = wap.tile([P, MTILE], BF16, name="wa",
                                              tag="wa")
                                nc.scalar.activation(
                                    wa, w, AF.Square, bias=neg3[:, 0:1],
                                    scale=1.0, accum_out=sw2B[:, k:k + 1])
                            return go
                        pend_act.append(mk_act())
                    else:
                        # vc = w - 3 = -relu(3-d)
                        pool_ = vcp if route == "P" else vcd
                        vc = pool_.tile([P, MTILE], BF16, name="vc", tag="vc")
                        nc.vector.tensor_scalar(
                            vc, w, CLASH, 0.0, OP.subtract, OP.min)
                        if route == "P":
                            v2 = v2pp.tile([P, MTILE], BF16, name="v2",
                                           tag="v2")
                            nc.gpsimd.tensor_tensor(v2, vc, vc, OP.mult)
                            def mk_sum(v2=v2, sw2B=sw2B, k=k):
                                def go():
                                    nc.vector.tensor_scalar(
                                        v2, v2, 3.4e38, 0.0, OP.min, OP.add,
                                        accum_out=sw2B[:, k:k + 1])
                                return go
                            pend_sum.append(mk_sum())
                        else:
                            v2 = v2d.tile([P, MTILE], BF16, name="v2",
                                          tag="v2")
                            nc.vector.tensor_tensor(v2, vc, vc, OP.mult)
                            nc.vector.tensor_scalar(
                                v2, v2, 3.4e38, 0.0, OP.min, OP.add,
                                accum_out=sw2B[:, k:k + 1])
                    flush(pend_act, keep=5)
                    flush(pend_sum, keep=9)
                if c == 1:
                    flush(pend_epi)
            flush(pend_act)
            flush(pend_sum)

            # ---- per-batch epilogue, part 1 ----
            stack2 = work.tile([P, 2], F32, name=f"stack2{b}")
            nc.vector.tensor_reduce(stack2[:, 1:2], sw2B, AX.X, OP.add)
            vrep = None
            if b == 0:
                vBm = work.tile([P, NCHUNK], F32, name=f"vBm{b}")
                nc.vector.tensor_reduce(
                    vBm, mnB.rearrange("p (c j) -> p c j", c=NCHUNK),
                    AX.X, OP.min)
                # rank selection ignores element order, so flatten [128, 8]
                # partition-major straight to DRAM and broadcast-read it
                # back with a zero partition stride.
                vBb = work.tile([P, NCHUNK], BF16, name=f"vBb{b}")
                nc.vector.tensor_copy(vBb, vBm)
                vfd = dpool.tile([1, N], BF16, name=f"vfd{b}")
                nc.sync.dma_start(
                    out=vfd[0:1, :].rearrange("o (p c) -> o p c", p=P),
                    in_=vBb)
                vrep = work.tile([P, N], BF16, name=f"vrep{b}")
                vfd_bcast = bass.AP(tensor=vfd.tensor, offset=vfd.offset,
                                    ap=[[0, P], vfd.ap[-1]])
                nc.sync.dma_start(out=vrep, in_=vfd_bcast)
            else:
                vBm = vBm8

            # part 2: rank selection + final combine; deferred into the
            # next batch's main loop so the DMA latency above is hidden
            def mk_epi(b=b, vBm=vBm, vrep=vrep, stack2=stack2,
                       vBm8=vBm8, rankE=rankE, vrepE=vrepE):
                def go():
                    rank8 = work.tile([P, NCHUNK], F32, name=f"rank8{b}")
                    if b == 0:
                        for c in range(NCHUNK):
                            nc.vector.tensor_scalar(
                                waste1k, vrep, vBm[:, c:c + 1], 0.0,
                                OP.is_lt, OP.add, accum_out=rank8[:, c:c + 1])
                    else:
                        NE = (NCHUNK - 1) * P
                        # chunk 7 rowmins land in vBm8[:, 7]
                        nc.vector.tensor_reduce(
                            vBm8[:, NCHUNK - 1:NCHUNK],
                            mnB[:, (NCHUNK - 1) * JPC:], AX.X, OP.min)
                        # its rank vs chunks 0-6
                        nc.vector.tensor_scalar(
                            waste1k[:, 0:NE], vrepE,
                            vBm8[:, NCHUNK - 1:NCHUNK], 0.0,
                            OP.is_lt, OP.add,
                            accum_out=rankE[:, NCHUNK - 1:NCHUNK])
                        # chunk-7 j-side: PE transpose [128,1]->[1,128]
                        # (PSUM is free now) + gpsimd broadcast
                        vtp = psum.tile([1, P], F32, name="vtp", tag="ps")
                        nc.tensor.transpose(vtp, vBm8[:, NCHUNK - 1:NCHUNK],
                                            ident)
                        vfl2 = work.tile([1, P], F32, name="vfl2")
                        nc.vector.tensor_copy(vfl2, vtp)
                        vrep2 = work.tile([P, P], F32, name="vrep2")
                        nc.gpsimd.partition_broadcast(vrep2, vfl2[0:1, :], P)
                        rankL = work.tile([P, NCHUNK], F32, name="rankL")
                        for c in range(NCHUNK):
                            nc.vector.tensor_scalar(
                                waste1k[:, 0:P], vrep2, vBm8[:, c:c + 1],
                                0.0, OP.is_lt, OP.add,
                                accum_out=rankL[:, c:c + 1])
                        nc.vector.tensor_add(rank8, rankE, rankL)
                    sel8 = work.tile([P, NCHUNK], F32, name=f"sel8{b}")
                    nc.vector.tensor_scalar(sel8, rank8, float(TOPK), None,
                                            OP.is_lt)
                    prod8 = work.tile([P, NCHUNK], F32, name=f"prod8{b}")
                    nc.vector.tensor_mul(prod8, sel8, vBm)
                    # sum of selected (rowmin-3); add 3 per selected row
                    # via the count
                    nc.vector.tensor_reduce(stack2[:, 0:1], prod8, AX.X,
                                            OP.add)
                    cnt8 = work.tile([P, 1], F32, name=f"cnt8{b}")
                    nc.vector.tensor_reduce(cnt8, sel8, AX.X, OP.add)
                    st3 = work.tile([P, 3], F32, name=f"st3{b}")
                    nc.vector.tensor_copy(st3[:, 0:2], stack2)
                    nc.vector.tensor_copy(st3[:, 2:3], cnt8)

                    # PE ones-matmul + final combine: deferred to after the
                    # main loops, when PE/PSUM are idle (a mid-stream fin
                    # matmul head-of-line-blocks the PE FIFO)
                    def fin_go(b=b, st3=st3):
                        finp = psum.tile([1, 3], F32, name="finp", tag="ps")
                        nc.tensor.matmul(finp, ones128, st3, start=True,
                                         stop=True)
                        fin = work.tile([1, 2], F32, name=f"fin{b}")
                        nc.vector.tensor_copy(fin, finp[0:1, 0:2])
                        fcnt = work.tile([1, 1], F32, name=f"fcnt{b}")
                        nc.vector.tensor_copy(fcnt, finp[0:1, 2:3])
                        en = work.tile([1, 2], F32, name=f"en{b}")
                        nc.vector.tensor_scalar(
                            en[0:1, 0:1], fcnt[0:1, 0:1],
                            CLASH * ATTRACT_SCALE / TOPK, None, OP.mult)
                        t0 = work.tile([1, 1], F32, name=f"t0{b}")
                        nc.vector.tensor_scalar(t0, fin[0:1, 0:1],
                                                ATTRACT_SCALE / TOPK, None,
                                                OP.mult)
                        nc.vector.tensor_scalar(en[0:1, 1:2], fin[0:1, 1:2],
                                                REPEL_SCALE, None, OP.mult)
                        en2 = work.tile([1, 1], F32, name=f"en2{b}")
                        nc.vector.tensor_add(en2, en[0:1, 0:1], en[0:1, 1:2])
                        en3 = work.tile([1, 1], F32, name=f"en3{b}")
                        nc.vector.tensor_add(en3, en2, t0)
                        nc.sync.dma_start(out=out[b:b + 1, 0:1],
                                          in_=en3[0:1, 0:1])
                    pend_fin.append(fin_go)
                return go
            pend_epi.append(mk_epi())
        flush(pend_epi)
        flush(pend_fin)

    nc.compile()
    return nc


def _get_program():
    if "nc" not in _prog_cache:
        _prog_cache["nc"] = build_program()
    return _prog_cache["nc"]


def make_in_maps(binder_trans, target_coords):
    x = np.asarray(binder_trans, dtype=np.float32)          # [B, N, 3]
    y = np.asarray(target_coords, dtype=np.float32)         # [M, 3]
    # lhsT per batch: [x0 x1 x2 ; ||x||^2 ; 1]  -> [B, 5, N]
    xT = x.transpose(0, 2, 1)                               # [B, 3, N]
    xsq = (x * x).sum(-1)[:, None, :]                       # [B, 1, N]
    ones_n = np.ones((B, 1, N), dtype=np.float32)
    lhs = np.ascontiguousarray(
        np.concatenate([xT, xsq, ones_n], axis=1))          # [B, 5, N]
    # rhs: [-2y0 -2y1 -2y2 ; 1 ; sum(y^2)+eps] -> [5, M]
    yT = -2.0 * y.T                                         # [3, M]
    ones_m = np.ones((1, MT), dtype=np.float32)
    ysq = (y * y).sum(-1)[None, :] + np.float32(EPS)        # [1, M]
    rhs = np.ascontiguousarray(
        np.concatenate([yT, ones_m, ysq], axis=0))          # [5, M]
    return [{"bnd": np.ascontiguousarray(lhs[c * BC:(c + 1) * BC]),
             "tgt": rhs}
            for c in range(NCORES)]


def kernel(binder_trans, target_coords):
    nc = _get_program()
    in_maps = make_in_maps(binder_trans, target_coords)
    res = run_bass_kernel_spmd(nc, in_maps, list(range(NCORES)))
    outs = [np.asarray(res.results[c]["out"], dtype=np.float32).reshape(BC)
            for c in range(NCORES)]
    return np.concatenate(outs).astype(np.float32)
